# revision 1
# baseline (speedup 1.0000x reference)
"""MixtureOfBlockAttention TRN2 kernel — 8-core head-parallel (TP) Bass/Tile implementation.

Semantics (verified equivalent to the reference, rel err ~3e-6 in fp32):
the reference mask `maximum(token_mask, causal*NEG_INF)` masks a position iff
it is BOTH future AND in a non-selected block. Consequences:
  - query blocks 0..7 attend to ALL tokens of key blocks 0..7 (dense, no mask);
  - query block i>=8 attends densely to key blocks 0..i-1, and within its own
    (diagonal) block applies strict causal masking ONLY for rows whose own
    block is not among their top-8 gating blocks.
Selection rank for query s in block i (i>=8): own block selected iff
  #{j < i : g[s,j] > g[s,i]} < 8, with g = q . (block sums of roped k)
(positive-scale invariant, so block sums replace means and the 1/sqrt(d)
factor is dropped).

Sharding: 16 query heads / 8 cores = 2 heads per core; KV head c serves both.
wq/wk/wv column-sliced, wo row-sliced; partial outputs summed on host.
Host-side layout prep: x is transposed to xT[c, s] (the PE contracts over the
partition dim, so both matmul operands need c on partitions) and float inputs
are pre-rounded to fp32r; both are pure data-layout transforms.

All big matmuls run in float32r (TF32-like input rounding, fp32 accumulate,
full PE rate at N>=256).
"""

import math
import sys

import numpy as np

if "/opt/trn_rl_repo" not in sys.path:
    sys.path.insert(0, "/opt/trn_rl_repo")

import concourse.bacc as bacc
import concourse.mybir as mybir
import concourse.tile as tile
from concourse.bass_utils import run_bass_kernel_spmd
from concourse.masks import make_identity

F32 = mybir.dt.float32
F32R = mybir.dt.float32r

SEQ = 4096
DIM = 2048
HEAD_DIM = 128
N_HEADS = 16
N_CORES = 8
HPC = N_HEADS // N_CORES       # heads per core = 2
DPC = HPC * HEAD_DIM           # q/o dims per core = 256
BLOCK = 128
NB = SEQ // BLOCK              # 32 key blocks
TOPK = 8
NCHUNK = 8                     # s-chunks of 512
CH = SEQ // NCHUNK             # 512
NCT = DIM // 128               # 16 contraction tiles
INV_SQRT_D = 1.0 / math.sqrt(HEAD_DIM)

_CACHE = {}


def _round_fp32r(a):
    """Round fp32 to the fp32r grid (top-11-bit mantissa, round-to-nearest)."""
    a = np.ascontiguousarray(a, dtype=np.float32)
    try:
        from neuron_dtypes import static_cast_fp32_to_fp32r

        return static_cast_fp32_to_fp32r(a).view(np.float32).astype(np.float32)
    except Exception:
        u = a.view(np.uint32)
        return ((u + np.uint32(0x800)) & np.uint32(0xFFFFF000)).view(np.float32).copy()


def _host_constants():
    if "consts" in _CACHE:
        return _CACHE["consts"]
    p = np.arange(HEAD_DIM // 2, dtype=np.float64)
    inv_freq = 1.0 / (10000.0 ** (2.0 * p / HEAD_DIM))
    ang = np.arange(SEQ, dtype=np.float64)[None, :] * inv_freq[:, None]  # [64, S]
    cos = np.cos(ang).astype(np.float32)
    sin = np.sin(ang).astype(np.float32)
    cos_ds = np.ascontiguousarray(np.repeat(cos, 2, axis=0))   # [128, S]
    sin_ds = np.empty((HEAD_DIM, SEQ), dtype=np.float32)       # signed sin
    sin_ds[0::2] = -sin
    sin_ds[1::2] = sin
    pswap = np.zeros((128, 128), dtype=np.float32)             # swap 2p <-> 2p+1
    idx = np.arange(128)
    pswap[idx, idx ^ 1] = 1.0
    r = np.arange(BLOCK)
    trikeep = (r[:, None] <= r[None, :]).astype(np.float32)    # keep iff sk <= sq
    ones_col = np.ones((128, 1), dtype=np.float32)
    ones_row = np.ones((1, 128), dtype=np.float32)
    _CACHE["consts"] = (cos_ds, sin_ds, pswap, trikeep, ones_col, ones_row)
    return _CACHE["consts"]


def make_in_maps(x, wq, wk, wv, wo):
    """Shard + lay out the full inputs for the 8 cores."""
    x2 = np.asarray(x, dtype=np.float32).reshape(SEQ, DIM)
    xT = _round_fp32r(np.ascontiguousarray(x2.T))
    wq = np.asarray(wq, dtype=np.float32)
    wk = np.asarray(wk, dtype=np.float32)
    wv = np.asarray(wv, dtype=np.float32)
    wo = np.asarray(wo, dtype=np.float32)
    cos_ds, sin_ds, pswap, trikeep, ones_col, ones_row = _host_constants()
    pswap_r = _round_fp32r(pswap)
    ones_col_r = _round_fp32r(ones_col)
    ones_row_r = _round_fp32r(ones_row)
    in_maps = []
    for c in range(N_CORES):
        in_maps.append(
            {
                "xT": xT,
                "wq": _round_fp32r(wq[:, c * DPC:(c + 1) * DPC]),
                "wk": _round_fp32r(wk[:, c * HEAD_DIM:(c + 1) * HEAD_DIM]),
                "wv": _round_fp32r(wv[:, c * HEAD_DIM:(c + 1) * HEAD_DIM]),
                "wo": _round_fp32r(wo[c * DPC:(c + 1) * DPC, :]),
                "cos_ds": cos_ds,
                "sin_ds": sin_ds,
                "pswap": pswap_r,
                "trikeep": trikeep,
                "ones_c": ones_col_r,
                "ones_r": ones_row_r,
            }
        )
    return in_maps


def _build_nc(reps=1):
    key = f"nc{reps}"
    if key in _CACHE:
        return _CACHE[key]
    nc = bacc.Bacc(None, target_bir_lowering=False)

    xT_d = nc.dram_tensor("xT", [DIM, SEQ], F32R, kind="ExternalInput")
    wq_d = nc.dram_tensor("wq", [DIM, DPC], F32R, kind="ExternalInput")
    wk_d = nc.dram_tensor("wk", [DIM, HEAD_DIM], F32R, kind="ExternalInput")
    wv_d = nc.dram_tensor("wv", [DIM, HEAD_DIM], F32R, kind="ExternalInput")
    wo_d = nc.dram_tensor("wo", [DPC, DIM], F32R, kind="ExternalInput")
    cos_d = nc.dram_tensor("cos_ds", [HEAD_DIM, SEQ], F32, kind="ExternalInput")
    sin_d = nc.dram_tensor("sin_ds", [HEAD_DIM, SEQ], F32, kind="ExternalInput")
    psw_d = nc.dram_tensor("pswap", [128, 128], F32R, kind="ExternalInput")
    trk_d = nc.dram_tensor("trikeep", [BLOCK, BLOCK], F32, kind="ExternalInput")
    onc_d = nc.dram_tensor("ones_c", [128, 1], F32R, kind="ExternalInput")
    onr_d = nc.dram_tensor("ones_r", [1, 128], F32R, kind="ExternalInput")
    out_d = nc.dram_tensor("out", [SEQ, DIM], F32, kind="ExternalOutput")

    with tile.TileContext(nc) as tc, nc.allow_low_precision(
        reason="float32r rounding of matmul operands is intentional"
    ):
      for _rep in range(reps):
        with tc.tile_pool(name="persist", bufs=1) as per:
            qT = [per.tile([128, SEQ], F32R, tag=f"qT{h}", name=f"qT{h}") for h in range(HPC)]
            kT = per.tile([128, SEQ], F32R, tag="kT")
            vN = per.tile([128, NB, 128], F32R, tag="vN")   # [s-in-tile, sk-tile, d]
            ident = per.tile([128, 128], F32, tag="ident")
            pswap = per.tile([128, 128], F32R, tag="pswap")
            trik = per.tile([BLOCK, BLOCK], F32, tag="trik")
            ones_c = per.tile([128, 1], F32R, tag="ones_c")
            ones_r = per.tile([1, 128], F32R, tag="ones_r")
            bm = per.tile([128, NB], F32R, tag="bm")
            # per-head notflag rows: Ft[h][0, (i-TOPK)*128:...] is the [1,128]
            # notflag row for query block i, at base partition 0
            Ft = [
                per.tile([1, (NB - TOPK) * 128], F32R, tag=f"Ft{h}", name=f"Ft{h}")
                for h in range(HPC)
            ]

            make_identity(nc, ident)
            nc.gpsimd.dma_start(out=pswap, in_=psw_d[:])
            nc.gpsimd.dma_start(out=trik, in_=trk_d[:])
            nc.gpsimd.dma_start(out=ones_c, in_=onc_d[:])
            nc.gpsimd.dma_start(out=ones_r, in_=onr_d[:])

            # ---------------- phase 1: projections + rope -------------------
            with (
                tc.tile_pool(name="wpool", bufs=1) as wp,
                tc.tile_pool(name="xtp", bufs=10) as xtp,
                tc.tile_pool(name="ropep", bufs=2) as rp,
                tc.tile_pool(name="csin", bufs=2) as csp,
                tc.tile_pool(name="pj_ps", bufs=2, space="PSUM") as trps,
                tc.tile_pool(name="acc_ps", bufs=4, space="PSUM") as accps,
            ):
                wq_sb = wp.tile([128, NCT, DPC], F32R, tag="wq")
                wk_sb = wp.tile([128, NCT, HEAD_DIM], F32R, tag="wk")
                wv_sb = wp.tile([128, NCT, HEAD_DIM], F32R, tag="wv")
                wq_r = wq_d.rearrange("(t p) d -> p t d", p=128)
                nc.gpsimd.dma_start(out=wq_sb[:, 0:4, :], in_=wq_r[:, 0:4, :])
                nc.gpsimd.dma_start(out=wq_sb[:, 4:16, :], in_=wq_r[:, 4:16, :])
                nc.gpsimd.dma_start(
                    out=wk_sb, in_=wk_d.rearrange("(t p) d -> p t d", p=128)
                )
                nc.gpsimd.dma_start(
                    out=wv_sb, in_=wv_d.rearrange("(t p) d -> p t d", p=128)
                )

                gp = wp  # reuse the bufs=1 pool scope for small gating tiles
                for m in range(NCHUNK):
                    cols = slice(m * CH, (m + 1) * CH)
                    ps_q0 = accps.tile([128, CH], F32, tag="acc")
                    ps_q1 = accps.tile([128, CH], F32, tag="acc")
                    ps_k = accps.tile([128, CH], F32, tag="acc")
                    ps_v = accps.tile([128, CH], F32, tag="acc")
                    for cc in range(NCT):
                        xt = xtp.tile([128, CH], F32R, tag="xt")
                        nc.sync.dma_start(
                            out=xt, in_=xT_d[cc * 128:(cc + 1) * 128, cols]
                        )
                        st0, sp0 = (cc == 0), (cc == NCT - 1)
                        nc.tensor.matmul(ps_q0, wq_sb[:, cc, 0:128], xt, start=st0, stop=sp0)
                        nc.tensor.matmul(ps_q1, wq_sb[:, cc, 128:256], xt, start=st0, stop=sp0)
                        nc.tensor.matmul(ps_k, wk_sb[:, cc, :], xt, start=st0, stop=sp0)
                        nc.tensor.matmul(ps_v, wv_sb[:, cc, :], xt, start=st0, stop=sp0)

                    cos_t = csp.tile([128, CH], F32, tag="cos")
                    nc.gpsimd.dma_start(out=cos_t, in_=cos_d[:, cols])
                    sin_t = csp.tile([128, CH], F32, tag="sin")
                    nc.gpsimd.dma_start(out=sin_t, in_=sin_d[:, cols])

                    for psrc, dstT in ((ps_q0, qT[0]), (ps_q1, qT[1]), (ps_k, kT)):
                        raw = rp.tile([128, CH], F32R, tag="qraw")
                        nc.vector.tensor_copy(raw, psrc)
                        ps_sw = trps.tile([128, CH], F32, tag="tr")
                        nc.tensor.matmul(ps_sw, pswap, raw, start=True, stop=True)
                        t2 = rp.tile([128, CH], F32, tag="t2")
                        nc.vector.tensor_tensor(
                            t2, raw.bitcast(F32), cos_t, op=mybir.AluOpType.mult
                        )
                        # sw *= sin in place (PSUM), then add -> rope output
                        nc.vector.tensor_tensor(ps_sw, ps_sw, sin_t, op=mybir.AluOpType.mult)
                        nc.vector.tensor_tensor(
                            dstT[:, cols], t2, ps_sw, op=mybir.AluOpType.add
                        )

                    # V: evacuate then PE-transpose to natural [s, d] layout
                    vtmp = rp.tile([128, CH], F32, tag="qraw2")
                    nc.vector.tensor_copy(vtmp, ps_v)
                    ps_vt = trps.tile([128, CH], F32, tag="tr")
                    for u in range(4):
                        nc.tensor.transpose(
                            ps_vt[:, u * 128:(u + 1) * 128],
                            vtmp[:, u * 128:(u + 1) * 128],
                            ident,
                        )
                    nc.vector.tensor_copy(
                        vN[:, 4 * m:4 * m + 4, :],
                        ps_vt.rearrange("p (u d) -> p u d", u=4),
                    )

                    # partial block sums for this chunk's 4 key blocks
                    nc.vector.tensor_reduce(
                        bm[:, 4 * m:4 * m + 4],
                        kT.bitcast(F32)[:, cols].rearrange("p (b t) -> p b t", b=4),
                        axis=mybir.AxisListType.X,
                        op=mybir.AluOpType.add,
                    )
                    # gating flags for this chunk's query blocks (needs bm 0..i)
                    if m >= 2:
                        for h in range(HPC):
                            for i in range(4 * m, 4 * m + 4):
                                nbk = 4 * m + 4  # even N; cols > i unused
                                ps_g = trps.tile([128, NB], F32, tag="g", bufs=1)
                                nc.tensor.matmul(
                                    ps_g[:, 0:nbk],
                                    qT[h][:, i * 128:(i + 1) * 128],
                                    bm[:, 0:nbk],
                                    start=True,
                                    stop=True,
                                )
                                cmp = gp.tile([128, NB], F32, tag="cmp", bufs=2)
                                cnt = gp.tile([128, 1], F32, tag="cnt", bufs=2)
                                nc.vector.tensor_scalar(
                                    out=cmp[:, 0:i],
                                    in0=ps_g[:, 0:i],
                                    scalar1=ps_g[:, i:i + 1],
                                    scalar2=None,
                                    op0=mybir.AluOpType.is_gt,
                                )
                                nc.vector.tensor_reduce(
                                    cnt,
                                    cmp[:, 0:i],
                                    axis=mybir.AxisListType.X,
                                    op=mybir.AluOpType.add,
                                )
                                # notflag: 1.0 -> own block selected (keep all)
                                nf = gp.tile([128, 1], F32, tag="nf", bufs=2)
                                nc.vector.tensor_scalar(
                                    out=nf,
                                    in0=cnt,
                                    scalar1=float(TOPK) - 0.5,
                                    scalar2=None,
                                    op0=mybir.AluOpType.is_lt,
                                )
                                ps_ft = trps.tile([1, 128], F32, tag="ft", bufs=1)
                                nc.tensor.transpose(ps_ft, nf, ident)
                                nc.vector.tensor_copy(
                                    Ft[h][:, (i - TOPK) * 128:(i - TOPK + 1) * 128],
                                    ps_ft,
                                )

            # ---------------- phases 3+4 ------------------------------------
            _phase34(nc, tc, qT, kT, vN, trik, ones_c, ones_r, Ft, wo_d, out_d)

    nc.compile()
    _CACHE[key] = nc
    return nc


def _phase34(nc, tc, qT, kT, vN, trik, ones_c, ones_r, Ft, wo_d, out_d):
    wop_cm = tc.tile_pool(name="wop", bufs=1)
    wop = wop_cm.__enter__()
    wo_sb = wop.tile([128, HPC, DIM], F32R, tag="wo")
    nc.gpsimd.dma_start(out=wo_sb, in_=wo_d.rearrange("(t p) d -> p t d", p=128))
    # ------- phase 3: attention with interleaved output projection -------
    # (wo(m) right after attn(m) so the 32MB output DMA spreads over the
    # whole kernel instead of piling into a DMA-bound tail phase)
    with (
        tc.tile_pool(name="att", bufs=4) as ap,
        tc.tile_pool(name="attb", bufs=2) as ab,
        tc.tile_pool(name="oTs", bufs=4) as otp,
        tc.tile_pool(name="outp", bufs=6) as outp,
        tc.tile_pool(name="att_s", bufs=2, space="PSUM") as pss,
        tc.tile_pool(name="att_o", bufs=3, space="PSUM") as pso,
        tc.tile_pool(name="att_d", bufs=2, space="PSUM") as psd,
        tc.tile_pool(name="att_b", bufs=1, space="PSUM") as psb,
    ):
        prev_wo = None  # deferred wo-section emitter (SW pipeline by 1 chunk)
        for m in range(NCHUNK):
            nsk = 8 if m < 2 else 4 * m + 4
            cols = slice(m * CH, (m + 1) * CH)
            ps_o = [pso.tile([128, CH], F32, tag="o", name=f"o{h}") for h in range(HPC)]
            oTc = [
                otp.tile([128, CH], F32R, tag="oTc", name=f"oTc{h}")
                for h in range(HPC)
            ]
            # precompute diagonal-mask tiles for this chunk's band (off the
            # exp->PV critical path)
            mks = {}
            if m >= 2:
                for j in range(4 * m, 4 * m + 4):
                    for h in range(HPC):
                        ps_bc = psb.tile([128, CH], F32, tag="bc", bufs=1)
                        nc.tensor.matmul(
                            ps_bc[:, 0:128],
                            ones_r,
                            Ft[h][:, (j - TOPK) * 128:(j - TOPK + 1) * 128],
                            start=True,
                            stop=True,
                        )
                        mk = ab.tile([128, 128], F32, tag="mk", bufs=10)
                        nc.vector.tensor_tensor(
                            mk, trik, ps_bc[:, 0:128], op=mybir.AluOpType.max
                        )
                        mks[(j, h)] = mk
            ps_den = [
                psd.tile([1, CH], F32, tag=f"den{h}", name=f"den{h}", bufs=1)
                for h in range(HPC)
            ]
            for j in range(nsk):
                band = m >= 2 and j >= 4 * m
                col0 = (j - 4 * m) * 128 if band else 0
                for h in range(HPC):
                    ps_s = pss.tile([128, CH], F32, tag="s", bufs=2)
                    nc.tensor.matmul(
                        ps_s[:, col0:],
                        kT[:, j * 128:(j + 1) * 128],
                        qT[h][:, m * CH + col0:(m + 1) * CH],
                        start=True,
                        stop=True,
                    )
                    pexp = ap.tile([128, CH], F32R, tag="pexp", bufs=8)
                    nc.scalar.activation(
                        out=pexp[:, col0:],
                        in_=ps_s[:, col0:],
                        func=mybir.ActivationFunctionType.Exp,
                        scale=INV_SQRT_D,
                    )
                    if band:
                        nc.vector.tensor_tensor(
                            pexp[:, col0:col0 + 128],
                            pexp[:, col0:col0 + 128].bitcast(F32),
                            mks[(j, h)],
                            op=mybir.AluOpType.mult,
                        )
                    nc.tensor.matmul(
                        ps_o[h][:, col0:],
                        vN[:, j, :],
                        pexp[:, col0:],
                        start=(j == 0),
                        stop=(j == nsk - 1),
                    )
                    nc.tensor.matmul(
                        ps_den[h][:, col0:],
                        ones_c,
                        pexp[:, col0:],
                        start=(j == 0),
                        stop=(j == nsk - 1),
                    )
            for h in range(HPC):
                rec = ab.tile([1, CH], F32R, tag="rec")
                nc.vector.reciprocal(rec, ps_den[h])
                ps_rb = psb.tile([128, CH], F32, tag="bc", bufs=1)
                for u in range(4):
                    nc.tensor.matmul(
                        ps_rb[:, u * 128:(u + 1) * 128],
                        ones_r,
                        rec[:, u * 128:(u + 1) * 128],
                        start=True,
                        stop=True,
                    )
                bc_sb = ab.tile([128, CH], F32, tag="bcs")
                nc.scalar.copy(bc_sb, ps_rb)
                nc.vector.tensor_tensor(
                    oTc[h], ps_o[h], bc_sb, op=mybir.AluOpType.mult
                )
            # ---- wo(m') emission: projection for s-tiles of chunk m' ----
            def emit_wo(mm, oTc_mm):
                for u in range(4):
                    st = 4 * mm + u
                    for n in range(4):
                        ncols = slice(n * 512, (n + 1) * 512)
                        ps_w = pso.tile([128, 512], F32, tag="o", name="psw")
                        nc.tensor.matmul(
                            ps_w,
                            oTc_mm[0][:, u * 128:(u + 1) * 128],
                            wo_sb[:, 0, ncols],
                            start=True,
                            stop=False,
                        )
                        nc.tensor.matmul(
                            ps_w,
                            oTc_mm[1][:, u * 128:(u + 1) * 128],
                            wo_sb[:, 1, ncols],
                            start=False,
                            stop=True,
                        )
                        osb = outp.tile([128, 512], F32, tag="ow")
                        if (st * 4 + n) % 2 == 0:
                            nc.scalar.copy(osb, ps_w)
                            nc.sync.dma_start(
                                out=out_d[st * 128:(st + 1) * 128, ncols], in_=osb
                            )
                        else:
                            nc.vector.tensor_copy(osb, ps_w)
                            nc.gpsimd.dma_start(
                                out=out_d[st * 128:(st + 1) * 128, ncols], in_=osb
                            )

            if prev_wo is not None:
                emit_wo(*prev_wo)
            prev_wo = (m, oTc)
        emit_wo(*prev_wo)
    wop_cm.__exit__(None, None, None)
def kernel(x, wq, wk, wv, wo):
    bs = np.asarray(x).shape[0]
    in_maps = make_in_maps(x, wq, wk, wv, wo)
    nc = _build_nc()
    res = run_bass_kernel_spmd(nc, in_maps, list(range(N_CORES)))
    out = res.results[0]["out"].astype(np.float64)
    for c in range(1, N_CORES):
        out += res.results[c]["out"]
    return out.astype(np.float32).reshape(bs, SEQ, DIM)


if __name__ == "__main__":
    rng = np.random.default_rng(0)
    xs = {
        "x": rng.standard_normal((1, SEQ, DIM), dtype=np.float32),
        "wq": rng.standard_normal((DIM, DIM), dtype=np.float32) * (DIM ** -0.5),
        "wk": rng.standard_normal((DIM, DIM // 2), dtype=np.float32) * (DIM ** -0.5),
        "wv": rng.standard_normal((DIM, DIM // 2), dtype=np.float32) * (DIM ** -0.5),
        "wo": rng.standard_normal((DIM, DIM), dtype=np.float32) * (DIM ** -0.5),
    }
    out = kernel(**xs)
    print("out", out.shape, out.dtype, np.abs(out).max())



# revision 42
# speedup vs baseline: 1.0787x; 1.0787x over previous
"""MixtureOfBlockAttention TRN2 kernel — 8-core head-parallel (TP) Bass/Tile implementation.

Semantics (verified equivalent to the reference, rel err ~2e-2 budget dominated
by top-k near-tie flips caused by fp32r rounding of x/w — irreducible without
dropping fp32r):
the reference mask `maximum(token_mask, causal*NEG_INF)` masks a position iff
it is BOTH future AND in a non-selected block. Consequences:
  - query blocks 0..7 attend to ALL tokens of key blocks 0..7 (dense, no mask);
  - query block i>=8 attends densely to key blocks 0..i-1, and within its own
    (diagonal) block applies strict causal masking ONLY for rows whose own
    block is not among their top-8 gating blocks.
Selection rank for query s in block i (i>=8): own block selected iff
  #{j < i : g[s,j] > g[s,i]} < 8, with g = q . (block sums of roped k)
(positive-scale invariant, so block sums replace means and the 1/sqrt(d)
factor is dropped).

Sharding: 16 query heads / 8 cores = 2 heads per core; KV head c serves both.
wq/wk/wv column-sliced, wo row-sliced; partial outputs summed on host.
Host-side layout prep: x is transposed to xT[c, s] (the PE contracts over the
partition dim, so both matmul operands need c on partitions) and float inputs
are pre-rounded to fp32r; both are pure data-layout transforms.

All big matmuls run in float32r (TF32-like input rounding, fp32 accumulate,
full PE rate at N>=256). Optimizations vs the original baseline (all
numerically neutral except the rope-table recurrence, which shifts the final
rel err by <1e-6 with no top-k flips):
  - per-j exp fused across the 2 heads (one Act instruction, PSUM [128,2,512]);
  - softmax denominators for both heads accumulate into one [2,512] PSUM bank
    via [128,2] one-hot-column stationaries;
  - reciprocal-broadcast and notflag-broadcast matmuls emitted at ap>=512
    instead of 4x/8x ap=128 pieces (fp32r runs 4 cyc/row below ap 256);
  - diagonal-band j=4m+3 matmuls padded from ap=128 to ap=256 with a -1e5
    PSUM memset in the pad so exp underflows to exact 0 there;
  - V-transpose and notflag-transpose in fp32r (1.5 cyc/row) not fp32 (2.0);
  - rope cos/sin tables generated on device by a per-chunk angle-rotation
    recurrence (saves 4MB/core of HBM reads and the phase-1 DMA deficit);
  - attention j-loop software-pipelined by one step (QK/exp of j+1 emitted
    before PV/den of j) so the in-order PE stream hides the exp latency;
  - the deferred wo projection is emitted as 16 per-chunk output tiles fired
    at most one per attention step from a cross-chunk FIFO, filling the PE's
    residual exp-latency slack without ever stalling it on the ps_w WAR;
  - gating reordered (all matmuls, then DVE compare chains, then batched
    transposes) and chunks 6-7's gating deferred into the attention phase;
  - PSUM: scores 2x[128,2,512] + 2 ps_o + 1 ps_w + 1 den2 = exactly 8 banks.
"""

import math
import sys

import numpy as np

if "/opt/trn_rl_repo" not in sys.path:
    sys.path.insert(0, "/opt/trn_rl_repo")

import concourse.bacc as bacc
import concourse.mybir as mybir
import concourse.tile as tile
from concourse.bass_utils import run_bass_kernel_spmd

F32 = mybir.dt.float32
F32R = mybir.dt.float32r

SEQ = 4096
DIM = 2048
HEAD_DIM = 128
N_HEADS = 16
N_CORES = 8
HPC = N_HEADS // N_CORES       # heads per core = 2
DPC = HPC * HEAD_DIM           # q/o dims per core = 256
BLOCK = 128
NB = SEQ // BLOCK              # 32 key blocks
TOPK = 8
NCHUNK = 8                     # s-chunks of 512
CH = SEQ // NCHUNK             # 512
NCT = DIM // 128               # 16 contraction tiles
INV_SQRT_D = 1.0 / math.sqrt(HEAD_DIM)
PAD_NEG = -100000.0

_CACHE = {}


def _round_fp32r(a):
    """Round fp32 to the fp32r grid (top-11-bit mantissa, round-to-nearest)."""
    a = np.ascontiguousarray(a, dtype=np.float32)
    try:
        from neuron_dtypes import static_cast_fp32_to_fp32r

        return static_cast_fp32_to_fp32r(a).view(np.float32).astype(np.float32)
    except Exception:
        u = a.view(np.uint32)
        return ((u + np.uint32(0x800)) & np.uint32(0xFFFFF000)).view(np.float32).copy()


def _host_constants():
    if "consts" in _CACHE:
        return _CACHE["consts"]
    p = np.arange(HEAD_DIM // 2, dtype=np.float64)
    inv_freq = 1.0 / (10000.0 ** (2.0 * p / HEAD_DIM))
    ang = np.arange(SEQ, dtype=np.float64)[None, :] * inv_freq[:, None]  # [64, S]
    cos = np.cos(ang).astype(np.float32)
    sin = np.sin(ang).astype(np.float32)
    cos_ds = np.ascontiguousarray(np.repeat(cos, 2, axis=0))   # [128, S]
    sin_ds = np.empty((HEAD_DIM, SEQ), dtype=np.float32)       # signed sin
    sin_ds[0::2] = -sin
    sin_ds[1::2] = sin
    # per-partition rotation by CH positions: next-chunk tables via
    # cos' = cos*C - sin_ds*S_row ; sin_ds' = sin_ds*C + cos*S_row
    # (S_row carries the sign convention of the interleaved sin_ds rows)
    inv_freq = 1.0 / (10000.0 ** (2.0 * p / HEAD_DIM))
    c512 = np.cos(CH * inv_freq)
    s512 = np.sin(CH * inv_freq)
    rotC = np.repeat(c512, 2).astype(np.float32)[:, None]      # [128, 1]
    rotS = np.empty((HEAD_DIM,), dtype=np.float64)
    rotS[0::2] = -s512
    rotS[1::2] = s512
    rotS = rotS.astype(np.float32)[:, None]                    # [128, 1]
    pswap = np.zeros((128, 128), dtype=np.float32)             # swap 2p <-> 2p+1
    idx = np.arange(128)
    pswap[idx, idx ^ 1] = 1.0
    identm = np.eye(128, dtype=np.float32)
    r = np.arange(BLOCK)
    trikeep = (r[:, None] <= r[None, :]).astype(np.float32)    # keep iff sk <= sq
    ones_row = np.ones((1, 128), dtype=np.float32)
    # one-hot-column stationaries for per-head den accumulation into [2, CH]:
    # oh2[:, h, :] is [128, 2] with column h all-ones
    oh2 = np.zeros((128, 2, 2), dtype=np.float32)
    oh2[:, 0, 0] = 1.0
    oh2[:, 1, 1] = 1.0
    # one-hot-row stationaries for per-head [2,CH] -> [128,CH] broadcast:
    # sel2[:, h, :] is [2, 128] with row h all-ones
    sel2 = np.zeros((2, 2, 128), dtype=np.float32)
    sel2[0, 0, :] = 1.0
    sel2[1, 1, :] = 1.0
    _CACHE["consts"] = (cos_ds, sin_ds, rotC, rotS, pswap, identm, trikeep, ones_row, oh2, sel2)
    return _CACHE["consts"]


def make_in_maps(x, wq, wk, wv, wo):
    """Shard + lay out the full inputs for the 8 cores."""
    x2 = np.asarray(x, dtype=np.float32).reshape(SEQ, DIM)
    xT = _round_fp32r(np.ascontiguousarray(x2.T))
    wq = np.asarray(wq, dtype=np.float32)
    wk = np.asarray(wk, dtype=np.float32)
    wv = np.asarray(wv, dtype=np.float32)
    wo = np.asarray(wo, dtype=np.float32)
    cos_ds, sin_ds, rotC, rotS, pswap, identm, trikeep, ones_row, oh2, sel2 = _host_constants()
    pswap_r = _round_fp32r(pswap)
    ones_row_r = _round_fp32r(ones_row)
    oh2_r = _round_fp32r(oh2)
    sel2_r = _round_fp32r(sel2)
    in_maps = []
    for c in range(N_CORES):
        in_maps.append(
            {
                "xT": xT,
                "wq": _round_fp32r(wq[:, c * DPC:(c + 1) * DPC]),
                "wk": _round_fp32r(wk[:, c * HEAD_DIM:(c + 1) * HEAD_DIM]),
                "wv": _round_fp32r(wv[:, c * HEAD_DIM:(c + 1) * HEAD_DIM]),
                "wo": _round_fp32r(wo[c * DPC:(c + 1) * DPC, :]),
                "cos0": np.ascontiguousarray(cos_ds[:, 0:CH]),
                "sin0": np.ascontiguousarray(sin_ds[:, 0:CH]),
                "rotC": rotC,
                "rotS": rotS,
                "pswap": pswap_r,
                "identm": _round_fp32r(identm),
                "trikeep": trikeep,
                "ones_r": ones_row_r,
                "oh2": oh2_r,
                "sel2": sel2_r,
            }
        )
    return in_maps


def _gating(nc, m, qT, bm, Ft, ident, ps_pool, ps_tag, sb_pool):
    """Own-block top-k flags for chunk m's 4 query blocks (both heads).

    All 8 gating matmuls first, then the DVE compare chains, then the 8
    transposes batched 4-per-PSUM-bank, so the in-order PE stream never
    waits mid-chain. PSUM scratch comes from (ps_pool, ps_tag) so this can
    run late, inside the attention phase, for the last two chunks.
    """
    import concourse.mybir as mybir

    F32 = mybir.dt.float32
    F32R = mybir.dt.float32r
    pairs = [(h, i) for h in range(HPC) for i in range(4 * m, 4 * m + 4)]
    nbk = 4 * m + 4  # even N; cols > i unused
    ps_g8 = ps_pool.tile([128, 8, NB], F32, tag=ps_tag, bufs=1, name="g8")
    for p, (h, i) in enumerate(pairs):
        nc.tensor.matmul(
            ps_g8[:, p, 0:nbk],
            qT[h][:, i * 128:(i + 1) * 128],
            bm[:, 0:nbk],
            start=True,
            stop=True,
        )
    nfs = []
    for p, (h, i) in enumerate(pairs):
        cmp = sb_pool.tile([128, NB], F32, tag="cmp", bufs=2, name="cmp")
        cnt = sb_pool.tile([128, 1], F32, tag="cnt", bufs=2, name="cnt")
        nc.vector.tensor_scalar(
            out=cmp[:, 0:i],
            in0=ps_g8[:, p, 0:i],
            scalar1=ps_g8[:, p, i:i + 1],
            scalar2=None,
            op0=mybir.AluOpType.is_gt,
        )
        nc.vector.tensor_reduce(
            cnt, cmp[:, 0:i], axis=mybir.AxisListType.X, op=mybir.AluOpType.add
        )
        # notflag: 1.0 -> own block selected (keep all)
        # (fp32r tiles: values are exactly 0.0/1.0)
        nf = sb_pool.tile([128, 1], F32R, tag="nf", bufs=8, name=f"nf{p}")
        nc.vector.tensor_scalar(
            out=nf,
            in0=cnt,
            scalar1=float(TOPK) - 0.5,
            scalar2=None,
            op0=mybir.AluOpType.is_lt,
        )
        nfs.append(nf)
    for h in range(HPC):
        ps_ft4 = ps_pool.tile([1, 4, 128], F32, tag=ps_tag, bufs=1, name="ft4")
        for t in range(4):
            nc.tensor.transpose(
                ps_ft4.bitcast(F32R)[:, t, :], nfs[4 * h + t], ident
            )
        nc.vector.tensor_copy(
            Ft[:, h, (4 * m - 8) * 128:(4 * m - 4) * 128],
            ps_ft4.rearrange("o f t -> o (f t)"),
        )



def _build_nc(reps=1):
    key = f"nc{reps}"
    if key in _CACHE:
        return _CACHE[key]
    nc = bacc.Bacc(None, target_bir_lowering=False)

    xT_d = nc.dram_tensor("xT", [DIM, SEQ], F32R, kind="ExternalInput")
    wq_d = nc.dram_tensor("wq", [DIM, DPC], F32R, kind="ExternalInput")
    wk_d = nc.dram_tensor("wk", [DIM, HEAD_DIM], F32R, kind="ExternalInput")
    wv_d = nc.dram_tensor("wv", [DIM, HEAD_DIM], F32R, kind="ExternalInput")
    wo_d = nc.dram_tensor("wo", [DPC, DIM], F32R, kind="ExternalInput")
    cos_d = nc.dram_tensor("cos0", [HEAD_DIM, CH], F32, kind="ExternalInput")
    sin_d = nc.dram_tensor("sin0", [HEAD_DIM, CH], F32, kind="ExternalInput")
    rotc_d = nc.dram_tensor("rotC", [HEAD_DIM, 1], F32, kind="ExternalInput")
    rots_d = nc.dram_tensor("rotS", [HEAD_DIM, 1], F32, kind="ExternalInput")
    psw_d = nc.dram_tensor("pswap", [128, 128], F32R, kind="ExternalInput")
    idm_d = nc.dram_tensor("identm", [128, 128], F32R, kind="ExternalInput")
    trk_d = nc.dram_tensor("trikeep", [BLOCK, BLOCK], F32, kind="ExternalInput")
    onr_d = nc.dram_tensor("ones_r", [1, 128], F32R, kind="ExternalInput")
    oh2_d = nc.dram_tensor("oh2", [128, 2, 2], F32R, kind="ExternalInput")
    sel2_d = nc.dram_tensor("sel2", [2, 2, 128], F32R, kind="ExternalInput")
    out_d = nc.dram_tensor("out", [SEQ, DIM], F32, kind="ExternalOutput")

    with tile.TileContext(nc) as tc, nc.allow_low_precision(
        reason="float32r rounding of matmul operands is intentional"
    ):
      for _rep in range(reps):
        with tc.tile_pool(name="persist", bufs=1) as per:
            qT = [per.tile([128, SEQ], F32R, tag=f"qT{h}", name=f"qT{h}") for h in range(HPC)]
            kT = per.tile([128, SEQ], F32R, tag="kT")
            vN = per.tile([128, NB, 128], F32R, tag="vN")   # [s-in-tile, sk-tile, d]
            ident = per.tile([128, 128], F32R, tag="ident")
            pswap = per.tile([128, 128], F32R, tag="pswap")
            trik = per.tile([BLOCK, BLOCK], F32, tag="trik")
            ones_r = per.tile([1, 128], F32R, tag="ones_r")
            oh2 = per.tile([128, 2, 2], F32R, tag="oh2")    # [k, h, den-col]
            sel2 = per.tile([2, 2, 128], F32R, tag="sel2")  # [den-row, h, p]
            bm = per.tile([128, NB], F32R, tag="bm")
            # per-head notflag rows: Ft[0, h, (i-TOPK)*128:...] is the [1,128]
            # notflag row for query block i of head h, at base partition 0
            Ft = per.tile([1, HPC, (NB - TOPK) * 128], F32R, tag="Ft")

            # dummy exp so the Exp act-table load overlaps the initial weight
            # DMAs instead of stalling the first attention chunk
            warm = per.tile([1, 1], F32, tag="warm")
            nc.vector.memset(warm, 0.0)
            nc.scalar.activation(
                out=warm, in_=warm, func=mybir.ActivationFunctionType.Exp
            )

            # ---------------- phase 1: projections + rope -------------------
            with (
                tc.tile_pool(name="wpool", bufs=1) as wp,
                tc.tile_pool(name="xtp", bufs=17) as xtp,
                tc.tile_pool(name="ropep", bufs=2) as rp,
                tc.tile_pool(name="csin", bufs=2) as csp,
                # acc_ps declared first so its PSUM range lines up with the
                # attention score pool's range: the last acc_ps readers (rope
                # copies) finish well before the gating tail that occupies
                # pj_ps, letting chunk-0 QK matmuls start during the tail
                tc.tile_pool(name="acc_ps", bufs=4, space="PSUM") as accps,
                tc.tile_pool(name="pj_ps", bufs=2, space="PSUM") as trps,
            ):
                wq_sb = wp.tile([128, NCT, DPC], F32R, tag="wq")
                wk_sb = wp.tile([128, NCT, HEAD_DIM], F32R, tag="wk")
                wv_sb = wp.tile([128, NCT, HEAD_DIM], F32R, tag="wv")
                wq_r = wq_d.rearrange("(t p) d -> p t d", p=128)
                wk_r = wk_d.rearrange("(t p) d -> p t d", p=128)
                wv_r = wv_d.rearrange("(t p) d -> p t d", p=128)
                # k/v weights first: chunk 0 runs its k/v matmuls while the
                # (2x bigger) wq still streams in
                nc.gpsimd.dma_start(out=wk_sb[:, 0:4, :], in_=wk_r[:, 0:4, :])
                nc.gpsimd.dma_start(out=wv_sb[:, 0:4, :], in_=wv_r[:, 0:4, :])
                nc.gpsimd.dma_start(out=wk_sb[:, 4:16, :], in_=wk_r[:, 4:16, :])
                nc.gpsimd.dma_start(out=wv_sb[:, 4:16, :], in_=wv_r[:, 4:16, :])
                nc.gpsimd.dma_start(out=wq_sb[:, 0:8, :], in_=wq_r[:, 0:8, :])
                nc.gpsimd.dma_start(out=wq_sb[:, 8:16, :], in_=wq_r[:, 8:16, :])

                rot_c = wp.tile([128, 1], F32, tag="rotc")
                nc.scalar.dma_start(out=rot_c, in_=rotc_d[:])
                rot_s = wp.tile([128, 1], F32, tag="rots")
                nc.scalar.dma_start(out=rot_s, in_=rots_d[:])
                gp = wp  # reuse the bufs=1 pool scope for small gating tiles
                for m in range(NCHUNK):
                    cols = slice(m * CH, (m + 1) * CH)
                    ps_q0 = accps.tile([128, CH], F32, tag="acc")
                    ps_q1 = accps.tile([128, CH], F32, tag="acc")
                    ps_k = accps.tile([128, CH], F32, tag="acc")
                    ps_v = accps.tile([128, CH], F32, tag="acc")
                    if m == 0:
                        # two passes (k/v then q) to match the weight-arrival
                        # order; the xt tiles stay resident for the q pass
                        xts = []
                        for cc in range(NCT):
                            xt = xtp.tile([128, CH], F32R, tag="xt")
                            # first tiles ride the otherwise-idle Act HWDGE
                            # queue in parallel with the sync queue
                            q = nc.scalar if cc < 4 else nc.sync
                            q.dma_start(
                                out=xt, in_=xT_d[cc * 128:(cc + 1) * 128, cols]
                            )
                            xts.append(xt)
                            st0, sp0 = (cc == 0), (cc == NCT - 1)
                            nc.tensor.matmul(ps_k, wk_sb[:, cc, :], xt, start=st0, stop=sp0)
                            nc.tensor.matmul(ps_v, wv_sb[:, cc, :], xt, start=st0, stop=sp0)
                        # consts (needed from the rope stage onwards) follow
                        # the early x tiles on the Act HWDGE queue
                        nc.scalar.dma_start(out=pswap, in_=psw_d[:])
                        nc.scalar.dma_start(out=ident, in_=idm_d[:])
                        nc.scalar.dma_start(out=trik, in_=trk_d[:])
                        nc.scalar.dma_start(out=ones_r, in_=onr_d[:])
                        nc.scalar.dma_start(out=oh2, in_=oh2_d[:])
                        nc.scalar.dma_start(out=sel2, in_=sel2_d[:])
                        for cc in range(NCT):
                            st0, sp0 = (cc == 0), (cc == NCT - 1)
                            nc.tensor.matmul(ps_q0, wq_sb[:, cc, 0:128], xts[cc], start=st0, stop=sp0)
                            nc.tensor.matmul(ps_q1, wq_sb[:, cc, 128:256], xts[cc], start=st0, stop=sp0)
                    else:
                      for cc in range(NCT):
                        xt = xtp.tile([128, CH], F32R, tag="xt")
                        nc.sync.dma_start(
                            out=xt, in_=xT_d[cc * 128:(cc + 1) * 128, cols]
                        )
                        st0, sp0 = (cc == 0), (cc == NCT - 1)
                        nc.tensor.matmul(ps_q0, wq_sb[:, cc, 0:128], xt, start=st0, stop=sp0)
                        nc.tensor.matmul(ps_q1, wq_sb[:, cc, 128:256], xt, start=st0, stop=sp0)
                        nc.tensor.matmul(ps_k, wk_sb[:, cc, :], xt, start=st0, stop=sp0)
                        nc.tensor.matmul(ps_v, wv_sb[:, cc, :], xt, start=st0, stop=sp0)

                    if m == 0:
                        cos_t = csp.tile([128, CH], F32, tag="cos", bufs=2)
                        nc.scalar.dma_start(out=cos_t, in_=cos_d[:])
                        sin_t = csp.tile([128, CH], F32, tag="sin", bufs=2)
                        nc.scalar.dma_start(out=sin_t, in_=sin_d[:])
                    else:
                        # rotate the previous chunk's tables by CH positions
                        # (per-partition angle), off the DMA wire entirely
                        cos_p, sin_p = cos_t, sin_t
                        ta = rp.tile([128, CH], F32, tag="t2")
                        nc.vector.tensor_scalar(
                            out=ta, in0=sin_p, scalar1=rot_s, scalar2=None,
                            op0=mybir.AluOpType.mult,
                        )
                        cos_t = csp.tile([128, CH], F32, tag="cos", bufs=2)
                        nc.vector.scalar_tensor_tensor(
                            out=cos_t, in0=cos_p, scalar=rot_c, in1=ta,
                            op0=mybir.AluOpType.mult,
                            op1=mybir.AluOpType.subtract,
                        )
                        tb = rp.tile([128, CH], F32, tag="t2")
                        nc.vector.tensor_scalar(
                            out=tb, in0=cos_p, scalar1=rot_s, scalar2=None,
                            op0=mybir.AluOpType.mult,
                        )
                        sin_t = csp.tile([128, CH], F32, tag="sin", bufs=2)
                        nc.vector.scalar_tensor_tensor(
                            out=sin_t, in0=sin_p, scalar=rot_c, in1=tb,
                            op0=mybir.AluOpType.mult,
                            op1=mybir.AluOpType.add,
                        )

                    for psrc, dstT in ((ps_q0, qT[0]), (ps_q1, qT[1]), (ps_k, kT)):
                        raw = rp.tile([128, CH], F32R, tag="qraw")
                        nc.vector.tensor_copy(raw, psrc)
                        ps_sw = trps.tile([128, CH], F32, tag="tr")
                        nc.tensor.matmul(ps_sw, pswap, raw, start=True, stop=True)
                        t2 = rp.tile([128, CH], F32, tag="t2")
                        nc.vector.tensor_tensor(
                            t2, raw.bitcast(F32), cos_t, op=mybir.AluOpType.mult
                        )
                        # sw *= sin in place (PSUM), then add -> rope output
                        nc.vector.tensor_tensor(ps_sw, ps_sw, sin_t, op=mybir.AluOpType.mult)
                        nc.vector.tensor_tensor(
                            dstT[:, cols], t2, ps_sw, op=mybir.AluOpType.add
                        )

                    # V: evacuate then PE-transpose to natural [s, d] layout
                    # (fp32r copy: vN is fp32r anyway, and fp32r transpose runs
                    # 1.5 cyc/row vs 2.0 for fp32)
                    vtmp = rp.tile([128, CH], F32R, tag="qraw2")
                    nc.vector.tensor_copy(vtmp, ps_v)
                    ps_vt = trps.tile([128, CH], F32, tag="tr")
                    for u in range(4):
                        nc.tensor.transpose(
                            ps_vt.bitcast(F32R)[:, u * 128:(u + 1) * 128],
                            vtmp[:, u * 128:(u + 1) * 128],
                            ident,
                        )
                    nc.vector.tensor_copy(
                        vN[:, 4 * m:4 * m + 4, :],
                        ps_vt.rearrange("p (u d) -> p u d", u=4),
                    )

                    # partial block sums for this chunk's 4 key blocks
                    nc.vector.tensor_reduce(
                        bm[:, 4 * m:4 * m + 4],
                        kT.bitcast(F32)[:, cols].rearrange("p (b t) -> p b t", b=4),
                        axis=mybir.AxisListType.X,
                        op=mybir.AluOpType.add,
                    )
                    # gating flags for this chunk's query blocks (needs
                    # bm 0..i); chunks 6-7 are deferred into the attention
                    # phase so the phase boundary is not serialized on them
                    if 2 <= m <= 5:
                        _gating(nc, m, qT, bm, Ft, ident, trps, "g", gp)

            # ---------------- phases 3+4 ------------------------------------
            _phase34(nc, tc, qT, kT, vN, trik, oh2, sel2, ones_r, Ft, wo_d,
                     out_d, bm, ident)

    nc.compile()
    _CACHE[key] = nc
    return nc


def _phase34(nc, tc, qT, kT, vN, trik, oh2, sel2, ones_r, Ft, wo_d, out_d,
             bm, ident):
    wop_cm = tc.tile_pool(name="wop", bufs=1)
    wop = wop_cm.__enter__()
    wo_sb = wop.tile([128, HPC, DIM], F32R, tag="wo")
    nc.gpsimd.dma_start(out=wo_sb, in_=wo_d.rearrange("(t p) d -> p t d", p=128))
    # ------- phase 3: attention with interleaved output projection -------
    # (wo(m) right after attn(m) so the 32MB output DMA spreads over the
    # whole kernel instead of piling into a DMA-bound tail phase)
    # PSUM budget (16KB/partition): pss "s" 2x[128,2,CH] = 8KB, pso "o"
    # 2x[128,CH] + "w" 1x[128,CH] = 6KB, psd "den2" 1x[2,CH] = 2KB.
    # Broadcast scratch and the final wo emission reuse the pss "s" slots.
    # ps_w gets its own tag so the deferred wo matmuls interleave freely into
    # the attention exp-latency gaps instead of queueing behind ps_o's WAR.
    with (
        tc.tile_pool(name="att", bufs=4) as ap,
        tc.tile_pool(name="attb", bufs=2) as ab,
        tc.tile_pool(name="oTs", bufs=4) as otp,
        tc.tile_pool(name="outp", bufs=6) as outp,
        tc.tile_pool(name="att_s", bufs=2, space="PSUM") as pss,
        tc.tile_pool(name="att_o", bufs=2, space="PSUM") as pso,
        tc.tile_pool(name="att_d", bufs=1, space="PSUM") as psd,
    ):
        # ---- wo(m') emission: projection for s-tiles of chunk m' ----
        # Emitted piecewise, one output tile per attention j-iteration of the
        # NEXT chunk, so the wo matmuls fill the PE's exp-latency gaps.
        def wo_pieces(mm, oTc_mm, final=False):
            for u in range(4):
                st = 4 * mm + u
                for n in range(4):
                    ncols = slice(n * 512, (n + 1) * 512)
                    if final:
                        # attention is done: reuse the (free) score slots
                        # for a 2-deep pipelined tail
                        ps_w2 = pss.tile([128, HPC, CH], F32, tag="s", name="psw2")
                        ps_w = ps_w2[:, 0, :]
                    else:
                        ps_w = pso.tile([128, 512], F32, tag="w", name="psw", bufs=1)
                    nc.tensor.matmul(
                        ps_w,
                        oTc_mm[0][:, u * 128:(u + 1) * 128],
                        wo_sb[:, 0, ncols],
                        start=True,
                        stop=False,
                    )
                    nc.tensor.matmul(
                        ps_w,
                        oTc_mm[1][:, u * 128:(u + 1) * 128],
                        wo_sb[:, 1, ncols],
                        start=False,
                        stop=True,
                    )
                    osb = outp.tile([128, 512], F32, tag="ow")
                    if (st * 4 + n) % 3 == 0:
                        nc.scalar.copy(osb, ps_w)
                        nc.sync.dma_start(
                            out=out_d[st * 128:(st + 1) * 128, ncols], in_=osb
                        )
                    else:
                        nc.vector.tensor_copy(osb, ps_w)
                        nc.gpsimd.dma_start(
                            out=out_d[st * 128:(st + 1) * 128, ncols], in_=osb
                        )
                    yield True

        wo_queue = []  # pending wo piece generators (FIFO across chunks)

        def fire_wo(n=1):
            # at most n pieces; a second piece per attention step would stall
            # the in-order PE stream on the single-bank ps_w WAR
            while n > 0 and wo_queue:
                if next(wo_queue[0], None) is None:
                    wo_queue.pop(0)
                else:
                    n -= 1

        for m in range(NCHUNK):
            nsk = 8 if m < 2 else 4 * m + 4
            ps_o = [pso.tile([128, CH], F32, tag="o", name=f"o{h}") for h in range(HPC)]
            oTc = [
                otp.tile([128, CH], F32R, tag="oTc", name=f"oTc{h}")
                for h in range(HPC)
            ]
            # precompute diagonal-mask tiles for this chunk's band (off the
            # exp->PV critical path): one broadcast matmul + one max for both
            # heads and all 4 band blocks at once
            mks = None
            if m >= 2:
                ps_bc = pss.tile([128, HPC, CH], F32, tag="s", name="ps_bc")
                for h in range(HPC):
                    nc.tensor.matmul(
                        ps_bc[:, h, :],
                        ones_r,
                        Ft[:, h, (4 * m - 8) * 128:(4 * m - 4) * 128],
                        start=True,
                        stop=True,
                    )
                mks = ab.tile([128, HPC, CH], F32, tag="mk", bufs=2)
                trik_b = trik.rearrange("p (a b t) -> p a b t", a=1, b=1).broadcast_to(
                    [128, HPC, 4, BLOCK]
                )
                nc.vector.tensor_tensor(
                    mks.rearrange("p h (b t) -> p h b t", b=4),
                    trik_b,
                    ps_bc.rearrange("p h (b t) -> p h b t", b=4),
                    op=mybir.AluOpType.max,
                )
            ps_den = psd.tile([2, CH], F32, tag="den2", name="den2", bufs=1)

            def colspan(j):
                band = m >= 2 and j >= 4 * m
                # pad ap=128 matmuls (4 cyc/row below ap 256) to ap=256
                col0 = (j - 4 * m) * 128 if band else 0
                colp = min(col0, CH - 256) if band else 0
                return band, col0, colp

            def emit_qk_exp(j):
                band, col0, colp = colspan(j)
                ps_s = pss.tile([128, HPC, CH], F32, tag="s", bufs=2)
                for h in range(HPC):
                    nc.tensor.matmul(
                        ps_s[:, h, colp:],
                        kT[:, j * 128:(j + 1) * 128],
                        qT[h][:, m * CH + colp:(m + 1) * CH],
                        start=True,
                        stop=True,
                    )
                if colp < col0:
                    # overwrite the pad region so exp underflows to exact 0
                    nc.vector.memset(ps_s[:, :, colp:col0], PAD_NEG)
                pexp = ap.tile([128, HPC, CH], F32R, tag="pexp", bufs=4)
                nc.scalar.activation(
                    out=pexp[:, :, colp:],
                    in_=ps_s[:, :, colp:],
                    func=mybir.ActivationFunctionType.Exp,
                    scale=INV_SQRT_D,
                )
                if band:
                    nc.vector.tensor_tensor(
                        pexp[:, :, col0:col0 + 128],
                        pexp.bitcast(F32)[:, :, col0:col0 + 128],
                        mks[:, :, col0:col0 + 128],
                        op=mybir.AluOpType.mult,
                    )
                return pexp

            def emit_pv_den(j, pexp):
                _, _, colp = colspan(j)
                for h in range(HPC):
                    nc.tensor.matmul(
                        ps_o[h][:, colp:],
                        vN[:, j, :],
                        pexp[:, h, colp:],
                        start=(j == 0),
                        stop=(j == nsk - 1),
                    )
                    nc.tensor.matmul(
                        ps_den[:, colp:],
                        oh2[:, h, :],
                        pexp[:, h, colp:],
                        start=(j == 0 and h == 0),
                        stop=(j == nsk - 1 and h == HPC - 1),
                    )

            # software-pipeline by one j: QK/exp of j+1 is emitted before
            # PV/den of j, so the PE never sits in-order behind exp latency;
            # wo output tiles of the previous chunk (spread evenly over the
            # j-loop) fill the remaining slack
            pexp_j = emit_qk_exp(0)
            for j in range(nsk):
                if j + 1 < nsk:
                    pexp_n = emit_qk_exp(j + 1)
                else:
                    pexp_n = None
                emit_pv_den(j, pexp_j)
                pexp_j = pexp_n
                fire_wo(1)
            fire_wo(1)  # one more piece fills the boundary chain
            # normalisation: reciprocal of the two dens, broadcast via the
            # "w" bank (keeps the "s" slots free so the next chunk's QKs can
            # start during this chain), then scale ps_o into fp32r oTc
            rec2 = ab.tile([2, CH], F32R, tag="rec")
            nc.vector.reciprocal(rec2, ps_den)
            bc_sb = ab.tile([128, HPC, CH], F32, tag="bcs")
            for h in range(HPC):
                ps_rb = pso.tile([128, 512], F32, tag="w", name="ps_rb", bufs=1)
                nc.tensor.matmul(
                    ps_rb, sel2[:, h, :], rec2, start=True, stop=True
                )
                nc.scalar.copy(bc_sb[:, h, :], ps_rb)
            for h in range(HPC):
                nc.vector.tensor_tensor(
                    oTc[h], ps_o[h], bc_sb[:, h, :], op=mybir.AluOpType.mult
                )
            # deferred gating for the last two phase-1 chunks: the PE
            # matmuls double as filler during this chunk's boundary chain
            if m == 2:
                _gating(nc, 6, qT, bm, Ft, ident, pso, "w", ab)
            elif m == 3:
                _gating(nc, 7, qT, bm, Ft, ident, pso, "w", ab)
            wo_queue.append(
                wo_pieces(m, oTc, final=(m == NCHUNK - 1))
            )
        while wo_queue:
            fire_wo(1)
    wop_cm.__exit__(None, None, None)


def kernel(x, wq, wk, wv, wo):
    bs = np.asarray(x).shape[0]
    in_maps = make_in_maps(x, wq, wk, wv, wo)
    nc = _build_nc()
    res = run_bass_kernel_spmd(nc, in_maps, list(range(N_CORES)))
    out = res.results[0]["out"].astype(np.float64)
    for c in range(1, N_CORES):
        out += res.results[c]["out"]
    return out.astype(np.float32).reshape(bs, SEQ, DIM)


if __name__ == "__main__":
    rng = np.random.default_rng(0)
    xs = {
        "x": rng.standard_normal((1, SEQ, DIM), dtype=np.float32),
        "wq": rng.standard_normal((DIM, DIM), dtype=np.float32) * (DIM ** -0.5),
        "wk": rng.standard_normal((DIM, DIM // 2), dtype=np.float32) * (DIM ** -0.5),
        "wv": rng.standard_normal((DIM, DIM // 2), dtype=np.float32) * (DIM ** -0.5),
        "wo": rng.standard_normal((DIM, DIM), dtype=np.float32) * (DIM ** -0.5),
    }
    out = kernel(**xs)
    print("out", out.shape, out.dtype, np.abs(out).max())


# revision 45
# speedup vs baseline: 1.0815x; 1.0026x over previous
"""MixtureOfBlockAttention TRN2 kernel — 8-core head-parallel (TP) Bass/Tile implementation.

Semantics (verified equivalent to the reference, rel err ~2e-2 budget dominated
by top-k near-tie flips caused by fp32r rounding of x/w — irreducible without
dropping fp32r):
the reference mask `maximum(token_mask, causal*NEG_INF)` masks a position iff
it is BOTH future AND in a non-selected block. Consequences:
  - query blocks 0..7 attend to ALL tokens of key blocks 0..7 (dense, no mask);
  - query block i>=8 attends densely to key blocks 0..i-1, and within its own
    (diagonal) block applies strict causal masking ONLY for rows whose own
    block is not among their top-8 gating blocks.
Selection rank for query s in block i (i>=8): own block selected iff
  #{j < i : g[s,j] > g[s,i]} < 8, with g = q . (block sums of roped k)
(positive-scale invariant, so block sums replace means and the 1/sqrt(d)
factor is dropped).

Sharding: 16 query heads / 8 cores = 2 heads per core; KV head c serves both.
wq/wk/wv column-sliced, wo row-sliced; partial outputs summed on host.
Host-side layout prep: x is transposed to xT[c, s] (the PE contracts over the
partition dim, so both matmul operands need c on partitions) and float inputs
are pre-rounded to fp32r; both are pure data-layout transforms.

All big matmuls run in float32r (TF32-like input rounding, fp32 accumulate,
full PE rate at N>=256). Optimizations vs the original baseline (all
numerically neutral except the rope-table recurrence, which shifts the final
rel err by <1e-6 with no top-k flips):
  - per-j exp fused across the 2 heads (one Act instruction, PSUM [128,2,512]);
  - softmax denominators for both heads accumulate into one [2,512] PSUM bank
    via [128,2] one-hot-column stationaries;
  - reciprocal-broadcast and notflag-broadcast matmuls emitted at ap>=512
    instead of 4x/8x ap=128 pieces (fp32r runs 4 cyc/row below ap 256);
  - diagonal-band j=4m+3 matmuls padded from ap=128 to ap=256 with a -1e5
    PSUM memset in the pad so exp underflows to exact 0 there;
  - V-transpose and notflag-transpose in fp32r (1.5 cyc/row) not fp32 (2.0);
  - rope cos/sin tables generated on device by a per-chunk angle-rotation
    recurrence (saves 4MB/core of HBM reads and the phase-1 DMA deficit);
  - attention j-loop software-pipelined by one step (QK/exp of j+1 emitted
    before PV/den of j) so the in-order PE stream hides the exp latency;
  - the deferred wo projection is emitted as 16 per-chunk output tiles fired
    at most one per attention step from a cross-chunk FIFO, filling the PE's
    residual exp-latency slack without ever stalling it on the ps_w WAR;
  - gating reordered (all matmuls, then DVE compare chains, then batched
    transposes) and chunks 6-7's gating deferred into the attention phase;
  - PSUM: scores 2x[128,2,512] + 2 ps_o + 1 ps_w + 1 den2 = exactly 8 banks.
"""

import math
import sys

import numpy as np

if "/opt/trn_rl_repo" not in sys.path:
    sys.path.insert(0, "/opt/trn_rl_repo")

import concourse.bacc as bacc
import concourse.mybir as mybir
import concourse.tile as tile
from concourse.bass_utils import run_bass_kernel_spmd

F32 = mybir.dt.float32
F32R = mybir.dt.float32r

SEQ = 4096
DIM = 2048
HEAD_DIM = 128
N_HEADS = 16
N_CORES = 8
HPC = N_HEADS // N_CORES       # heads per core = 2
DPC = HPC * HEAD_DIM           # q/o dims per core = 256
BLOCK = 128
NB = SEQ // BLOCK              # 32 key blocks
TOPK = 8
NCHUNK = 8                     # s-chunks of 512
CH = SEQ // NCHUNK             # 512
NCT = DIM // 128               # 16 contraction tiles
INV_SQRT_D = 1.0 / math.sqrt(HEAD_DIM)
PAD_NEG = -100000.0

_CACHE = {}


def _round_fp32r(a):
    """Round fp32 to the fp32r grid (top-11-bit mantissa, round-to-nearest)."""
    a = np.ascontiguousarray(a, dtype=np.float32)
    try:
        from neuron_dtypes import static_cast_fp32_to_fp32r

        return static_cast_fp32_to_fp32r(a).view(np.float32).astype(np.float32)
    except Exception:
        u = a.view(np.uint32)
        return ((u + np.uint32(0x800)) & np.uint32(0xFFFFF000)).view(np.float32).copy()


def _host_constants():
    if "consts" in _CACHE:
        return _CACHE["consts"]
    p = np.arange(HEAD_DIM // 2, dtype=np.float64)
    inv_freq = 1.0 / (10000.0 ** (2.0 * p / HEAD_DIM))
    ang = np.arange(SEQ, dtype=np.float64)[None, :] * inv_freq[:, None]  # [64, S]
    cos = np.cos(ang).astype(np.float32)
    sin = np.sin(ang).astype(np.float32)
    cos_ds = np.ascontiguousarray(np.repeat(cos, 2, axis=0))   # [128, S]
    sin_ds = np.empty((HEAD_DIM, SEQ), dtype=np.float32)       # signed sin
    sin_ds[0::2] = -sin
    sin_ds[1::2] = sin
    # per-partition rotation by CH positions: next-chunk tables via
    # cos' = cos*C - sin_ds*S_row ; sin_ds' = sin_ds*C + cos*S_row
    # (S_row carries the sign convention of the interleaved sin_ds rows)
    inv_freq = 1.0 / (10000.0 ** (2.0 * p / HEAD_DIM))
    c512 = np.cos(CH * inv_freq)
    s512 = np.sin(CH * inv_freq)
    rotC = np.repeat(c512, 2).astype(np.float32)[:, None]      # [128, 1]
    rotS = np.empty((HEAD_DIM,), dtype=np.float64)
    rotS[0::2] = -s512
    rotS[1::2] = s512
    rotS = rotS.astype(np.float32)[:, None]                    # [128, 1]
    pswap = np.zeros((128, 128), dtype=np.float32)             # swap 2p <-> 2p+1
    idx = np.arange(128)
    pswap[idx, idx ^ 1] = 1.0
    identm = np.eye(128, dtype=np.float32)
    r = np.arange(BLOCK)
    trikeep = (r[:, None] <= r[None, :]).astype(np.float32)    # keep iff sk <= sq
    ones_row = np.ones((1, 128), dtype=np.float32)
    # one-hot-column stationaries for per-head den accumulation into [2, CH]:
    # oh2[:, h, :] is [128, 2] with column h all-ones
    oh2 = np.zeros((128, 2, 2), dtype=np.float32)
    oh2[:, 0, 0] = 1.0
    oh2[:, 1, 1] = 1.0
    # one-hot-row stationaries for per-head [2,CH] -> [128,CH] broadcast:
    # sel2[:, h, :] is [2, 128] with row h all-ones
    sel2 = np.zeros((2, 2, 128), dtype=np.float32)
    sel2[0, 0, :] = 1.0
    sel2[1, 1, :] = 1.0
    _CACHE["consts"] = (cos_ds, sin_ds, rotC, rotS, pswap, identm, trikeep, ones_row, oh2, sel2)
    return _CACHE["consts"]


def make_in_maps(x, wq, wk, wv, wo):
    """Shard + lay out the full inputs for the 8 cores."""
    x2 = np.asarray(x, dtype=np.float32).reshape(SEQ, DIM)
    xT = _round_fp32r(np.ascontiguousarray(x2.T))
    wq = np.asarray(wq, dtype=np.float32)
    wk = np.asarray(wk, dtype=np.float32)
    wv = np.asarray(wv, dtype=np.float32)
    wo = np.asarray(wo, dtype=np.float32)
    cos_ds, sin_ds, rotC, rotS, pswap, identm, trikeep, ones_row, oh2, sel2 = _host_constants()
    pswap_r = _round_fp32r(pswap)
    ones_row_r = _round_fp32r(ones_row)
    oh2_r = _round_fp32r(oh2)
    sel2_r = _round_fp32r(sel2)
    in_maps = []
    for c in range(N_CORES):
        in_maps.append(
            {
                "xT": xT,
                "wq": _round_fp32r(wq[:, c * DPC:(c + 1) * DPC]),
                "wk": _round_fp32r(wk[:, c * HEAD_DIM:(c + 1) * HEAD_DIM]),
                "wv": _round_fp32r(wv[:, c * HEAD_DIM:(c + 1) * HEAD_DIM]),
                "wo": _round_fp32r(wo[c * DPC:(c + 1) * DPC, :]),
                "cos0": np.ascontiguousarray(cos_ds[:, 0:CH]),
                "sin0": np.ascontiguousarray(sin_ds[:, 0:CH]),
                "rotC": rotC,
                "rotS": rotS,
                "pswap": pswap_r,
                "identm": _round_fp32r(identm),
                "trikeep": trikeep,
                "ones_r": ones_row_r,
                "oh2": oh2_r,
                "sel2": sel2_r,
            }
        )
    return in_maps


def _gating(nc, m, qT, bm, Ft, ident, ps_pool, ps_tag, sb_pool):
    """Own-block top-k flags for chunk m's 4 query blocks (both heads).

    All 8 gating matmuls first, then the DVE compare chains, then the 8
    transposes batched 4-per-PSUM-bank, so the in-order PE stream never
    waits mid-chain. PSUM scratch comes from (ps_pool, ps_tag) so this can
    run late, inside the attention phase, for the last two chunks.
    """
    import concourse.mybir as mybir

    F32 = mybir.dt.float32
    F32R = mybir.dt.float32r
    pairs = [(h, i) for h in range(HPC) for i in range(4 * m, 4 * m + 4)]
    nbk = 4 * m + 4  # even N; cols > i unused
    ps_g8 = ps_pool.tile([128, 8, NB], F32, tag=ps_tag, bufs=1, name="g8")
    for p, (h, i) in enumerate(pairs):
        nc.tensor.matmul(
            ps_g8[:, p, 0:nbk],
            qT[h][:, i * 128:(i + 1) * 128],
            bm[:, 0:nbk],
            start=True,
            stop=True,
        )
    nfs = []
    for p, (h, i) in enumerate(pairs):
        cmp = sb_pool.tile([128, NB], F32, tag="cmp", bufs=2, name="cmp")
        cnt = sb_pool.tile([128, 1], F32, tag="cnt", bufs=2, name="cnt")
        nc.vector.tensor_scalar(
            out=cmp[:, 0:i],
            in0=ps_g8[:, p, 0:i],
            scalar1=ps_g8[:, p, i:i + 1],
            scalar2=None,
            op0=mybir.AluOpType.is_gt,
        )
        nc.vector.tensor_reduce(
            cnt, cmp[:, 0:i], axis=mybir.AxisListType.X, op=mybir.AluOpType.add
        )
        # notflag: 1.0 -> own block selected (keep all)
        # (fp32r tiles: values are exactly 0.0/1.0)
        nf = sb_pool.tile([128, 1], F32R, tag="nf", bufs=8, name=f"nf{p}")
        nc.vector.tensor_scalar(
            out=nf,
            in0=cnt,
            scalar1=float(TOPK) - 0.5,
            scalar2=None,
            op0=mybir.AluOpType.is_lt,
        )
        nfs.append(nf)
    for h in range(HPC):
        ps_ft4 = ps_pool.tile([1, 4, 128], F32, tag=ps_tag, bufs=1, name="ft4")
        for t in range(4):
            nc.tensor.transpose(
                ps_ft4.bitcast(F32R)[:, t, :], nfs[4 * h + t], ident
            )
        nc.vector.tensor_copy(
            Ft[:, h, (4 * m - 8) * 128:(4 * m - 4) * 128],
            ps_ft4.rearrange("o f t -> o (f t)"),
        )



def _build_nc(reps=1):
    key = f"nc{reps}"
    if key in _CACHE:
        return _CACHE[key]
    nc = bacc.Bacc(None, target_bir_lowering=False)

    xT_d = nc.dram_tensor("xT", [DIM, SEQ], F32R, kind="ExternalInput")
    wq_d = nc.dram_tensor("wq", [DIM, DPC], F32R, kind="ExternalInput")
    wk_d = nc.dram_tensor("wk", [DIM, HEAD_DIM], F32R, kind="ExternalInput")
    wv_d = nc.dram_tensor("wv", [DIM, HEAD_DIM], F32R, kind="ExternalInput")
    wo_d = nc.dram_tensor("wo", [DPC, DIM], F32R, kind="ExternalInput")
    cos_d = nc.dram_tensor("cos0", [HEAD_DIM, CH], F32, kind="ExternalInput")
    sin_d = nc.dram_tensor("sin0", [HEAD_DIM, CH], F32, kind="ExternalInput")
    rotc_d = nc.dram_tensor("rotC", [HEAD_DIM, 1], F32, kind="ExternalInput")
    rots_d = nc.dram_tensor("rotS", [HEAD_DIM, 1], F32, kind="ExternalInput")
    psw_d = nc.dram_tensor("pswap", [128, 128], F32R, kind="ExternalInput")
    idm_d = nc.dram_tensor("identm", [128, 128], F32R, kind="ExternalInput")
    trk_d = nc.dram_tensor("trikeep", [BLOCK, BLOCK], F32, kind="ExternalInput")
    onr_d = nc.dram_tensor("ones_r", [1, 128], F32R, kind="ExternalInput")
    oh2_d = nc.dram_tensor("oh2", [128, 2, 2], F32R, kind="ExternalInput")
    sel2_d = nc.dram_tensor("sel2", [2, 2, 128], F32R, kind="ExternalInput")
    out_d = nc.dram_tensor("out", [SEQ, DIM], F32, kind="ExternalOutput")

    with tile.TileContext(nc) as tc, nc.allow_low_precision(
        reason="float32r rounding of matmul operands is intentional"
    ):
      for _rep in range(reps):
        with tc.tile_pool(name="persist", bufs=1) as per:
            qT = [per.tile([128, SEQ], F32R, tag=f"qT{h}", name=f"qT{h}") for h in range(HPC)]
            kT = per.tile([128, SEQ], F32R, tag="kT")
            vN = per.tile([128, NB, 128], F32R, tag="vN")   # [s-in-tile, sk-tile, d]
            ident = per.tile([128, 128], F32R, tag="ident")
            pswap = per.tile([128, 128], F32R, tag="pswap")
            trik = per.tile([BLOCK, BLOCK], F32, tag="trik")
            ones_r = per.tile([1, 128], F32R, tag="ones_r")
            oh2 = per.tile([128, 2, 2], F32R, tag="oh2")    # [k, h, den-col]
            sel2 = per.tile([2, 2, 128], F32R, tag="sel2")  # [den-row, h, p]
            bm = per.tile([128, NB], F32R, tag="bm")
            # per-head notflag rows: Ft[0, h, (i-TOPK)*128:...] is the [1,128]
            # notflag row for query block i of head h, at base partition 0
            Ft = per.tile([1, HPC, (NB - TOPK) * 128], F32R, tag="Ft")

            # dummy exp so the Exp act-table load overlaps the initial weight
            # DMAs instead of stalling the first attention chunk
            warm = per.tile([1, 1], F32, tag="warm")
            nc.vector.memset(warm, 0.0)
            nc.scalar.activation(
                out=warm, in_=warm, func=mybir.ActivationFunctionType.Exp
            )

            # ---------------- phase 1: projections + rope -------------------
            with (
                tc.tile_pool(name="wpool", bufs=1) as wp,
                tc.tile_pool(name="xtp", bufs=17) as xtp,
                tc.tile_pool(name="ropep", bufs=2) as rp,
                tc.tile_pool(name="csin", bufs=2) as csp,
                # acc_ps declared first so its PSUM range lines up with the
                # attention score pool's range: the last acc_ps readers (rope
                # copies) finish well before the gating tail that occupies
                # pj_ps, letting chunk-0 QK matmuls start during the tail
                tc.tile_pool(name="acc_ps", bufs=4, space="PSUM") as accps,
                tc.tile_pool(name="pj_ps", bufs=2, space="PSUM") as trps,
            ):
                wq_sb = wp.tile([128, NCT, DPC], F32R, tag="wq")
                wk_sb = wp.tile([128, NCT, HEAD_DIM], F32R, tag="wk")
                wv_sb = wp.tile([128, NCT, HEAD_DIM], F32R, tag="wv")
                wq_r = wq_d.rearrange("(t p) d -> p t d", p=128)
                wk_r = wk_d.rearrange("(t p) d -> p t d", p=128)
                wv_r = wv_d.rearrange("(t p) d -> p t d", p=128)
                # k/v weights first: chunk 0 runs its k/v matmuls while the
                # (2x bigger) wq still streams in
                nc.gpsimd.dma_start(out=wk_sb[:, 0:4, :], in_=wk_r[:, 0:4, :])
                nc.gpsimd.dma_start(out=wv_sb[:, 0:4, :], in_=wv_r[:, 0:4, :])
                nc.gpsimd.dma_start(out=wk_sb[:, 4:16, :], in_=wk_r[:, 4:16, :])
                nc.gpsimd.dma_start(out=wv_sb[:, 4:16, :], in_=wv_r[:, 4:16, :])
                nc.gpsimd.dma_start(out=wq_sb[:, 0:8, :], in_=wq_r[:, 0:8, :])
                nc.gpsimd.dma_start(out=wq_sb[:, 8:16, :], in_=wq_r[:, 8:16, :])

                rot_c = wp.tile([128, 1], F32, tag="rotc")
                nc.scalar.dma_start(out=rot_c, in_=rotc_d[:])
                rot_s = wp.tile([128, 1], F32, tag="rots")
                nc.scalar.dma_start(out=rot_s, in_=rots_d[:])
                gp = wp  # reuse the bufs=1 pool scope for small gating tiles
                for m in range(NCHUNK):
                    cols = slice(m * CH, (m + 1) * CH)
                    ps_q0 = accps.tile([128, CH], F32, tag="acc")
                    ps_q1 = accps.tile([128, CH], F32, tag="acc")
                    ps_k = accps.tile([128, CH], F32, tag="acc")
                    ps_v = accps.tile([128, CH], F32, tag="acc")
                    if m == 0:
                        # two passes (k/v then q) to match the weight-arrival
                        # order; the xt tiles stay resident for the q pass
                        xts = []
                        for cc in range(NCT):
                            xt = xtp.tile([128, CH], F32R, tag="xt")
                            # first tiles ride the otherwise-idle Act HWDGE
                            # queue in parallel with the sync queue
                            q = nc.scalar if cc < 4 else nc.sync
                            q.dma_start(
                                out=xt, in_=xT_d[cc * 128:(cc + 1) * 128, cols]
                            )
                            xts.append(xt)
                            st0, sp0 = (cc == 0), (cc == NCT - 1)
                            nc.tensor.matmul(ps_k, wk_sb[:, cc, :], xt, start=st0, stop=sp0)
                            nc.tensor.matmul(ps_v, wv_sb[:, cc, :], xt, start=st0, stop=sp0)
                        # consts (needed from the rope stage onwards) follow
                        # the early x tiles on the Act HWDGE queue
                        nc.scalar.dma_start(out=pswap, in_=psw_d[:])
                        nc.scalar.dma_start(out=ident, in_=idm_d[:])
                        nc.scalar.dma_start(out=trik, in_=trk_d[:])
                        nc.scalar.dma_start(out=ones_r, in_=onr_d[:])
                        nc.scalar.dma_start(out=oh2, in_=oh2_d[:])
                        nc.scalar.dma_start(out=sel2, in_=sel2_d[:])
                        for cc in range(NCT):
                            st0, sp0 = (cc == 0), (cc == NCT - 1)
                            nc.tensor.matmul(ps_q0, wq_sb[:, cc, 0:128], xts[cc], start=st0, stop=sp0)
                            nc.tensor.matmul(ps_q1, wq_sb[:, cc, 128:256], xts[cc], start=st0, stop=sp0)
                    else:
                      for cc in range(NCT):
                        xt = xtp.tile([128, CH], F32R, tag="xt")
                        nc.sync.dma_start(
                            out=xt, in_=xT_d[cc * 128:(cc + 1) * 128, cols]
                        )
                        st0, sp0 = (cc == 0), (cc == NCT - 1)
                        nc.tensor.matmul(ps_q0, wq_sb[:, cc, 0:128], xt, start=st0, stop=sp0)
                        nc.tensor.matmul(ps_q1, wq_sb[:, cc, 128:256], xt, start=st0, stop=sp0)
                        nc.tensor.matmul(ps_k, wk_sb[:, cc, :], xt, start=st0, stop=sp0)
                        nc.tensor.matmul(ps_v, wv_sb[:, cc, :], xt, start=st0, stop=sp0)

                    if m == 0:
                        cos_t = csp.tile([128, CH], F32, tag="cos", bufs=2)
                        nc.scalar.dma_start(out=cos_t, in_=cos_d[:])
                        sin_t = csp.tile([128, CH], F32, tag="sin", bufs=2)
                        nc.scalar.dma_start(out=sin_t, in_=sin_d[:])
                    else:
                        # rotate the previous chunk's tables by CH positions
                        # (per-partition angle), off the DMA wire entirely
                        cos_p, sin_p = cos_t, sin_t
                        ta = rp.tile([128, CH], F32, tag="t2")
                        nc.vector.tensor_scalar(
                            out=ta, in0=sin_p, scalar1=rot_s, scalar2=None,
                            op0=mybir.AluOpType.mult,
                        )
                        cos_t = csp.tile([128, CH], F32, tag="cos", bufs=2)
                        nc.vector.scalar_tensor_tensor(
                            out=cos_t, in0=cos_p, scalar=rot_c, in1=ta,
                            op0=mybir.AluOpType.mult,
                            op1=mybir.AluOpType.subtract,
                        )
                        tb = rp.tile([128, CH], F32, tag="t2")
                        nc.vector.tensor_scalar(
                            out=tb, in0=cos_p, scalar1=rot_s, scalar2=None,
                            op0=mybir.AluOpType.mult,
                        )
                        sin_t = csp.tile([128, CH], F32, tag="sin", bufs=2)
                        nc.vector.scalar_tensor_tensor(
                            out=sin_t, in0=sin_p, scalar=rot_c, in1=tb,
                            op0=mybir.AluOpType.mult,
                            op1=mybir.AluOpType.add,
                        )

                    for psrc, dstT in ((ps_q0, qT[0]), (ps_q1, qT[1]), (ps_k, kT)):
                        raw = rp.tile([128, CH], F32R, tag="qraw")
                        nc.vector.tensor_copy(raw, psrc)
                        ps_sw = trps.tile([128, CH], F32, tag="tr")
                        nc.tensor.matmul(ps_sw, pswap, raw, start=True, stop=True)
                        t2 = rp.tile([128, CH], F32, tag="t2")
                        nc.vector.tensor_tensor(
                            t2, raw.bitcast(F32), cos_t, op=mybir.AluOpType.mult
                        )
                        # sw *= sin in place (PSUM), then add -> rope output
                        nc.vector.tensor_tensor(ps_sw, ps_sw, sin_t, op=mybir.AluOpType.mult)
                        nc.vector.tensor_tensor(
                            dstT[:, cols], t2, ps_sw, op=mybir.AluOpType.add
                        )

                    # V: evacuate then PE-transpose to natural [s, d] layout
                    # (fp32r copy: vN is fp32r anyway, and fp32r transpose runs
                    # 1.5 cyc/row vs 2.0 for fp32)
                    vtmp = rp.tile([128, CH], F32R, tag="qraw2")
                    nc.vector.tensor_copy(vtmp, ps_v)
                    ps_vt = trps.tile([128, CH], F32, tag="tr")
                    for u in range(4):
                        nc.tensor.transpose(
                            ps_vt.bitcast(F32R)[:, u * 128:(u + 1) * 128],
                            vtmp[:, u * 128:(u + 1) * 128],
                            ident,
                        )
                    nc.vector.tensor_copy(
                        vN[:, 4 * m:4 * m + 4, :],
                        ps_vt.rearrange("p (u d) -> p u d", u=4),
                    )

                    # partial block sums for this chunk's 4 key blocks
                    nc.vector.tensor_reduce(
                        bm[:, 4 * m:4 * m + 4],
                        kT.bitcast(F32)[:, cols].rearrange("p (b t) -> p b t", b=4),
                        axis=mybir.AxisListType.X,
                        op=mybir.AluOpType.add,
                    )
                    # gating flags for this chunk's query blocks (needs
                    # bm 0..i); chunks 6-7 are deferred into the attention
                    # phase so the phase boundary is not serialized on them
                    if 2 <= m <= 5:
                        _gating(nc, m, qT, bm, Ft, ident, trps, "g", gp)

            # ---------------- phases 3+4 ------------------------------------
            _phase34(nc, tc, qT, kT, vN, trik, oh2, sel2, ones_r, Ft, wo_d,
                     out_d, bm, ident)

    nc.compile()
    _CACHE[key] = nc
    return nc


def _phase34(nc, tc, qT, kT, vN, trik, oh2, sel2, ones_r, Ft, wo_d, out_d,
             bm, ident):
    wop_cm = tc.tile_pool(name="wop", bufs=1)
    wop = wop_cm.__enter__()
    wo_sb = wop.tile([128, HPC, DIM], F32R, tag="wo")
    nc.gpsimd.dma_start(out=wo_sb, in_=wo_d.rearrange("(t p) d -> p t d", p=128))
    # ------- phase 3: attention with interleaved output projection -------
    # (wo(m) right after attn(m) so the 32MB output DMA spreads over the
    # whole kernel instead of piling into a DMA-bound tail phase)
    # PSUM budget (16KB/partition): pss "s" 2x[128,2,CH] = 8KB, pso "o"
    # 2x[128,CH] + "w" 1x[128,CH] = 6KB, psd "den2" 1x[2,CH] = 2KB.
    # Broadcast scratch and the final wo emission reuse the pss "s" slots.
    # ps_w gets its own tag so the deferred wo matmuls interleave freely into
    # the attention exp-latency gaps instead of queueing behind ps_o's WAR.
    with (
        tc.tile_pool(name="att", bufs=4) as ap,
        tc.tile_pool(name="attb", bufs=2) as ab,
        tc.tile_pool(name="oTs", bufs=4) as otp,
        tc.tile_pool(name="outp", bufs=6) as outp,
        tc.tile_pool(name="att_s", bufs=2, space="PSUM") as pss,
        tc.tile_pool(name="att_o", bufs=2, space="PSUM") as pso,
        tc.tile_pool(name="att_d", bufs=1, space="PSUM") as psd,
    ):
        # ---- wo(m') emission: projection for s-tiles of chunk m' ----
        # Emitted piecewise, one output tile per attention j-iteration of the
        # NEXT chunk, so the wo matmuls fill the PE's exp-latency gaps.
        def wo_pieces(mm, oTc_mm, final=False):
            for u in range(4):
                st = 4 * mm + u
                for n in range(4):
                    ncols = slice(n * 512, (n + 1) * 512)
                    if final:
                        # attention is done: reuse the (free) score slots
                        # for a 2-deep pipelined tail
                        ps_w2 = pss.tile([128, HPC, CH], F32, tag="s", name="psw2")
                        ps_w = ps_w2[:, 0, :]
                    else:
                        ps_w = pso.tile([128, 512], F32, tag="w", name="psw", bufs=1)
                    nc.tensor.matmul(
                        ps_w,
                        oTc_mm[0][:, u * 128:(u + 1) * 128],
                        wo_sb[:, 0, ncols],
                        start=True,
                        stop=False,
                    )
                    nc.tensor.matmul(
                        ps_w,
                        oTc_mm[1][:, u * 128:(u + 1) * 128],
                        wo_sb[:, 1, ncols],
                        start=False,
                        stop=True,
                    )
                    osb = outp.tile([128, 512], F32, tag="ow")
                    if (st * 4 + n) % 3 == 0:
                        nc.scalar.copy(osb, ps_w)
                        nc.sync.dma_start(
                            out=out_d[st * 128:(st + 1) * 128, ncols], in_=osb
                        )
                    else:
                        nc.vector.tensor_copy(osb, ps_w)
                        nc.gpsimd.dma_start(
                            out=out_d[st * 128:(st + 1) * 128, ncols], in_=osb
                        )
                    yield True

        wo_queue = []  # pending wo piece generators (FIFO across chunks)

        def fire_wo(n=1):
            # at most n pieces; a second piece per attention step would stall
            # the in-order PE stream on the single-bank ps_w WAR
            while n > 0 and wo_queue:
                if next(wo_queue[0], None) is None:
                    wo_queue.pop(0)
                else:
                    n -= 1

        for m in range(NCHUNK):
            nsk = 8 if m < 2 else 4 * m + 4
            ps_o = [pso.tile([128, CH], F32, tag="o", name=f"o{h}") for h in range(HPC)]
            oTc = [
                otp.tile([128, CH], F32R, tag="oTc", name=f"oTc{h}")
                for h in range(HPC)
            ]
            # precompute diagonal-mask tiles for this chunk's band (off the
            # exp->PV critical path): one broadcast matmul + one max for both
            # heads and all 4 band blocks at once
            mks = None
            if m >= 2:
                ps_bc = pss.tile([128, HPC, CH], F32, tag="s", name="ps_bc")
                for h in range(HPC):
                    nc.tensor.matmul(
                        ps_bc[:, h, :],
                        ones_r,
                        Ft[:, h, (4 * m - 8) * 128:(4 * m - 4) * 128],
                        start=True,
                        stop=True,
                    )
                mks = ab.tile([128, HPC, CH], F32, tag="mk", bufs=2)
                trik_b = trik.rearrange("p (a b t) -> p a b t", a=1, b=1).broadcast_to(
                    [128, HPC, 4, BLOCK]
                )
                nc.vector.tensor_tensor(
                    mks.rearrange("p h (b t) -> p h b t", b=4),
                    trik_b,
                    ps_bc.rearrange("p h (b t) -> p h b t", b=4),
                    op=mybir.AluOpType.max,
                )
            ps_den = psd.tile([2, CH], F32, tag="den2", name="den2", bufs=1)

            def colspan(j):
                band = m >= 2 and j >= 4 * m
                # pad ap=128 matmuls (4 cyc/row below ap 256) to ap=256
                col0 = (j - 4 * m) * 128 if band else 0
                colp = min(col0, CH - 256) if band else 0
                return band, col0, colp

            def emit_qk_exp(j):
                band, col0, colp = colspan(j)
                ps_s = pss.tile([128, HPC, CH], F32, tag="s", bufs=2)
                for h in range(HPC):
                    nc.tensor.matmul(
                        ps_s[:, h, colp:],
                        kT[:, j * 128:(j + 1) * 128],
                        qT[h][:, m * CH + colp:(m + 1) * CH],
                        start=True,
                        stop=True,
                    )
                if colp < col0:
                    # overwrite the pad region so exp underflows to exact 0
                    nc.vector.memset(ps_s[:, :, colp:col0], PAD_NEG)
                pexp = ap.tile([128, HPC, CH], F32R, tag="pexp", bufs=4)
                nc.scalar.activation(
                    out=pexp[:, :, colp:],
                    in_=ps_s[:, :, colp:],
                    func=mybir.ActivationFunctionType.Exp,
                    scale=INV_SQRT_D,
                )
                if band:
                    nc.vector.tensor_tensor(
                        pexp[:, :, col0:col0 + 128],
                        pexp.bitcast(F32)[:, :, col0:col0 + 128],
                        mks[:, :, col0:col0 + 128],
                        op=mybir.AluOpType.mult,
                    )
                return pexp

            def emit_pv_den(j, pexp):
                _, _, colp = colspan(j)
                for h in range(HPC):
                    nc.tensor.matmul(
                        ps_o[h][:, colp:],
                        vN[:, j, :],
                        pexp[:, h, colp:],
                        start=(j == 0),
                        stop=(j == nsk - 1),
                    )
                    nc.tensor.matmul(
                        ps_den[:, colp:],
                        oh2[:, h, :],
                        pexp[:, h, colp:],
                        start=(j == 0 and h == 0),
                        stop=(j == nsk - 1 and h == HPC - 1),
                    )

            # software-pipeline by one j: QK/exp of j+1 is emitted before
            # PV/den of j, so the PE never sits in-order behind exp latency;
            # wo output tiles of the previous chunk (spread evenly over the
            # j-loop) fill the remaining slack
            pexp_j = emit_qk_exp(0)
            for j in range(nsk):
                if j + 1 < nsk:
                    pexp_n = emit_qk_exp(j + 1)
                else:
                    pexp_n = None
                emit_pv_den(j, pexp_j)
                pexp_j = pexp_n
                # spread the ~16 pending pieces over all nsk steps (still at
                # most one per step), so late js keep their latency filler
                if len(wo_queue) > 1 or j % max(1, nsk // 16) == 0:
                    fire_wo(1)
            fire_wo(1)  # one more piece fills the boundary chain
            # normalisation: reciprocal of the two dens, broadcast via the
            # "w" bank (keeps the "s" slots free so the next chunk's QKs can
            # start during this chain), then scale ps_o into fp32r oTc
            rec2 = ab.tile([2, CH], F32R, tag="rec")
            nc.vector.reciprocal(rec2, ps_den)
            bc_sb = ab.tile([128, HPC, CH], F32, tag="bcs")
            for h in range(HPC):
                ps_rb = pso.tile([128, 512], F32, tag="w", name="ps_rb", bufs=1)
                nc.tensor.matmul(
                    ps_rb, sel2[:, h, :], rec2, start=True, stop=True
                )
                nc.scalar.copy(bc_sb[:, h, :], ps_rb)
            for h in range(HPC):
                nc.vector.tensor_tensor(
                    oTc[h], ps_o[h], bc_sb[:, h, :], op=mybir.AluOpType.mult
                )
            # deferred gating for the last two phase-1 chunks: the PE
            # matmuls double as filler during this chunk's boundary chain
            if m == 2:
                _gating(nc, 6, qT, bm, Ft, ident, pso, "w", ab)
            elif m == 3:
                _gating(nc, 7, qT, bm, Ft, ident, pso, "w", ab)
            wo_queue.append(
                wo_pieces(m, oTc, final=(m == NCHUNK - 1))
            )
        while wo_queue:
            fire_wo(1)
    wop_cm.__exit__(None, None, None)


def kernel(x, wq, wk, wv, wo):
    bs = np.asarray(x).shape[0]
    in_maps = make_in_maps(x, wq, wk, wv, wo)
    nc = _build_nc()
    res = run_bass_kernel_spmd(nc, in_maps, list(range(N_CORES)))
    out = res.results[0]["out"].astype(np.float64)
    for c in range(1, N_CORES):
        out += res.results[c]["out"]
    return out.astype(np.float32).reshape(bs, SEQ, DIM)


if __name__ == "__main__":
    rng = np.random.default_rng(0)
    xs = {
        "x": rng.standard_normal((1, SEQ, DIM), dtype=np.float32),
        "wq": rng.standard_normal((DIM, DIM), dtype=np.float32) * (DIM ** -0.5),
        "wk": rng.standard_normal((DIM, DIM // 2), dtype=np.float32) * (DIM ** -0.5),
        "wv": rng.standard_normal((DIM, DIM // 2), dtype=np.float32) * (DIM ** -0.5),
        "wo": rng.standard_normal((DIM, DIM), dtype=np.float32) * (DIM ** -0.5),
    }
    out = kernel(**xs)
    print("out", out.shape, out.dtype, np.abs(out).max())


# revision 46
# speedup vs baseline: 1.0845x; 1.0027x over previous
"""MixtureOfBlockAttention TRN2 kernel — 8-core head-parallel (TP) Bass/Tile implementation.

Semantics (verified equivalent to the reference, rel err ~2e-2 budget dominated
by top-k near-tie flips caused by fp32r rounding of x/w — irreducible without
dropping fp32r):
the reference mask `maximum(token_mask, causal*NEG_INF)` masks a position iff
it is BOTH future AND in a non-selected block. Consequences:
  - query blocks 0..7 attend to ALL tokens of key blocks 0..7 (dense, no mask);
  - query block i>=8 attends densely to key blocks 0..i-1, and within its own
    (diagonal) block applies strict causal masking ONLY for rows whose own
    block is not among their top-8 gating blocks.
Selection rank for query s in block i (i>=8): own block selected iff
  #{j < i : g[s,j] > g[s,i]} < 8, with g = q . (block sums of roped k)
(positive-scale invariant, so block sums replace means and the 1/sqrt(d)
factor is dropped).

Sharding: 16 query heads / 8 cores = 2 heads per core; KV head c serves both.
wq/wk/wv column-sliced, wo row-sliced; partial outputs summed on host.
Host-side layout prep: x is transposed to xT[c, s] (the PE contracts over the
partition dim, so both matmul operands need c on partitions) and float inputs
are pre-rounded to fp32r; both are pure data-layout transforms.

All big matmuls run in float32r (TF32-like input rounding, fp32 accumulate,
full PE rate at N>=256). Optimizations vs the original baseline (all
numerically neutral except the rope-table recurrence, which shifts the final
rel err by <1e-6 with no top-k flips):
  - per-j exp fused across the 2 heads (one Act instruction, PSUM [128,2,512]);
  - softmax denominators for both heads accumulate into one [2,512] PSUM bank
    via [128,2] one-hot-column stationaries;
  - reciprocal-broadcast and notflag-broadcast matmuls emitted at ap>=512
    instead of 4x/8x ap=128 pieces (fp32r runs 4 cyc/row below ap 256);
  - diagonal-band j=4m+3 matmuls padded from ap=128 to ap=256 with a -1e5
    PSUM memset in the pad so exp underflows to exact 0 there;
  - V-transpose and notflag-transpose in fp32r (1.5 cyc/row) not fp32 (2.0);
  - rope cos/sin tables generated on device by a per-chunk angle-rotation
    recurrence (saves 4MB/core of HBM reads and the phase-1 DMA deficit);
  - attention j-loop software-pipelined by one step (QK/exp of j+1 emitted
    before PV/den of j) so the in-order PE stream hides the exp latency;
  - the deferred wo projection is emitted as 16 per-chunk output tiles fired
    at most one per attention step from a cross-chunk FIFO, filling the PE's
    residual exp-latency slack without ever stalling it on the ps_w WAR;
  - gating reordered (all matmuls, then DVE compare chains, then batched
    transposes) and chunks 6-7's gating deferred into the attention phase;
  - PSUM: scores 2x[128,2,512] + 2 ps_o + 1 ps_w + 1 den2 = exactly 8 banks.
"""

import math
import sys

import numpy as np

if "/opt/trn_rl_repo" not in sys.path:
    sys.path.insert(0, "/opt/trn_rl_repo")

import concourse.bacc as bacc
import concourse.mybir as mybir
import concourse.tile as tile
from concourse.bass_utils import run_bass_kernel_spmd

F32 = mybir.dt.float32
F32R = mybir.dt.float32r

SEQ = 4096
DIM = 2048
HEAD_DIM = 128
N_HEADS = 16
N_CORES = 8
HPC = N_HEADS // N_CORES       # heads per core = 2
DPC = HPC * HEAD_DIM           # q/o dims per core = 256
BLOCK = 128
NB = SEQ // BLOCK              # 32 key blocks
TOPK = 8
NCHUNK = 8                     # s-chunks of 512
CH = SEQ // NCHUNK             # 512
NCT = DIM // 128               # 16 contraction tiles
INV_SQRT_D = 1.0 / math.sqrt(HEAD_DIM)
PAD_NEG = -100000.0

_CACHE = {}


def _round_fp32r(a):
    """Round fp32 to the fp32r grid (top-11-bit mantissa, round-to-nearest)."""
    a = np.ascontiguousarray(a, dtype=np.float32)
    try:
        from neuron_dtypes import static_cast_fp32_to_fp32r

        return static_cast_fp32_to_fp32r(a).view(np.float32).astype(np.float32)
    except Exception:
        u = a.view(np.uint32)
        return ((u + np.uint32(0x800)) & np.uint32(0xFFFFF000)).view(np.float32).copy()


def _host_constants():
    if "consts" in _CACHE:
        return _CACHE["consts"]
    p = np.arange(HEAD_DIM // 2, dtype=np.float64)
    inv_freq = 1.0 / (10000.0 ** (2.0 * p / HEAD_DIM))
    ang = np.arange(SEQ, dtype=np.float64)[None, :] * inv_freq[:, None]  # [64, S]
    cos = np.cos(ang).astype(np.float32)
    sin = np.sin(ang).astype(np.float32)
    cos_ds = np.ascontiguousarray(np.repeat(cos, 2, axis=0))   # [128, S]
    sin_ds = np.empty((HEAD_DIM, SEQ), dtype=np.float32)       # signed sin
    sin_ds[0::2] = -sin
    sin_ds[1::2] = sin
    # per-partition rotation by CH positions: next-chunk tables via
    # cos' = cos*C - sin_ds*S_row ; sin_ds' = sin_ds*C + cos*S_row
    # (S_row carries the sign convention of the interleaved sin_ds rows)
    inv_freq = 1.0 / (10000.0 ** (2.0 * p / HEAD_DIM))
    c512 = np.cos(CH * inv_freq)
    s512 = np.sin(CH * inv_freq)
    rotC = np.repeat(c512, 2).astype(np.float32)[:, None]      # [128, 1]
    rotS = np.empty((HEAD_DIM,), dtype=np.float64)
    rotS[0::2] = -s512
    rotS[1::2] = s512
    rotS = rotS.astype(np.float32)[:, None]                    # [128, 1]
    pswap = np.zeros((128, 128), dtype=np.float32)             # swap 2p <-> 2p+1
    idx = np.arange(128)
    pswap[idx, idx ^ 1] = 1.0
    identm = np.eye(128, dtype=np.float32)
    r = np.arange(BLOCK)
    trikeep = (r[:, None] <= r[None, :]).astype(np.float32)    # keep iff sk <= sq
    ones_row = np.ones((1, 128), dtype=np.float32)
    # one-hot-column stationaries for per-head den accumulation into [2, CH]:
    # oh2[:, h, :] is [128, 2] with column h all-ones
    oh2 = np.zeros((128, 2, 2), dtype=np.float32)
    oh2[:, 0, 0] = 1.0
    oh2[:, 1, 1] = 1.0
    # one-hot-row stationaries for per-head [2,CH] -> [128,CH] broadcast:
    # sel2[:, h, :] is [2, 128] with row h all-ones
    sel2 = np.zeros((2, 2, 128), dtype=np.float32)
    sel2[0, 0, :] = 1.0
    sel2[1, 1, :] = 1.0
    _CACHE["consts"] = (cos_ds, sin_ds, rotC, rotS, pswap, identm, trikeep, ones_row, oh2, sel2)
    return _CACHE["consts"]


def make_in_maps(x, wq, wk, wv, wo):
    """Shard + lay out the full inputs for the 8 cores."""
    x2 = np.asarray(x, dtype=np.float32).reshape(SEQ, DIM)
    xT = _round_fp32r(np.ascontiguousarray(x2.T))
    wq = np.asarray(wq, dtype=np.float32)
    wk = np.asarray(wk, dtype=np.float32)
    wv = np.asarray(wv, dtype=np.float32)
    wo = np.asarray(wo, dtype=np.float32)
    cos_ds, sin_ds, rotC, rotS, pswap, identm, trikeep, ones_row, oh2, sel2 = _host_constants()
    pswap_r = _round_fp32r(pswap)
    ones_row_r = _round_fp32r(ones_row)
    oh2_r = _round_fp32r(oh2)
    sel2_r = _round_fp32r(sel2)
    in_maps = []
    for c in range(N_CORES):
        in_maps.append(
            {
                "xT": xT,
                "wq": _round_fp32r(wq[:, c * DPC:(c + 1) * DPC]),
                "wk": _round_fp32r(wk[:, c * HEAD_DIM:(c + 1) * HEAD_DIM]),
                "wv": _round_fp32r(wv[:, c * HEAD_DIM:(c + 1) * HEAD_DIM]),
                "wo": _round_fp32r(wo[c * DPC:(c + 1) * DPC, :]),
                "cos0": np.ascontiguousarray(cos_ds[:, 0:CH]),
                "sin0": np.ascontiguousarray(sin_ds[:, 0:CH]),
                "rotC": rotC,
                "rotS": rotS,
                "pswap": pswap_r,
                "identm": _round_fp32r(identm),
                "trikeep": trikeep,
                "ones_r": ones_row_r,
                "oh2": oh2_r,
                "sel2": sel2_r,
            }
        )
    return in_maps


def _gating(nc, m, qT, bm, Ft, ident, ps_pool, ps_tag, sb_pool):
    """Own-block top-k flags for chunk m's 4 query blocks (both heads).

    All 8 gating matmuls first, then the DVE compare chains, then the 8
    transposes batched 4-per-PSUM-bank, so the in-order PE stream never
    waits mid-chain. PSUM scratch comes from (ps_pool, ps_tag) so this can
    run late, inside the attention phase, for the last two chunks.
    """
    import concourse.mybir as mybir

    F32 = mybir.dt.float32
    F32R = mybir.dt.float32r
    pairs = [(h, i) for h in range(HPC) for i in range(4 * m, 4 * m + 4)]
    nbk = 4 * m + 4  # even N; cols > i unused
    ps_g8 = ps_pool.tile([128, 8, NB], F32, tag=ps_tag, bufs=1, name="g8")
    for p, (h, i) in enumerate(pairs):
        nc.tensor.matmul(
            ps_g8[:, p, 0:nbk],
            qT[h][:, i * 128:(i + 1) * 128],
            bm[:, 0:nbk],
            start=True,
            stop=True,
        )
    nfs = []
    for p, (h, i) in enumerate(pairs):
        cmp = sb_pool.tile([128, NB], F32, tag="cmp", bufs=2, name="cmp")
        cnt = sb_pool.tile([128, 1], F32, tag="cnt", bufs=2, name="cnt")
        nc.vector.tensor_scalar(
            out=cmp[:, 0:i],
            in0=ps_g8[:, p, 0:i],
            scalar1=ps_g8[:, p, i:i + 1],
            scalar2=None,
            op0=mybir.AluOpType.is_gt,
        )
        nc.vector.tensor_reduce(
            cnt, cmp[:, 0:i], axis=mybir.AxisListType.X, op=mybir.AluOpType.add
        )
        # notflag: 1.0 -> own block selected (keep all)
        # (fp32r tiles: values are exactly 0.0/1.0)
        nf = sb_pool.tile([128, 1], F32R, tag="nf", bufs=8, name=f"nf{p}")
        nc.vector.tensor_scalar(
            out=nf,
            in0=cnt,
            scalar1=float(TOPK) - 0.5,
            scalar2=None,
            op0=mybir.AluOpType.is_lt,
        )
        nfs.append(nf)
    for h in range(HPC):
        ps_ft4 = ps_pool.tile([1, 4, 128], F32, tag=ps_tag, bufs=1, name="ft4")
        for t in range(4):
            nc.tensor.transpose(
                ps_ft4.bitcast(F32R)[:, t, :], nfs[4 * h + t], ident
            )
        nc.vector.tensor_copy(
            Ft[:, h, (4 * m - 8) * 128:(4 * m - 4) * 128],
            ps_ft4.rearrange("o f t -> o (f t)"),
        )



def _build_nc(reps=1):
    key = f"nc{reps}"
    if key in _CACHE:
        return _CACHE[key]
    nc = bacc.Bacc(None, target_bir_lowering=False)

    xT_d = nc.dram_tensor("xT", [DIM, SEQ], F32R, kind="ExternalInput")
    wq_d = nc.dram_tensor("wq", [DIM, DPC], F32R, kind="ExternalInput")
    wk_d = nc.dram_tensor("wk", [DIM, HEAD_DIM], F32R, kind="ExternalInput")
    wv_d = nc.dram_tensor("wv", [DIM, HEAD_DIM], F32R, kind="ExternalInput")
    wo_d = nc.dram_tensor("wo", [DPC, DIM], F32R, kind="ExternalInput")
    cos_d = nc.dram_tensor("cos0", [HEAD_DIM, CH], F32, kind="ExternalInput")
    sin_d = nc.dram_tensor("sin0", [HEAD_DIM, CH], F32, kind="ExternalInput")
    rotc_d = nc.dram_tensor("rotC", [HEAD_DIM, 1], F32, kind="ExternalInput")
    rots_d = nc.dram_tensor("rotS", [HEAD_DIM, 1], F32, kind="ExternalInput")
    psw_d = nc.dram_tensor("pswap", [128, 128], F32R, kind="ExternalInput")
    idm_d = nc.dram_tensor("identm", [128, 128], F32R, kind="ExternalInput")
    trk_d = nc.dram_tensor("trikeep", [BLOCK, BLOCK], F32, kind="ExternalInput")
    onr_d = nc.dram_tensor("ones_r", [1, 128], F32R, kind="ExternalInput")
    oh2_d = nc.dram_tensor("oh2", [128, 2, 2], F32R, kind="ExternalInput")
    sel2_d = nc.dram_tensor("sel2", [2, 2, 128], F32R, kind="ExternalInput")
    out_d = nc.dram_tensor("out", [SEQ, DIM], F32, kind="ExternalOutput")

    with tile.TileContext(nc) as tc, nc.allow_low_precision(
        reason="float32r rounding of matmul operands is intentional"
    ):
      for _rep in range(reps):
        with tc.tile_pool(name="persist", bufs=1) as per:
            qT = [per.tile([128, SEQ], F32R, tag=f"qT{h}", name=f"qT{h}") for h in range(HPC)]
            kT = per.tile([128, SEQ], F32R, tag="kT")
            vN = per.tile([128, NB, 128], F32R, tag="vN")   # [s-in-tile, sk-tile, d]
            ident = per.tile([128, 128], F32R, tag="ident")
            pswap = per.tile([128, 128], F32R, tag="pswap")
            trik = per.tile([BLOCK, BLOCK], F32, tag="trik")
            ones_r = per.tile([1, 128], F32R, tag="ones_r")
            oh2 = per.tile([128, 2, 2], F32R, tag="oh2")    # [k, h, den-col]
            sel2 = per.tile([2, 2, 128], F32R, tag="sel2")  # [den-row, h, p]
            bm = per.tile([128, NB], F32R, tag="bm")
            # per-head notflag rows: Ft[0, h, (i-TOPK)*128:...] is the [1,128]
            # notflag row for query block i of head h, at base partition 0
            Ft = per.tile([1, HPC, (NB - TOPK) * 128], F32R, tag="Ft")

            # dummy exp so the Exp act-table load overlaps the initial weight
            # DMAs instead of stalling the first attention chunk
            warm = per.tile([1, 1], F32, tag="warm")
            nc.vector.memset(warm, 0.0)
            nc.scalar.activation(
                out=warm, in_=warm, func=mybir.ActivationFunctionType.Exp
            )

            # ---------------- phase 1: projections + rope -------------------
            with (
                tc.tile_pool(name="wpool", bufs=1) as wp,
                tc.tile_pool(name="xtp", bufs=17) as xtp,
                tc.tile_pool(name="ropep", bufs=2) as rp,
                tc.tile_pool(name="csin", bufs=2) as csp,
                # acc_ps declared first so its PSUM range lines up with the
                # attention score pool's range: the last acc_ps readers (rope
                # copies) finish well before the gating tail that occupies
                # pj_ps, letting chunk-0 QK matmuls start during the tail
                tc.tile_pool(name="acc_ps", bufs=4, space="PSUM") as accps,
                tc.tile_pool(name="pj_ps", bufs=2, space="PSUM") as trps,
            ):
                wq_sb = wp.tile([128, NCT, DPC], F32R, tag="wq")
                wk_sb = wp.tile([128, NCT, HEAD_DIM], F32R, tag="wk")
                wv_sb = wp.tile([128, NCT, HEAD_DIM], F32R, tag="wv")
                wq_r = wq_d.rearrange("(t p) d -> p t d", p=128)
                wk_r = wk_d.rearrange("(t p) d -> p t d", p=128)
                wv_r = wv_d.rearrange("(t p) d -> p t d", p=128)
                # k/v weights first: chunk 0 runs its k/v matmuls while the
                # (2x bigger) wq still streams in
                nc.gpsimd.dma_start(out=wk_sb[:, 0:4, :], in_=wk_r[:, 0:4, :])
                nc.gpsimd.dma_start(out=wv_sb[:, 0:4, :], in_=wv_r[:, 0:4, :])
                nc.gpsimd.dma_start(out=wk_sb[:, 4:16, :], in_=wk_r[:, 4:16, :])
                nc.gpsimd.dma_start(out=wv_sb[:, 4:16, :], in_=wv_r[:, 4:16, :])
                nc.gpsimd.dma_start(out=wq_sb[:, 0:8, :], in_=wq_r[:, 0:8, :])
                nc.gpsimd.dma_start(out=wq_sb[:, 8:16, :], in_=wq_r[:, 8:16, :])

                rot_c = wp.tile([128, 1], F32, tag="rotc")
                nc.scalar.dma_start(out=rot_c, in_=rotc_d[:])
                rot_s = wp.tile([128, 1], F32, tag="rots")
                nc.scalar.dma_start(out=rot_s, in_=rots_d[:])
                gp = wp  # reuse the bufs=1 pool scope for small gating tiles
                for m in range(NCHUNK):
                    cols = slice(m * CH, (m + 1) * CH)
                    ps_q0 = accps.tile([128, CH], F32, tag="acc")
                    ps_q1 = accps.tile([128, CH], F32, tag="acc")
                    ps_k = accps.tile([128, CH], F32, tag="acc")
                    ps_v = accps.tile([128, CH], F32, tag="acc")
                    if m == 0:
                        # two passes (k/v then q) to match the weight-arrival
                        # order; the xt tiles stay resident for the q pass
                        xts = []
                        for cc in range(NCT):
                            xt = xtp.tile([128, CH], F32R, tag="xt")
                            # first tiles ride the otherwise-idle Act HWDGE
                            # queue in parallel with the sync queue
                            q = nc.scalar if cc < 4 else nc.sync
                            q.dma_start(
                                out=xt, in_=xT_d[cc * 128:(cc + 1) * 128, cols]
                            )
                            xts.append(xt)
                            st0, sp0 = (cc == 0), (cc == NCT - 1)
                            nc.tensor.matmul(ps_k, wk_sb[:, cc, :], xt, start=st0, stop=sp0)
                            nc.tensor.matmul(ps_v, wv_sb[:, cc, :], xt, start=st0, stop=sp0)
                        # consts (needed from the rope stage onwards) follow
                        # the early x tiles on the Act HWDGE queue
                        nc.scalar.dma_start(out=pswap, in_=psw_d[:])
                        nc.scalar.dma_start(out=ident, in_=idm_d[:])
                        nc.scalar.dma_start(out=trik, in_=trk_d[:])
                        nc.scalar.dma_start(out=ones_r, in_=onr_d[:])
                        nc.scalar.dma_start(out=oh2, in_=oh2_d[:])
                        nc.scalar.dma_start(out=sel2, in_=sel2_d[:])
                        for cc in range(NCT):
                            st0, sp0 = (cc == 0), (cc == NCT - 1)
                            nc.tensor.matmul(ps_q0, wq_sb[:, cc, 0:128], xts[cc], start=st0, stop=sp0)
                            nc.tensor.matmul(ps_q1, wq_sb[:, cc, 128:256], xts[cc], start=st0, stop=sp0)
                    else:
                      for cc in range(NCT):
                        xt = xtp.tile([128, CH], F32R, tag="xt")
                        nc.sync.dma_start(
                            out=xt, in_=xT_d[cc * 128:(cc + 1) * 128, cols]
                        )
                        st0, sp0 = (cc == 0), (cc == NCT - 1)
                        nc.tensor.matmul(ps_q0, wq_sb[:, cc, 0:128], xt, start=st0, stop=sp0)
                        nc.tensor.matmul(ps_q1, wq_sb[:, cc, 128:256], xt, start=st0, stop=sp0)
                        nc.tensor.matmul(ps_k, wk_sb[:, cc, :], xt, start=st0, stop=sp0)
                        nc.tensor.matmul(ps_v, wv_sb[:, cc, :], xt, start=st0, stop=sp0)

                    if m == 0:
                        cos_t = csp.tile([128, CH], F32, tag="cos", bufs=2)
                        nc.scalar.dma_start(out=cos_t, in_=cos_d[:])
                        sin_t = csp.tile([128, CH], F32, tag="sin", bufs=2)
                        nc.scalar.dma_start(out=sin_t, in_=sin_d[:])
                    else:
                        # rotate the previous chunk's tables by CH positions
                        # (per-partition angle), off the DMA wire entirely
                        cos_p, sin_p = cos_t, sin_t
                        ta = rp.tile([128, CH], F32, tag="t2")
                        nc.vector.tensor_scalar(
                            out=ta, in0=sin_p, scalar1=rot_s, scalar2=None,
                            op0=mybir.AluOpType.mult,
                        )
                        cos_t = csp.tile([128, CH], F32, tag="cos", bufs=2)
                        nc.vector.scalar_tensor_tensor(
                            out=cos_t, in0=cos_p, scalar=rot_c, in1=ta,
                            op0=mybir.AluOpType.mult,
                            op1=mybir.AluOpType.subtract,
                        )
                        tb = rp.tile([128, CH], F32, tag="t2")
                        nc.vector.tensor_scalar(
                            out=tb, in0=cos_p, scalar1=rot_s, scalar2=None,
                            op0=mybir.AluOpType.mult,
                        )
                        sin_t = csp.tile([128, CH], F32, tag="sin", bufs=2)
                        nc.vector.scalar_tensor_tensor(
                            out=sin_t, in0=sin_p, scalar=rot_c, in1=tb,
                            op0=mybir.AluOpType.mult,
                            op1=mybir.AluOpType.add,
                        )

                    for psrc, dstT in ((ps_q0, qT[0]), (ps_q1, qT[1]), (ps_k, kT)):
                        raw = rp.tile([128, CH], F32R, tag="qraw")
                        nc.vector.tensor_copy(raw, psrc)
                        ps_sw = trps.tile([128, CH], F32, tag="tr")
                        nc.tensor.matmul(ps_sw, pswap, raw, start=True, stop=True)
                        t2 = rp.tile([128, CH], F32, tag="t2")
                        nc.vector.tensor_tensor(
                            t2, raw.bitcast(F32), cos_t, op=mybir.AluOpType.mult
                        )
                        # sw *= sin in place (PSUM), then add -> rope output
                        nc.vector.tensor_tensor(ps_sw, ps_sw, sin_t, op=mybir.AluOpType.mult)
                        nc.vector.tensor_tensor(
                            dstT[:, cols], t2, ps_sw, op=mybir.AluOpType.add
                        )

                    # V: evacuate then PE-transpose to natural [s, d] layout
                    # (fp32r copy: vN is fp32r anyway, and fp32r transpose runs
                    # 1.5 cyc/row vs 2.0 for fp32)
                    vtmp = rp.tile([128, CH], F32R, tag="qraw2")
                    nc.vector.tensor_copy(vtmp, ps_v)
                    ps_vt = trps.tile([128, CH], F32, tag="tr")
                    for u in range(4):
                        nc.tensor.transpose(
                            ps_vt.bitcast(F32R)[:, u * 128:(u + 1) * 128],
                            vtmp[:, u * 128:(u + 1) * 128],
                            ident,
                        )
                    nc.vector.tensor_copy(
                        vN[:, 4 * m:4 * m + 4, :],
                        ps_vt.rearrange("p (u d) -> p u d", u=4),
                    )

                    # partial block sums for this chunk's 4 key blocks
                    nc.vector.tensor_reduce(
                        bm[:, 4 * m:4 * m + 4],
                        kT.bitcast(F32)[:, cols].rearrange("p (b t) -> p b t", b=4),
                        axis=mybir.AxisListType.X,
                        op=mybir.AluOpType.add,
                    )
                    # gating flags for this chunk's query blocks (needs
                    # bm 0..i); chunks 6-7 are deferred into the attention
                    # phase so the phase boundary is not serialized on them
                    if 2 <= m <= 5:
                        _gating(nc, m, qT, bm, Ft, ident, trps, "g", gp)

            # ---------------- phases 3+4 ------------------------------------
            _phase34(nc, tc, qT, kT, vN, trik, oh2, sel2, ones_r, Ft, wo_d,
                     out_d, bm, ident)

    nc.compile()
    _CACHE[key] = nc
    return nc


def _phase34(nc, tc, qT, kT, vN, trik, oh2, sel2, ones_r, Ft, wo_d, out_d,
             bm, ident):
    wop_cm = tc.tile_pool(name="wop", bufs=1)
    wop = wop_cm.__enter__()
    wo_sb = wop.tile([128, HPC, DIM], F32R, tag="wo")
    nc.gpsimd.dma_start(out=wo_sb, in_=wo_d.rearrange("(t p) d -> p t d", p=128))
    # ------- phase 3: attention with interleaved output projection -------
    # (wo(m) right after attn(m) so the 32MB output DMA spreads over the
    # whole kernel instead of piling into a DMA-bound tail phase)
    # PSUM budget (16KB/partition): pss "s" 2x[128,2,CH] = 8KB, pso "o"
    # 2x[128,CH] + "w" 1x[128,CH] = 6KB, psd "den2" 1x[2,CH] = 2KB.
    # Broadcast scratch and the final wo emission reuse the pss "s" slots.
    # ps_w gets its own tag so the deferred wo matmuls interleave freely into
    # the attention exp-latency gaps instead of queueing behind ps_o's WAR.
    with (
        tc.tile_pool(name="att", bufs=4) as ap,
        tc.tile_pool(name="attb", bufs=2) as ab,
        tc.tile_pool(name="oTs", bufs=4) as otp,
        tc.tile_pool(name="outp", bufs=6) as outp,
        tc.tile_pool(name="att_s", bufs=2, space="PSUM") as pss,
        tc.tile_pool(name="att_o", bufs=2, space="PSUM") as pso,
        tc.tile_pool(name="att_d", bufs=1, space="PSUM") as psd,
    ):
        # ---- wo(m') emission: projection for s-tiles of chunk m' ----
        # Emitted piecewise, one output tile per attention j-iteration of the
        # NEXT chunk, so the wo matmuls fill the PE's exp-latency gaps.
        def wo_pieces(mm, oTc_mm, final=False):
            for u in range(4):
                st = 4 * mm + u
                for n in range(4):
                    ncols = slice(n * 512, (n + 1) * 512)
                    if final:
                        # attention is done: reuse the (free) score slots
                        # for a 2-deep pipelined tail
                        ps_w2 = pss.tile([128, HPC, CH], F32, tag="s", name="psw2")
                        ps_w = ps_w2[:, 0, :]
                    else:
                        ps_w = pso.tile([128, 512], F32, tag="w", name="psw", bufs=1)
                    nc.tensor.matmul(
                        ps_w,
                        oTc_mm[0][:, u * 128:(u + 1) * 128],
                        wo_sb[:, 0, ncols],
                        start=True,
                        stop=False,
                    )
                    nc.tensor.matmul(
                        ps_w,
                        oTc_mm[1][:, u * 128:(u + 1) * 128],
                        wo_sb[:, 1, ncols],
                        start=False,
                        stop=True,
                    )
                    osb = outp.tile([128, 512], F32, tag="ow")
                    if (st * 4 + n) % 3 == 0:
                        nc.scalar.copy(osb, ps_w)
                        nc.sync.dma_start(
                            out=out_d[st * 128:(st + 1) * 128, ncols], in_=osb
                        )
                    else:
                        nc.vector.tensor_copy(osb, ps_w)
                        nc.gpsimd.dma_start(
                            out=out_d[st * 128:(st + 1) * 128, ncols], in_=osb
                        )
                    yield True

        wo_queue = []  # pending wo piece generators (FIFO across chunks)

        def fire_wo(n=1):
            # at most n pieces; a second piece per attention step would stall
            # the in-order PE stream on the single-bank ps_w WAR
            while n > 0 and wo_queue:
                if next(wo_queue[0], None) is None:
                    wo_queue.pop(0)
                else:
                    n -= 1

        for m in range(NCHUNK):
            nsk = 8 if m < 2 else 4 * m + 4
            ps_o = [pso.tile([128, CH], F32, tag="o", name=f"o{h}") for h in range(HPC)]
            oTc = [
                otp.tile([128, CH], F32R, tag="oTc", name=f"oTc{h}")
                for h in range(HPC)
            ]
            # precompute diagonal-mask tiles for this chunk's band (off the
            # exp->PV critical path): one broadcast matmul + one max for both
            # heads and all 4 band blocks at once
            mks = None
            if m >= 2:
                ps_bc = pss.tile([128, HPC, CH], F32, tag="s", name="ps_bc")
                for h in range(HPC):
                    nc.tensor.matmul(
                        ps_bc[:, h, :],
                        ones_r,
                        Ft[:, h, (4 * m - 8) * 128:(4 * m - 4) * 128],
                        start=True,
                        stop=True,
                    )
                mks = ab.tile([128, HPC, CH], F32, tag="mk", bufs=2)
                trik_b = trik.rearrange("p (a b t) -> p a b t", a=1, b=1).broadcast_to(
                    [128, HPC, 4, BLOCK]
                )
                nc.vector.tensor_tensor(
                    mks.rearrange("p h (b t) -> p h b t", b=4),
                    trik_b,
                    ps_bc.rearrange("p h (b t) -> p h b t", b=4),
                    op=mybir.AluOpType.max,
                )
            ps_den = psd.tile([2, CH], F32, tag="den2", name="den2", bufs=1)

            def colspan(j):
                band = m >= 2 and j >= 4 * m
                # pad ap=128 matmuls (4 cyc/row below ap 256) to ap=256
                col0 = (j - 4 * m) * 128 if band else 0
                colp = min(col0, CH - 256) if band else 0
                return band, col0, colp

            def emit_qk_exp(j):
                band, col0, colp = colspan(j)
                ps_s = pss.tile([128, HPC, CH], F32, tag="s", bufs=2)
                for h in range(HPC):
                    nc.tensor.matmul(
                        ps_s[:, h, colp:],
                        kT[:, j * 128:(j + 1) * 128],
                        qT[h][:, m * CH + colp:(m + 1) * CH],
                        start=True,
                        stop=True,
                    )
                if colp < col0:
                    # overwrite the pad region so exp underflows to exact 0
                    nc.vector.memset(ps_s[:, :, colp:col0], PAD_NEG)
                pexp = ap.tile([128, HPC, CH], F32R, tag="pexp", bufs=4)
                nc.scalar.activation(
                    out=pexp[:, :, colp:],
                    in_=ps_s[:, :, colp:],
                    func=mybir.ActivationFunctionType.Exp,
                    scale=INV_SQRT_D,
                )
                if band:
                    nc.vector.tensor_tensor(
                        pexp[:, :, col0:col0 + 128],
                        pexp.bitcast(F32)[:, :, col0:col0 + 128],
                        mks[:, :, col0:col0 + 128],
                        op=mybir.AluOpType.mult,
                    )
                return pexp

            def emit_pv_den(j, pexp):
                _, _, colp = colspan(j)
                for h in range(HPC):
                    nc.tensor.matmul(
                        ps_o[h][:, colp:],
                        vN[:, j, :],
                        pexp[:, h, colp:],
                        start=(j == 0),
                        stop=(j == nsk - 1),
                    )
                    nc.tensor.matmul(
                        ps_den[:, colp:],
                        oh2[:, h, :],
                        pexp[:, h, colp:],
                        start=(j == 0 and h == 0),
                        stop=(j == nsk - 1 and h == HPC - 1),
                    )

            # software-pipeline by two j: QK/exp of j+1/j+2 are emitted before
            # PV/den of j, so the PE never sits in-order behind exp latency;
            # wo output tiles of the previous chunk (spread evenly over the
            # j-loop) fill the remaining slack
            pexps = [emit_qk_exp(0)]
            if nsk > 1:
                pexps.append(emit_qk_exp(1))
            for j in range(nsk):
                if j + 2 < nsk:
                    pexps.append(emit_qk_exp(j + 2))
                emit_pv_den(j, pexps[j])
                pexps[j] = None  # release reference
                # spread the ~16 pending pieces over all nsk steps (still at
                # most one per step), so late js keep their latency filler
                if len(wo_queue) > 1 or j % max(1, nsk // 16) == 0:
                    fire_wo(1)
            fire_wo(1)  # one more piece fills the boundary chain
            # normalisation: reciprocal of the two dens, broadcast via the
            # "w" bank (keeps the "s" slots free so the next chunk's QKs can
            # start during this chain), then scale ps_o into fp32r oTc
            rec2 = ab.tile([2, CH], F32R, tag="rec")
            nc.vector.reciprocal(rec2, ps_den)
            bc_sb = ab.tile([128, HPC, CH], F32, tag="bcs")
            for h in range(HPC):
                ps_rb = pso.tile([128, 512], F32, tag="w", name="ps_rb", bufs=1)
                nc.tensor.matmul(
                    ps_rb, sel2[:, h, :], rec2, start=True, stop=True
                )
                nc.scalar.copy(bc_sb[:, h, :], ps_rb)
            for h in range(HPC):
                nc.vector.tensor_tensor(
                    oTc[h], ps_o[h], bc_sb[:, h, :], op=mybir.AluOpType.mult
                )
            # deferred gating for the last two phase-1 chunks: the PE
            # matmuls double as filler during this chunk's boundary chain
            if m == 2:
                _gating(nc, 6, qT, bm, Ft, ident, pso, "w", ab)
            elif m == 3:
                _gating(nc, 7, qT, bm, Ft, ident, pso, "w", ab)
            wo_queue.append(
                wo_pieces(m, oTc, final=(m == NCHUNK - 1))
            )
        while wo_queue:
            fire_wo(1)
    wop_cm.__exit__(None, None, None)


def kernel(x, wq, wk, wv, wo):
    bs = np.asarray(x).shape[0]
    in_maps = make_in_maps(x, wq, wk, wv, wo)
    nc = _build_nc()
    res = run_bass_kernel_spmd(nc, in_maps, list(range(N_CORES)))
    out = res.results[0]["out"].astype(np.float64)
    for c in range(1, N_CORES):
        out += res.results[c]["out"]
    return out.astype(np.float32).reshape(bs, SEQ, DIM)


if __name__ == "__main__":
    rng = np.random.default_rng(0)
    xs = {
        "x": rng.standard_normal((1, SEQ, DIM), dtype=np.float32),
        "wq": rng.standard_normal((DIM, DIM), dtype=np.float32) * (DIM ** -0.5),
        "wk": rng.standard_normal((DIM, DIM // 2), dtype=np.float32) * (DIM ** -0.5),
        "wv": rng.standard_normal((DIM, DIM // 2), dtype=np.float32) * (DIM ** -0.5),
        "wo": rng.standard_normal((DIM, DIM), dtype=np.float32) * (DIM ** -0.5),
    }
    out = kernel(**xs)
    print("out", out.shape, out.dtype, np.abs(out).max())


# revision 49
# speedup vs baseline: 1.0910x; 1.0061x over previous
"""MixtureOfBlockAttention TRN2 kernel — 8-core head-parallel (TP) Bass/Tile implementation.

Semantics (verified equivalent to the reference, rel err ~2e-2 budget dominated
by top-k near-tie flips caused by fp32r rounding of x/w — irreducible without
dropping fp32r):
the reference mask `maximum(token_mask, causal*NEG_INF)` masks a position iff
it is BOTH future AND in a non-selected block. Consequences:
  - query blocks 0..7 attend to ALL tokens of key blocks 0..7 (dense, no mask);
  - query block i>=8 attends densely to key blocks 0..i-1, and within its own
    (diagonal) block applies strict causal masking ONLY for rows whose own
    block is not among their top-8 gating blocks.
Selection rank for query s in block i (i>=8): own block selected iff
  #{j < i : g[s,j] > g[s,i]} < 8, with g = q . (block sums of roped k)
(positive-scale invariant, so block sums replace means and the 1/sqrt(d)
factor is dropped).

Sharding: 16 query heads / 8 cores = 2 heads per core; KV head c serves both.
wq/wk/wv column-sliced, wo row-sliced; partial outputs summed on host.
Host-side layout prep: x is transposed to xT[c, s] (the PE contracts over the
partition dim, so both matmul operands need c on partitions) and float inputs
are pre-rounded to fp32r; both are pure data-layout transforms.

All big matmuls run in float32r (TF32-like input rounding, fp32 accumulate,
full PE rate at N>=256). Optimizations vs the original baseline (all
numerically neutral except the rope-table recurrence, which shifts the final
rel err by <1e-6 with no top-k flips):
  - per-j exp fused across the 2 heads (one Act instruction, PSUM [128,2,512]);
  - softmax denominators for both heads accumulate into one [2,512] PSUM bank
    via [128,2] one-hot-column stationaries;
  - reciprocal-broadcast and notflag-broadcast matmuls emitted at ap>=512
    instead of 4x/8x ap=128 pieces (fp32r runs 4 cyc/row below ap 256);
  - diagonal-band j=4m+3 matmuls padded from ap=128 to ap=256 with a -1e5
    PSUM memset in the pad so exp underflows to exact 0 there;
  - V-transpose and notflag-transpose in fp32r (1.5 cyc/row) not fp32 (2.0);
  - rope cos/sin tables generated on device by a per-chunk angle-rotation
    recurrence (saves 4MB/core of HBM reads and the phase-1 DMA deficit);
  - attention j-loop software-pipelined by one step (QK/exp of j+1 emitted
    before PV/den of j) so the in-order PE stream hides the exp latency;
  - the deferred wo projection is emitted as 16 per-chunk output tiles fired
    at most one per attention step from a cross-chunk FIFO, filling the PE's
    residual exp-latency slack without ever stalling it on the ps_w WAR;
  - gating reordered (all matmuls, then DVE compare chains, then batched
    transposes) and chunks 6-7's gating deferred into the attention phase;
  - PSUM: scores 2x[128,2,512] + 2 ps_o + 1 ps_w + 1 den2 = exactly 8 banks.
"""

import math
import sys

import numpy as np

if "/opt/trn_rl_repo" not in sys.path:
    sys.path.insert(0, "/opt/trn_rl_repo")

import concourse.bacc as bacc
import concourse.mybir as mybir
import concourse.tile as tile
from concourse.bass_utils import run_bass_kernel_spmd

F32 = mybir.dt.float32
F32R = mybir.dt.float32r

SEQ = 4096
DIM = 2048
HEAD_DIM = 128
N_HEADS = 16
N_CORES = 8
HPC = N_HEADS // N_CORES       # heads per core = 2
DPC = HPC * HEAD_DIM           # q/o dims per core = 256
BLOCK = 128
NB = SEQ // BLOCK              # 32 key blocks
TOPK = 8
NCHUNK = 8                     # s-chunks of 512
CH = SEQ // NCHUNK             # 512
NCT = DIM // 128               # 16 contraction tiles
INV_SQRT_D = 1.0 / math.sqrt(HEAD_DIM)
PAD_NEG = -100000.0

_CACHE = {}


def _round_fp32r(a):
    """Round fp32 to the fp32r grid (top-11-bit mantissa, round-to-nearest)."""
    a = np.ascontiguousarray(a, dtype=np.float32)
    try:
        from neuron_dtypes import static_cast_fp32_to_fp32r

        return static_cast_fp32_to_fp32r(a).view(np.float32).astype(np.float32)
    except Exception:
        u = a.view(np.uint32)
        return ((u + np.uint32(0x800)) & np.uint32(0xFFFFF000)).view(np.float32).copy()


def _host_constants():
    if "consts" in _CACHE:
        return _CACHE["consts"]
    p = np.arange(HEAD_DIM // 2, dtype=np.float64)
    inv_freq = 1.0 / (10000.0 ** (2.0 * p / HEAD_DIM))
    ang = np.arange(SEQ, dtype=np.float64)[None, :] * inv_freq[:, None]  # [64, S]
    cos = np.cos(ang).astype(np.float32)
    sin = np.sin(ang).astype(np.float32)
    cos_ds = np.ascontiguousarray(np.repeat(cos, 2, axis=0))   # [128, S]
    sin_ds = np.empty((HEAD_DIM, SEQ), dtype=np.float32)       # signed sin
    sin_ds[0::2] = -sin
    sin_ds[1::2] = sin
    # per-partition rotation by CH positions: next-chunk tables via
    # cos' = cos*C - sin_ds*S_row ; sin_ds' = sin_ds*C + cos*S_row
    # (S_row carries the sign convention of the interleaved sin_ds rows)
    inv_freq = 1.0 / (10000.0 ** (2.0 * p / HEAD_DIM))
    c512 = np.cos(CH * inv_freq)
    s512 = np.sin(CH * inv_freq)
    rotC = np.repeat(c512, 2).astype(np.float32)[:, None]      # [128, 1]
    rotS = np.empty((HEAD_DIM,), dtype=np.float64)
    rotS[0::2] = -s512
    rotS[1::2] = s512
    rotS = rotS.astype(np.float32)[:, None]                    # [128, 1]
    pswap = np.zeros((128, 128), dtype=np.float32)             # swap 2p <-> 2p+1
    idx = np.arange(128)
    pswap[idx, idx ^ 1] = 1.0
    identm = np.eye(128, dtype=np.float32)
    r = np.arange(BLOCK)
    trikeep = (r[:, None] <= r[None, :]).astype(np.float32)    # keep iff sk <= sq
    ones_row = np.ones((1, 128), dtype=np.float32)
    # one-hot-column stationaries for per-head den accumulation into [2, CH]:
    # oh2[:, h, :] is [128, 2] with column h all-ones
    oh2 = np.zeros((128, 2, 2), dtype=np.float32)
    oh2[:, 0, 0] = 1.0
    oh2[:, 1, 1] = 1.0
    # one-hot-row stationaries for per-head [2,CH] -> [128,CH] broadcast:
    # sel2[:, h, :] is [2, 128] with row h all-ones
    sel2 = np.zeros((2, 2, 128), dtype=np.float32)
    sel2[0, 0, :] = 1.0
    sel2[1, 1, :] = 1.0
    _CACHE["consts"] = (cos_ds, sin_ds, rotC, rotS, pswap, identm, trikeep, ones_row, oh2, sel2)
    return _CACHE["consts"]


def make_in_maps(x, wq, wk, wv, wo):
    """Shard + lay out the full inputs for the 8 cores."""
    x2 = np.asarray(x, dtype=np.float32).reshape(SEQ, DIM)
    xT = _round_fp32r(np.ascontiguousarray(x2.T))
    wq = np.asarray(wq, dtype=np.float32)
    wk = np.asarray(wk, dtype=np.float32)
    wv = np.asarray(wv, dtype=np.float32)
    wo = np.asarray(wo, dtype=np.float32)
    cos_ds, sin_ds, rotC, rotS, pswap, identm, trikeep, ones_row, oh2, sel2 = _host_constants()
    pswap_r = _round_fp32r(pswap)
    ones_row_r = _round_fp32r(ones_row)
    oh2_r = _round_fp32r(oh2)
    sel2_r = _round_fp32r(sel2)
    in_maps = []
    for c in range(N_CORES):
        in_maps.append(
            {
                "xT": xT,
                "wq": _round_fp32r(wq[:, c * DPC:(c + 1) * DPC]),
                "wk": _round_fp32r(wk[:, c * HEAD_DIM:(c + 1) * HEAD_DIM]),
                "wv": _round_fp32r(wv[:, c * HEAD_DIM:(c + 1) * HEAD_DIM]),
                "wo": _round_fp32r(wo[c * DPC:(c + 1) * DPC, :]),
                "cos0": np.ascontiguousarray(cos_ds[:, 0:CH]),
                "sin0": np.ascontiguousarray(sin_ds[:, 0:CH]),
                "rotC": rotC,
                "rotS": rotS,
                "pswap": pswap_r,
                "identm": _round_fp32r(identm),
                "trikeep": trikeep,
                "ones_r": ones_row_r,
                "oh2": oh2_r,
                "sel2": sel2_r,
            }
        )
    return in_maps


def _gating(nc, m, qT, bm, Ft, ident, ps_pool, ps_tag, sb_pool):
    """Own-block top-k flags for chunk m's 4 query blocks (both heads).

    All 8 gating matmuls first, then the DVE compare chains, then the 8
    transposes batched 4-per-PSUM-bank, so the in-order PE stream never
    waits mid-chain. PSUM scratch comes from (ps_pool, ps_tag) so this can
    run late, inside the attention phase, for the last two chunks.
    """
    import concourse.mybir as mybir

    F32 = mybir.dt.float32
    F32R = mybir.dt.float32r
    pairs = [(h, i) for h in range(HPC) for i in range(4 * m, 4 * m + 4)]
    nbk = 4 * m + 4  # even N; cols > i unused
    ps_g8 = ps_pool.tile([128, 8, NB], F32, tag=ps_tag, bufs=1, name="g8")
    for p, (h, i) in enumerate(pairs):
        nc.tensor.matmul(
            ps_g8[:, p, 0:nbk],
            qT[h][:, i * 128:(i + 1) * 128],
            bm[:, 0:nbk],
            start=True,
            stop=True,
        )
    nfs = []
    for p, (h, i) in enumerate(pairs):
        cmp = sb_pool.tile([128, NB], F32, tag="cmp", bufs=2, name="cmp")
        cnt = sb_pool.tile([128, 1], F32, tag="cnt", bufs=2, name="cnt")
        nc.vector.tensor_scalar(
            out=cmp[:, 0:i],
            in0=ps_g8[:, p, 0:i],
            scalar1=ps_g8[:, p, i:i + 1],
            scalar2=None,
            op0=mybir.AluOpType.is_gt,
        )
        nc.vector.tensor_reduce(
            cnt, cmp[:, 0:i], axis=mybir.AxisListType.X, op=mybir.AluOpType.add
        )
        # notflag: 1.0 -> own block selected (keep all)
        # (fp32r tiles: values are exactly 0.0/1.0)
        nf = sb_pool.tile([128, 1], F32R, tag="nf", bufs=8, name=f"nf{p}")
        nc.vector.tensor_scalar(
            out=nf,
            in0=cnt,
            scalar1=float(TOPK) - 0.5,
            scalar2=None,
            op0=mybir.AluOpType.is_lt,
        )
        nfs.append(nf)
    for h in range(HPC):
        ps_ft4 = ps_pool.tile([1, 4, 128], F32, tag=ps_tag, bufs=1, name="ft4")
        for t in range(4):
            nc.tensor.transpose(
                ps_ft4.bitcast(F32R)[:, t, :], nfs[4 * h + t], ident
            )
        nc.vector.tensor_copy(
            Ft[:, h, (4 * m - 8) * 128:(4 * m - 4) * 128],
            ps_ft4.rearrange("o f t -> o (f t)"),
        )



def _build_nc(reps=1):
    key = f"nc{reps}"
    if key in _CACHE:
        return _CACHE[key]
    nc = bacc.Bacc(None, target_bir_lowering=False)

    xT_d = nc.dram_tensor("xT", [DIM, SEQ], F32R, kind="ExternalInput")
    wq_d = nc.dram_tensor("wq", [DIM, DPC], F32R, kind="ExternalInput")
    wk_d = nc.dram_tensor("wk", [DIM, HEAD_DIM], F32R, kind="ExternalInput")
    wv_d = nc.dram_tensor("wv", [DIM, HEAD_DIM], F32R, kind="ExternalInput")
    wo_d = nc.dram_tensor("wo", [DPC, DIM], F32R, kind="ExternalInput")
    cos_d = nc.dram_tensor("cos0", [HEAD_DIM, CH], F32, kind="ExternalInput")
    sin_d = nc.dram_tensor("sin0", [HEAD_DIM, CH], F32, kind="ExternalInput")
    rotc_d = nc.dram_tensor("rotC", [HEAD_DIM, 1], F32, kind="ExternalInput")
    rots_d = nc.dram_tensor("rotS", [HEAD_DIM, 1], F32, kind="ExternalInput")
    psw_d = nc.dram_tensor("pswap", [128, 128], F32R, kind="ExternalInput")
    idm_d = nc.dram_tensor("identm", [128, 128], F32R, kind="ExternalInput")
    trk_d = nc.dram_tensor("trikeep", [BLOCK, BLOCK], F32, kind="ExternalInput")
    onr_d = nc.dram_tensor("ones_r", [1, 128], F32R, kind="ExternalInput")
    oh2_d = nc.dram_tensor("oh2", [128, 2, 2], F32R, kind="ExternalInput")
    sel2_d = nc.dram_tensor("sel2", [2, 2, 128], F32R, kind="ExternalInput")
    out_d = nc.dram_tensor("out", [SEQ, DIM], F32, kind="ExternalOutput")

    with tile.TileContext(nc) as tc, nc.allow_low_precision(
        reason="float32r rounding of matmul operands is intentional"
    ):
      for _rep in range(reps):
        with tc.tile_pool(name="persist", bufs=1) as per:
            qT = [per.tile([128, SEQ], F32R, tag=f"qT{h}", name=f"qT{h}") for h in range(HPC)]
            kT = per.tile([128, SEQ], F32R, tag="kT")
            vN = per.tile([128, NB, 128], F32R, tag="vN")   # [s-in-tile, sk-tile, d]
            ident = per.tile([128, 128], F32R, tag="ident")
            pswap = per.tile([128, 128], F32R, tag="pswap")
            trik = per.tile([BLOCK, BLOCK], F32, tag="trik")
            ones_r = per.tile([1, 128], F32R, tag="ones_r")
            oh2 = per.tile([128, 2, 2], F32R, tag="oh2")    # [k, h, den-col]
            sel2 = per.tile([2, 2, 128], F32R, tag="sel2")  # [den-row, h, p]
            bm = per.tile([128, NB], F32R, tag="bm")
            # per-head notflag rows: Ft[0, h, (i-TOPK)*128:...] is the [1,128]
            # notflag row for query block i of head h, at base partition 0
            Ft = per.tile([1, HPC, (NB - TOPK) * 128], F32R, tag="Ft")

            # dummy exp so the Exp act-table load overlaps the initial weight
            # DMAs instead of stalling the first attention chunk
            warm = per.tile([1, 1], F32, tag="warm")
            nc.vector.memset(warm, 0.0)
            nc.scalar.activation(
                out=warm, in_=warm, func=mybir.ActivationFunctionType.Exp
            )

            # ---------------- phase 1: projections + rope -------------------
            with (
                tc.tile_pool(name="wpool", bufs=1) as wp,
                tc.tile_pool(name="xtp", bufs=17) as xtp,
                tc.tile_pool(name="ropep", bufs=2) as rp,
                tc.tile_pool(name="csin", bufs=2) as csp,
                # acc_ps declared first so its PSUM range lines up with the
                # attention score pool's range: the last acc_ps readers (rope
                # copies) finish well before the gating tail that occupies
                # pj_ps, letting chunk-0 QK matmuls start during the tail
                tc.tile_pool(name="acc_ps", bufs=4, space="PSUM") as accps,
                tc.tile_pool(name="pj_ps", bufs=2, space="PSUM") as trps,
            ):
                wq_sb = wp.tile([128, NCT, DPC], F32R, tag="wq")
                wk_sb = wp.tile([128, NCT, HEAD_DIM], F32R, tag="wk")
                wv_sb = wp.tile([128, NCT, HEAD_DIM], F32R, tag="wv")
                wq_r = wq_d.rearrange("(t p) d -> p t d", p=128)
                wk_r = wk_d.rearrange("(t p) d -> p t d", p=128)
                wv_r = wv_d.rearrange("(t p) d -> p t d", p=128)
                # k/v weights first: chunk 0 runs its k/v matmuls while the
                # (2x bigger) wq still streams in
                nc.gpsimd.dma_start(out=wk_sb[:, 0:4, :], in_=wk_r[:, 0:4, :])
                nc.gpsimd.dma_start(out=wv_sb[:, 0:4, :], in_=wv_r[:, 0:4, :])
                nc.gpsimd.dma_start(out=wk_sb[:, 4:16, :], in_=wk_r[:, 4:16, :])
                nc.gpsimd.dma_start(out=wv_sb[:, 4:16, :], in_=wv_r[:, 4:16, :])
                nc.gpsimd.dma_start(out=wq_sb[:, 0:8, :], in_=wq_r[:, 0:8, :])
                nc.gpsimd.dma_start(out=wq_sb[:, 8:16, :], in_=wq_r[:, 8:16, :])

                rot_c = wp.tile([128, 1], F32, tag="rotc")
                nc.scalar.dma_start(out=rot_c, in_=rotc_d[:])
                rot_s = wp.tile([128, 1], F32, tag="rots")
                nc.scalar.dma_start(out=rot_s, in_=rots_d[:])
                gp = wp  # reuse the bufs=1 pool scope for small gating tiles
                for m in range(NCHUNK):
                    cols = slice(m * CH, (m + 1) * CH)
                    ps_q0 = accps.tile([128, CH], F32, tag="acc")
                    ps_q1 = accps.tile([128, CH], F32, tag="acc")
                    ps_k = accps.tile([128, CH], F32, tag="acc")
                    ps_v = accps.tile([128, CH], F32, tag="acc")
                    if m <= 1:
                        # two passes (k/v then q) to match the weight-arrival
                        # order; the xt tiles stay resident for the q pass
                        xts = []
                        for cc in range(NCT):
                            xt = xtp.tile([128, CH], F32R, tag="xt")
                            # first tiles ride the otherwise-idle Act HWDGE
                            # queue in parallel with the sync queue
                            q = nc.scalar if (m == 0 and cc < 4) else nc.sync
                            q.dma_start(
                                out=xt, in_=xT_d[cc * 128:(cc + 1) * 128, cols]
                            )
                            xts.append(xt)
                            st0, sp0 = (cc == 0), (cc == NCT - 1)
                            nc.tensor.matmul(ps_k, wk_sb[:, cc, :], xt, start=st0, stop=sp0)
                            nc.tensor.matmul(ps_v, wv_sb[:, cc, :], xt, start=st0, stop=sp0)
                        # consts (needed from the rope stage onwards) follow
                        # the early x tiles on the Act HWDGE queue
                        if m == 0:
                          nc.scalar.dma_start(out=pswap, in_=psw_d[:])
                          nc.scalar.dma_start(out=ident, in_=idm_d[:])
                          nc.scalar.dma_start(out=trik, in_=trk_d[:])
                          nc.scalar.dma_start(out=ones_r, in_=onr_d[:])
                          nc.scalar.dma_start(out=oh2, in_=oh2_d[:])
                          nc.scalar.dma_start(out=sel2, in_=sel2_d[:])
                        for cc in range(NCT):
                            st0, sp0 = (cc == 0), (cc == NCT - 1)
                            nc.tensor.matmul(ps_q0, wq_sb[:, cc, 0:128], xts[cc], start=st0, stop=sp0)
                            nc.tensor.matmul(ps_q1, wq_sb[:, cc, 128:256], xts[cc], start=st0, stop=sp0)
                    else:
                      for cc in range(NCT):
                        xt = xtp.tile([128, CH], F32R, tag="xt")
                        nc.sync.dma_start(
                            out=xt, in_=xT_d[cc * 128:(cc + 1) * 128, cols]
                        )
                        st0, sp0 = (cc == 0), (cc == NCT - 1)
                        nc.tensor.matmul(ps_q0, wq_sb[:, cc, 0:128], xt, start=st0, stop=sp0)
                        nc.tensor.matmul(ps_q1, wq_sb[:, cc, 128:256], xt, start=st0, stop=sp0)
                        nc.tensor.matmul(ps_k, wk_sb[:, cc, :], xt, start=st0, stop=sp0)
                        nc.tensor.matmul(ps_v, wv_sb[:, cc, :], xt, start=st0, stop=sp0)

                    if m == 0:
                        cos_t = csp.tile([128, CH], F32, tag="cos", bufs=2)
                        nc.scalar.dma_start(out=cos_t, in_=cos_d[:])
                        sin_t = csp.tile([128, CH], F32, tag="sin", bufs=2)
                        nc.scalar.dma_start(out=sin_t, in_=sin_d[:])
                    else:
                        # rotate the previous chunk's tables by CH positions
                        # (per-partition angle), off the DMA wire entirely
                        cos_p, sin_p = cos_t, sin_t
                        ta = rp.tile([128, CH], F32, tag="t2")
                        nc.vector.tensor_scalar(
                            out=ta, in0=sin_p, scalar1=rot_s, scalar2=None,
                            op0=mybir.AluOpType.mult,
                        )
                        cos_t = csp.tile([128, CH], F32, tag="cos", bufs=2)
                        nc.vector.scalar_tensor_tensor(
                            out=cos_t, in0=cos_p, scalar=rot_c, in1=ta,
                            op0=mybir.AluOpType.mult,
                            op1=mybir.AluOpType.subtract,
                        )
                        tb = rp.tile([128, CH], F32, tag="t2")
                        nc.vector.tensor_scalar(
                            out=tb, in0=cos_p, scalar1=rot_s, scalar2=None,
                            op0=mybir.AluOpType.mult,
                        )
                        sin_t = csp.tile([128, CH], F32, tag="sin", bufs=2)
                        nc.vector.scalar_tensor_tensor(
                            out=sin_t, in0=sin_p, scalar=rot_c, in1=tb,
                            op0=mybir.AluOpType.mult,
                            op1=mybir.AluOpType.add,
                        )

                    # V: evacuate then PE-transpose to natural [s, d] layout
                    # (fp32r copy: vN is fp32r anyway, and fp32r transpose runs
                    # 1.5 cyc/row vs 2.0 for fp32)
                    vtmp = rp.tile([128, CH], F32R, tag="qraw2")
                    nc.vector.tensor_copy(vtmp, ps_v)
                    ps_vt = trps.tile([128, CH], F32, tag="tr")
                    for u in range(4):
                        nc.tensor.transpose(
                            ps_vt.bitcast(F32R)[:, u * 128:(u + 1) * 128],
                            vtmp[:, u * 128:(u + 1) * 128],
                            ident,
                        )
                    nc.vector.tensor_copy(
                        vN[:, 4 * m:4 * m + 4, :],
                        ps_vt.rearrange("p (u d) -> p u d", u=4),
                    )

                    for psrc, dstT in ((ps_q0, qT[0]), (ps_q1, qT[1]), (ps_k, kT)):
                        raw = rp.tile([128, CH], F32R, tag="qraw")
                        nc.vector.tensor_copy(raw, psrc)
                        ps_sw = trps.tile([128, CH], F32, tag="tr")
                        nc.tensor.matmul(ps_sw, pswap, raw, start=True, stop=True)
                        t2 = rp.tile([128, CH], F32, tag="t2")
                        nc.vector.tensor_tensor(
                            t2, raw.bitcast(F32), cos_t, op=mybir.AluOpType.mult
                        )
                        # sw *= sin in place (PSUM), then add -> rope output
                        nc.vector.tensor_tensor(ps_sw, ps_sw, sin_t, op=mybir.AluOpType.mult)
                        nc.vector.tensor_tensor(
                            dstT[:, cols], t2, ps_sw, op=mybir.AluOpType.add
                        )

                    # partial block sums for this chunk's 4 key blocks
                    nc.vector.tensor_reduce(
                        bm[:, 4 * m:4 * m + 4],
                        kT.bitcast(F32)[:, cols].rearrange("p (b t) -> p b t", b=4),
                        axis=mybir.AxisListType.X,
                        op=mybir.AluOpType.add,
                    )
                    # gating flags for this chunk's query blocks (needs
                    # bm 0..i); chunks 6-7 are deferred into the attention
                    # phase so the phase boundary is not serialized on them
                    if 2 <= m <= 5:
                        _gating(nc, m, qT, bm, Ft, ident, trps, "g", gp)

            # ---------------- phases 3+4 ------------------------------------
            _phase34(nc, tc, qT, kT, vN, trik, oh2, sel2, ones_r, Ft, wo_d,
                     out_d, bm, ident)

    nc.compile()
    _CACHE[key] = nc
    return nc


def _phase34(nc, tc, qT, kT, vN, trik, oh2, sel2, ones_r, Ft, wo_d, out_d,
             bm, ident):
    wop_cm = tc.tile_pool(name="wop", bufs=1)
    wop = wop_cm.__enter__()
    wo_sb = wop.tile([128, HPC, DIM], F32R, tag="wo")
    nc.gpsimd.dma_start(out=wo_sb, in_=wo_d.rearrange("(t p) d -> p t d", p=128))
    # ------- phase 3: attention with interleaved output projection -------
    # (wo(m) right after attn(m) so the 32MB output DMA spreads over the
    # whole kernel instead of piling into a DMA-bound tail phase)
    # PSUM budget (16KB/partition): pss "s" 2x[128,2,CH] = 8KB, pso "o"
    # 2x[128,CH] + "w" 1x[128,CH] = 6KB, psd "den2" 1x[2,CH] = 2KB.
    # Broadcast scratch and the final wo emission reuse the pss "s" slots.
    # ps_w gets its own tag so the deferred wo matmuls interleave freely into
    # the attention exp-latency gaps instead of queueing behind ps_o's WAR.
    with (
        tc.tile_pool(name="att", bufs=4) as ap,
        tc.tile_pool(name="attb", bufs=2) as ab,
        tc.tile_pool(name="oTs", bufs=4) as otp,
        tc.tile_pool(name="outp", bufs=6) as outp,
        tc.tile_pool(name="att_s", bufs=2, space="PSUM") as pss,
        tc.tile_pool(name="att_o", bufs=2, space="PSUM") as pso,
        tc.tile_pool(name="att_d", bufs=1, space="PSUM") as psd,
    ):
        # ---- wo(m') emission: projection for s-tiles of chunk m' ----
        # Emitted piecewise, one output tile per attention j-iteration of the
        # NEXT chunk, so the wo matmuls fill the PE's exp-latency gaps.
        def wo_pieces(mm, oTc_mm, final=False):
            for u in range(4):
                st = 4 * mm + u
                for n in range(4):
                    ncols = slice(n * 512, (n + 1) * 512)
                    if final:
                        # attention is done: reuse the (free) score slots
                        # for a 2-deep pipelined tail
                        ps_w2 = pss.tile([128, HPC, CH], F32, tag="s", name="psw2")
                        ps_w = ps_w2[:, 0, :]
                    else:
                        ps_w = pso.tile([128, 512], F32, tag="w", name="psw", bufs=1)
                    nc.tensor.matmul(
                        ps_w,
                        oTc_mm[0][:, u * 128:(u + 1) * 128],
                        wo_sb[:, 0, ncols],
                        start=True,
                        stop=False,
                    )
                    nc.tensor.matmul(
                        ps_w,
                        oTc_mm[1][:, u * 128:(u + 1) * 128],
                        wo_sb[:, 1, ncols],
                        start=False,
                        stop=True,
                    )
                    osb = outp.tile([128, 512], F32, tag="ow")
                    # copies stay off the Act engine: at chunk boundaries an
                    # Act-side copy queues in front of the next chunk's exps,
                    # which hold the score slots and stall the QK stream
                    nc.vector.tensor_copy(osb, ps_w)
                    if (st * 4 + n) % 3 == 0:
                        nc.sync.dma_start(
                            out=out_d[st * 128:(st + 1) * 128, ncols], in_=osb
                        )
                    else:
                        nc.gpsimd.dma_start(
                            out=out_d[st * 128:(st + 1) * 128, ncols], in_=osb
                        )
                    yield True

        wo_queue = []  # pending wo piece generators (FIFO across chunks)

        def fire_wo(n=1):
            # at most n pieces; a second piece per attention step would stall
            # the in-order PE stream on the single-bank ps_w WAR
            while n > 0 and wo_queue:
                if next(wo_queue[0], None) is None:
                    wo_queue.pop(0)
                else:
                    n -= 1

        for m in range(NCHUNK):
            nsk = 8 if m < 2 else 4 * m + 4
            ps_o = [pso.tile([128, CH], F32, tag="o", name=f"o{h}") for h in range(HPC)]
            oTc = [
                otp.tile([128, CH], F32R, tag="oTc", name=f"oTc{h}")
                for h in range(HPC)
            ]
            # precompute diagonal-mask tiles for this chunk's band (off the
            # exp->PV critical path): one broadcast matmul + one max for both
            # heads and all 4 band blocks at once
            mks = None
            if m >= 2:
                ps_bc = pss.tile([128, HPC, CH], F32, tag="s", name="ps_bc")
                for h in range(HPC):
                    nc.tensor.matmul(
                        ps_bc[:, h, :],
                        ones_r,
                        Ft[:, h, (4 * m - 8) * 128:(4 * m - 4) * 128],
                        start=True,
                        stop=True,
                    )
                mks = ab.tile([128, HPC, CH], F32, tag="mk", bufs=2)
                trik_b = trik.rearrange("p (a b t) -> p a b t", a=1, b=1).broadcast_to(
                    [128, HPC, 4, BLOCK]
                )
                nc.vector.tensor_tensor(
                    mks.rearrange("p h (b t) -> p h b t", b=4),
                    trik_b,
                    ps_bc.rearrange("p h (b t) -> p h b t", b=4),
                    op=mybir.AluOpType.max,
                )
            ps_den = psd.tile([2, CH], F32, tag="den2", name="den2", bufs=1)

            def colspan(j):
                band = m >= 2 and j >= 4 * m
                # pad ap=128 matmuls (4 cyc/row below ap 256) to ap=256
                col0 = (j - 4 * m) * 128 if band else 0
                colp = min(col0, CH - 256) if band else 0
                return band, col0, colp

            def emit_qk_exp(j):
                band, col0, colp = colspan(j)
                ps_s = pss.tile([128, HPC, CH], F32, tag="s", bufs=2)
                for h in range(HPC):
                    nc.tensor.matmul(
                        ps_s[:, h, colp:],
                        kT[:, j * 128:(j + 1) * 128],
                        qT[h][:, m * CH + colp:(m + 1) * CH],
                        start=True,
                        stop=True,
                    )
                if colp < col0:
                    # overwrite the pad region so exp underflows to exact 0
                    nc.vector.memset(ps_s[:, :, colp:col0], PAD_NEG)
                pexp = ap.tile([128, HPC, CH], F32R, tag="pexp", bufs=4)
                nc.scalar.activation(
                    out=pexp[:, :, colp:],
                    in_=ps_s[:, :, colp:],
                    func=mybir.ActivationFunctionType.Exp,
                    scale=INV_SQRT_D,
                )
                if band:
                    nc.vector.tensor_tensor(
                        pexp[:, :, col0:col0 + 128],
                        pexp.bitcast(F32)[:, :, col0:col0 + 128],
                        mks[:, :, col0:col0 + 128],
                        op=mybir.AluOpType.mult,
                    )
                return pexp

            def emit_pv_den(j, pexp):
                _, _, colp = colspan(j)
                for h in range(HPC):
                    nc.tensor.matmul(
                        ps_o[h][:, colp:],
                        vN[:, j, :],
                        pexp[:, h, colp:],
                        start=(j == 0),
                        stop=(j == nsk - 1),
                    )
                    nc.tensor.matmul(
                        ps_den[:, colp:],
                        oh2[:, h, :],
                        pexp[:, h, colp:],
                        start=(j == 0 and h == 0),
                        stop=(j == nsk - 1 and h == HPC - 1),
                    )

            # software-pipeline by two j: QK/exp of j+1/j+2 are emitted before
            # PV/den of j, so the PE never sits in-order behind exp latency;
            # wo output tiles of the previous chunk (spread evenly over the
            # j-loop) fill the remaining slack
            pexps = [emit_qk_exp(0)]
            if nsk > 1:
                pexps.append(emit_qk_exp(1))
            for j in range(nsk):
                if j + 2 < nsk:
                    pexps.append(emit_qk_exp(j + 2))
                emit_pv_den(j, pexps[j])
                pexps[j] = None  # release reference
                # spread the ~16 pending pieces over all nsk steps (still at
                # most one per step), so late js keep their latency filler
                if len(wo_queue) > 1 or j % max(1, nsk // 16) == 0:
                    fire_wo(1)
            fire_wo(1)  # one more piece fills the boundary chain
            # normalisation: reciprocal of the two dens, broadcast via the
            # "w" bank (keeps the "s" slots free so the next chunk's QKs can
            # start during this chain), then scale ps_o into fp32r oTc
            rec2 = ab.tile([2, CH], F32R, tag="rec")
            nc.vector.reciprocal(rec2, ps_den)
            bc_sb = ab.tile([128, HPC, CH], F32, tag="bcs")
            for h in range(HPC):
                ps_rb = pso.tile([128, 512], F32, tag="w", name="ps_rb", bufs=1)
                nc.tensor.matmul(
                    ps_rb, sel2[:, h, :], rec2, start=True, stop=True
                )
                nc.scalar.copy(bc_sb[:, h, :], ps_rb)
                nc.vector.tensor_tensor(
                    oTc[h], ps_o[h], bc_sb[:, h, :], op=mybir.AluOpType.mult
                )
            # deferred gating for the last two phase-1 chunks: the PE
            # matmuls double as filler during this chunk's boundary chain
            if m == 2:
                _gating(nc, 6, qT, bm, Ft, ident, pso, "w", ab)
            elif m == 3:
                _gating(nc, 7, qT, bm, Ft, ident, pso, "w", ab)
            wo_queue.append(
                wo_pieces(m, oTc, final=(m == NCHUNK - 1))
            )
        while wo_queue:
            fire_wo(1)
    wop_cm.__exit__(None, None, None)


def kernel(x, wq, wk, wv, wo):
    bs = np.asarray(x).shape[0]
    in_maps = make_in_maps(x, wq, wk, wv, wo)
    nc = _build_nc()
    res = run_bass_kernel_spmd(nc, in_maps, list(range(N_CORES)))
    out = res.results[0]["out"].astype(np.float64)
    for c in range(1, N_CORES):
        out += res.results[c]["out"]
    return out.astype(np.float32).reshape(bs, SEQ, DIM)


if __name__ == "__main__":
    rng = np.random.default_rng(0)
    xs = {
        "x": rng.standard_normal((1, SEQ, DIM), dtype=np.float32),
        "wq": rng.standard_normal((DIM, DIM), dtype=np.float32) * (DIM ** -0.5),
        "wk": rng.standard_normal((DIM, DIM // 2), dtype=np.float32) * (DIM ** -0.5),
        "wv": rng.standard_normal((DIM, DIM // 2), dtype=np.float32) * (DIM ** -0.5),
        "wo": rng.standard_normal((DIM, DIM), dtype=np.float32) * (DIM ** -0.5),
    }
    out = kernel(**xs)
    print("out", out.shape, out.dtype, np.abs(out).max())


# revision 52
# speedup vs baseline: 1.1006x; 1.0088x over previous
"""MixtureOfBlockAttention TRN2 kernel — 8-core head-parallel (TP) Bass/Tile implementation.

Semantics (verified equivalent to the reference, rel err ~2e-2 budget dominated
by top-k near-tie flips caused by fp32r rounding of x/w — irreducible without
dropping fp32r):
the reference mask `maximum(token_mask, causal*NEG_INF)` masks a position iff
it is BOTH future AND in a non-selected block. Consequences:
  - query blocks 0..7 attend to ALL tokens of key blocks 0..7 (dense, no mask);
  - query block i>=8 attends densely to key blocks 0..i-1, and within its own
    (diagonal) block applies strict causal masking ONLY for rows whose own
    block is not among their top-8 gating blocks.
Selection rank for query s in block i (i>=8): own block selected iff
  #{j < i : g[s,j] > g[s,i]} < 8, with g = q . (block sums of roped k)
(positive-scale invariant, so block sums replace means and the 1/sqrt(d)
factor is dropped).

Sharding: 16 query heads / 8 cores = 2 heads per core; KV head c serves both.
wq/wk/wv column-sliced, wo row-sliced; partial outputs summed on host.
Host-side layout prep: x is transposed to xT[c, s] (the PE contracts over the
partition dim, so both matmul operands need c on partitions) and float inputs
are pre-rounded to fp32r; both are pure data-layout transforms.

All big matmuls run in float32r (TF32-like input rounding, fp32 accumulate,
full PE rate at N>=256). Optimizations vs the original baseline (all
numerically neutral except the rope-table recurrence, which shifts the final
rel err by <1e-6 with no top-k flips):
  - per-j exp fused across the 2 heads (one Act instruction, PSUM [128,2,512]);
  - softmax denominators for both heads accumulate into one [2,512] PSUM bank
    via [128,2] one-hot-column stationaries;
  - reciprocal-broadcast and notflag-broadcast matmuls emitted at ap>=512
    instead of 4x/8x ap=128 pieces (fp32r runs 4 cyc/row below ap 256);
  - diagonal-band j=4m+3 matmuls padded from ap=128 to ap=256 with a -1e5
    PSUM memset in the pad so exp underflows to exact 0 there;
  - V-transpose and notflag-transpose in fp32r (1.5 cyc/row) not fp32 (2.0);
  - rope cos/sin tables generated on device by a per-chunk angle-rotation
    recurrence (saves 4MB/core of HBM reads and the phase-1 DMA deficit);
  - attention j-loop software-pipelined by one step (QK/exp of j+1 emitted
    before PV/den of j) so the in-order PE stream hides the exp latency;
  - the deferred wo projection is emitted as 16 per-chunk output tiles fired
    at most one per attention step from a cross-chunk FIFO, filling the PE's
    residual exp-latency slack without ever stalling it on the ps_w WAR;
  - gating reordered (all matmuls, then DVE compare chains, then batched
    transposes) and chunks 6-7's gating deferred into the attention phase;
  - PSUM: scores 2x[128,2,512] + 2 ps_o + 1 ps_w + 1 den2 = exactly 8 banks.
"""

import math
import sys

import numpy as np

if "/opt/trn_rl_repo" not in sys.path:
    sys.path.insert(0, "/opt/trn_rl_repo")

import concourse.bacc as bacc
import concourse.mybir as mybir
import concourse.tile as tile
from concourse.bass_utils import run_bass_kernel_spmd

F32 = mybir.dt.float32
F32R = mybir.dt.float32r

SEQ = 4096
DIM = 2048
HEAD_DIM = 128
N_HEADS = 16
N_CORES = 8
HPC = N_HEADS // N_CORES       # heads per core = 2
DPC = HPC * HEAD_DIM           # q/o dims per core = 256
BLOCK = 128
NB = SEQ // BLOCK              # 32 key blocks
TOPK = 8
NCHUNK = 8                     # s-chunks of 512
CH = SEQ // NCHUNK             # 512
NCT = DIM // 128               # 16 contraction tiles
INV_SQRT_D = 1.0 / math.sqrt(HEAD_DIM)
PAD_NEG = -100000.0

_CACHE = {}


def _round_fp32r(a):
    """Round fp32 to the fp32r grid (top-11-bit mantissa, round-to-nearest)."""
    a = np.ascontiguousarray(a, dtype=np.float32)
    try:
        from neuron_dtypes import static_cast_fp32_to_fp32r

        return static_cast_fp32_to_fp32r(a).view(np.float32).astype(np.float32)
    except Exception:
        u = a.view(np.uint32)
        return ((u + np.uint32(0x800)) & np.uint32(0xFFFFF000)).view(np.float32).copy()


def _host_constants():
    if "consts" in _CACHE:
        return _CACHE["consts"]
    p = np.arange(HEAD_DIM // 2, dtype=np.float64)
    inv_freq = 1.0 / (10000.0 ** (2.0 * p / HEAD_DIM))
    ang = np.arange(SEQ, dtype=np.float64)[None, :] * inv_freq[:, None]  # [64, S]
    cos = np.cos(ang).astype(np.float32)
    sin = np.sin(ang).astype(np.float32)
    cos_ds = np.ascontiguousarray(np.repeat(cos, 2, axis=0))   # [128, S]
    sin_ds = np.empty((HEAD_DIM, SEQ), dtype=np.float32)       # signed sin
    sin_ds[0::2] = -sin
    sin_ds[1::2] = sin
    # per-partition rotation by CH positions: next-chunk tables via
    # cos' = cos*C - sin_ds*S_row ; sin_ds' = sin_ds*C + cos*S_row
    # (S_row carries the sign convention of the interleaved sin_ds rows)
    inv_freq = 1.0 / (10000.0 ** (2.0 * p / HEAD_DIM))
    c512 = np.cos(CH * inv_freq)
    s512 = np.sin(CH * inv_freq)
    rotC = np.repeat(c512, 2).astype(np.float32)[:, None]      # [128, 1]
    rotS = np.empty((HEAD_DIM,), dtype=np.float64)
    rotS[0::2] = -s512
    rotS[1::2] = s512
    rotS = rotS.astype(np.float32)[:, None]                    # [128, 1]
    pswap = np.zeros((128, 128), dtype=np.float32)             # swap 2p <-> 2p+1
    idx = np.arange(128)
    pswap[idx, idx ^ 1] = 1.0
    identm = np.eye(128, dtype=np.float32)
    r = np.arange(BLOCK)
    trikeep = (r[:, None] <= r[None, :]).astype(np.float32)    # keep iff sk <= sq
    ones_row = np.ones((1, 128), dtype=np.float32)
    # one-hot-column stationaries for per-head den accumulation into [2, CH]:
    # oh2[:, h, :] is [128, 2] with column h all-ones
    oh2 = np.zeros((128, 2, 2), dtype=np.float32)
    oh2[:, 0, 0] = 1.0
    oh2[:, 1, 1] = 1.0
    # one-hot-row stationaries for per-head [2,CH] -> [128,CH] broadcast:
    # sel2[:, h, :] is [2, 128] with row h all-ones
    sel2 = np.zeros((2, 2, 128), dtype=np.float32)
    sel2[0, 0, :] = 1.0
    sel2[1, 1, :] = 1.0
    _CACHE["consts"] = (cos_ds, sin_ds, rotC, rotS, pswap, identm, trikeep, ones_row, oh2, sel2)
    return _CACHE["consts"]


def make_in_maps(x, wq, wk, wv, wo):
    """Shard + lay out the full inputs for the 8 cores."""
    x2 = np.asarray(x, dtype=np.float32).reshape(SEQ, DIM)
    xT = _round_fp32r(np.ascontiguousarray(x2.T))
    wq = np.asarray(wq, dtype=np.float32)
    wk = np.asarray(wk, dtype=np.float32)
    wv = np.asarray(wv, dtype=np.float32)
    wo = np.asarray(wo, dtype=np.float32)
    cos_ds, sin_ds, rotC, rotS, pswap, identm, trikeep, ones_row, oh2, sel2 = _host_constants()
    pswap_r = _round_fp32r(pswap)
    ones_row_r = _round_fp32r(ones_row)
    oh2_r = _round_fp32r(oh2)
    sel2_r = _round_fp32r(sel2)
    in_maps = []
    for c in range(N_CORES):
        in_maps.append(
            {
                "xT": xT,
                "wq": _round_fp32r(wq[:, c * DPC:(c + 1) * DPC]),
                "wk": _round_fp32r(wk[:, c * HEAD_DIM:(c + 1) * HEAD_DIM]),
                "wv": _round_fp32r(wv[:, c * HEAD_DIM:(c + 1) * HEAD_DIM]),
                "wo": _round_fp32r(wo[c * DPC:(c + 1) * DPC, :]),
                "cos0": np.ascontiguousarray(cos_ds[:, 0:CH]),
                "sin0": np.ascontiguousarray(sin_ds[:, 0:CH]),
                "rotC": rotC,
                "rotS": rotS,
                "pswap": pswap_r,
                "identm": _round_fp32r(identm),
                "trikeep": trikeep,
                "ones_r": ones_row_r,
                "oh2": oh2_r,
                "sel2": sel2_r,
            }
        )
    return in_maps


def _gating(nc, m, qT, bm, Ft, ident, ps_pool, ps_tag, sb_pool):
    """Own-block top-k flags for chunk m's 4 query blocks (both heads).

    All 8 gating matmuls first, then the DVE compare chains, then the 8
    transposes batched 4-per-PSUM-bank, so the in-order PE stream never
    waits mid-chain. PSUM scratch comes from (ps_pool, ps_tag) so this can
    run late, inside the attention phase, for the last two chunks.
    """
    import concourse.mybir as mybir

    F32 = mybir.dt.float32
    F32R = mybir.dt.float32r
    pairs = [(h, i) for h in range(HPC) for i in range(4 * m, 4 * m + 4)]
    nbk = 4 * m + 4  # even N; cols > i unused
    ps_g8 = ps_pool.tile([128, 8, NB], F32, tag=ps_tag, bufs=1, name="g8")
    for p, (h, i) in enumerate(pairs):
        nc.tensor.matmul(
            ps_g8[:, p, 0:nbk],
            qT[h][:, i * 128:(i + 1) * 128],
            bm[:, 0:nbk],
            start=True,
            stop=True,
        )
    nfs = []
    for p, (h, i) in enumerate(pairs):
        cmp = sb_pool.tile([128, NB], F32, tag="cmp", bufs=2, name="cmp")
        cnt = sb_pool.tile([128, 1], F32, tag="cnt", bufs=2, name="cnt")
        nc.vector.tensor_scalar(
            out=cmp[:, 0:i],
            in0=ps_g8[:, p, 0:i],
            scalar1=ps_g8[:, p, i:i + 1],
            scalar2=None,
            op0=mybir.AluOpType.is_gt,
        )
        nc.vector.tensor_reduce(
            cnt, cmp[:, 0:i], axis=mybir.AxisListType.X, op=mybir.AluOpType.add
        )
        # notflag: 1.0 -> own block selected (keep all)
        # (fp32r tiles: values are exactly 0.0/1.0)
        nf = sb_pool.tile([128, 1], F32R, tag="nf", bufs=8, name=f"nf{p}")
        nc.vector.tensor_scalar(
            out=nf,
            in0=cnt,
            scalar1=float(TOPK) - 0.5,
            scalar2=None,
            op0=mybir.AluOpType.is_lt,
        )
        nfs.append(nf)
    for h in range(HPC):
        ps_ft4 = ps_pool.tile([1, 4, 128], F32, tag=ps_tag, bufs=1, name="ft4")
        for t in range(4):
            nc.tensor.transpose(
                ps_ft4.bitcast(F32R)[:, t, :], nfs[4 * h + t], ident
            )
        nc.vector.tensor_copy(
            Ft[:, h, (4 * m - 8) * 128:(4 * m - 4) * 128],
            ps_ft4.rearrange("o f t -> o (f t)"),
        )



def _build_nc(reps=1):
    key = f"nc{reps}"
    if key in _CACHE:
        return _CACHE[key]
    nc = bacc.Bacc(None, target_bir_lowering=False)

    xT_d = nc.dram_tensor("xT", [DIM, SEQ], F32R, kind="ExternalInput")
    wq_d = nc.dram_tensor("wq", [DIM, DPC], F32R, kind="ExternalInput")
    wk_d = nc.dram_tensor("wk", [DIM, HEAD_DIM], F32R, kind="ExternalInput")
    wv_d = nc.dram_tensor("wv", [DIM, HEAD_DIM], F32R, kind="ExternalInput")
    wo_d = nc.dram_tensor("wo", [DPC, DIM], F32R, kind="ExternalInput")
    cos_d = nc.dram_tensor("cos0", [HEAD_DIM, CH], F32, kind="ExternalInput")
    sin_d = nc.dram_tensor("sin0", [HEAD_DIM, CH], F32, kind="ExternalInput")
    rotc_d = nc.dram_tensor("rotC", [HEAD_DIM, 1], F32, kind="ExternalInput")
    rots_d = nc.dram_tensor("rotS", [HEAD_DIM, 1], F32, kind="ExternalInput")
    psw_d = nc.dram_tensor("pswap", [128, 128], F32R, kind="ExternalInput")
    idm_d = nc.dram_tensor("identm", [128, 128], F32R, kind="ExternalInput")
    trk_d = nc.dram_tensor("trikeep", [BLOCK, BLOCK], F32, kind="ExternalInput")
    onr_d = nc.dram_tensor("ones_r", [1, 128], F32R, kind="ExternalInput")
    oh2_d = nc.dram_tensor("oh2", [128, 2, 2], F32R, kind="ExternalInput")
    sel2_d = nc.dram_tensor("sel2", [2, 2, 128], F32R, kind="ExternalInput")
    out_d = nc.dram_tensor("out", [SEQ, DIM], F32, kind="ExternalOutput")

    with tile.TileContext(nc) as tc, nc.allow_low_precision(
        reason="float32r rounding of matmul operands is intentional"
    ):
      for _rep in range(reps):
        with tc.tile_pool(name="persist", bufs=1) as per:
            qT = [per.tile([128, SEQ], F32R, tag=f"qT{h}", name=f"qT{h}") for h in range(HPC)]
            kT = per.tile([128, SEQ], F32R, tag="kT")
            vN = per.tile([128, NB, 128], F32R, tag="vN")   # [s-in-tile, sk-tile, d]
            ident = per.tile([128, 128], F32R, tag="ident")
            pswap = per.tile([128, 128], F32R, tag="pswap")
            trik = per.tile([BLOCK, BLOCK], F32, tag="trik")
            ones_r = per.tile([1, 128], F32R, tag="ones_r")
            oh2 = per.tile([128, 2, 2], F32R, tag="oh2")    # [k, h, den-col]
            sel2 = per.tile([2, 2, 128], F32R, tag="sel2")  # [den-row, h, p]
            bm = per.tile([128, NB], F32R, tag="bm")
            # per-head notflag rows: Ft[0, h, (i-TOPK)*128:...] is the [1,128]
            # notflag row for query block i of head h, at base partition 0
            Ft = per.tile([1, HPC, (NB - TOPK) * 128], F32R, tag="Ft")

            # dummy exp so the Exp act-table load overlaps the initial weight
            # DMAs instead of stalling the first attention chunk
            warm = per.tile([1, 1], F32, tag="warm")
            nc.vector.memset(warm, 0.0)
            nc.scalar.activation(
                out=warm, in_=warm, func=mybir.ActivationFunctionType.Exp
            )

            # ---------------- phase 1: projections + rope -------------------
            with (
                tc.tile_pool(name="wpool", bufs=1) as wp,
                tc.tile_pool(name="xtp", bufs=17) as xtp,
                tc.tile_pool(name="ropep", bufs=2) as rp,
                tc.tile_pool(name="csin", bufs=2) as csp,
                # acc_ps declared first so its PSUM range lines up with the
                # attention score pool's range: the last acc_ps readers (rope
                # copies) finish well before the gating tail that occupies
                # pj_ps, letting chunk-0 QK matmuls start during the tail
                tc.tile_pool(name="acc_ps", bufs=4, space="PSUM") as accps,
                tc.tile_pool(name="pj_ps", bufs=2, space="PSUM") as trps,
            ):
                wq_sb = wp.tile([128, NCT, DPC], F32R, tag="wq")
                wk_sb = wp.tile([128, NCT, HEAD_DIM], F32R, tag="wk")
                wv_sb = wp.tile([128, NCT, HEAD_DIM], F32R, tag="wv")
                wq_r = wq_d.rearrange("(t p) d -> p t d", p=128)
                wk_r = wk_d.rearrange("(t p) d -> p t d", p=128)
                wv_r = wv_d.rearrange("(t p) d -> p t d", p=128)
                # k/v weights first: chunk 0 runs its k/v matmuls while the
                # (2x bigger) wq still streams in
                nc.gpsimd.dma_start(out=wk_sb[:, 0:4, :], in_=wk_r[:, 0:4, :])
                nc.gpsimd.dma_start(out=wv_sb[:, 0:4, :], in_=wv_r[:, 0:4, :])
                nc.gpsimd.dma_start(out=wk_sb[:, 4:16, :], in_=wk_r[:, 4:16, :])
                nc.gpsimd.dma_start(out=wv_sb[:, 4:16, :], in_=wv_r[:, 4:16, :])
                nc.gpsimd.dma_start(out=wq_sb[:, 0:8, :], in_=wq_r[:, 0:8, :])
                nc.gpsimd.dma_start(out=wq_sb[:, 8:16, :], in_=wq_r[:, 8:16, :])

                rot_c = wp.tile([128, 1], F32, tag="rotc")
                nc.scalar.dma_start(out=rot_c, in_=rotc_d[:])
                rot_s = wp.tile([128, 1], F32, tag="rots")
                nc.scalar.dma_start(out=rot_s, in_=rots_d[:])
                gp = wp  # reuse the bufs=1 pool scope for small gating tiles
                for m in range(NCHUNK):
                    cols = slice(m * CH, (m + 1) * CH)
                    ps_q0 = accps.tile([128, CH], F32, tag="acc")
                    ps_q1 = accps.tile([128, CH], F32, tag="acc")
                    ps_k = accps.tile([128, CH], F32, tag="acc")
                    ps_v = accps.tile([128, CH], F32, tag="acc")
                    if m <= 1:
                        # two passes (k/v then q) to match the weight-arrival
                        # order; the xt tiles stay resident for the q pass
                        xts = []
                        for cc in range(NCT):
                            xt = xtp.tile([128, CH], F32R, tag="xt")
                            # first tiles ride the otherwise-idle Act HWDGE
                            # queue in parallel with the sync queue
                            q = nc.scalar if (m == 0 and cc < 4) else nc.sync
                            q.dma_start(
                                out=xt, in_=xT_d[cc * 128:(cc + 1) * 128, cols]
                            )
                            xts.append(xt)
                            st0, sp0 = (cc == 0), (cc == NCT - 1)
                            nc.tensor.matmul(ps_k, wk_sb[:, cc, :], xt, start=st0, stop=sp0)
                            nc.tensor.matmul(ps_v, wv_sb[:, cc, :], xt, start=st0, stop=sp0)
                        # consts (needed from the rope stage onwards) follow
                        # the early x tiles on the Act HWDGE queue
                        if m == 0:
                          nc.scalar.dma_start(out=pswap, in_=psw_d[:])
                          nc.scalar.dma_start(out=ident, in_=idm_d[:])
                          nc.scalar.dma_start(out=trik, in_=trk_d[:])
                          nc.scalar.dma_start(out=ones_r, in_=onr_d[:])
                          nc.scalar.dma_start(out=oh2, in_=oh2_d[:])
                          nc.scalar.dma_start(out=sel2, in_=sel2_d[:])
                        for cc in range(NCT):
                            st0, sp0 = (cc == 0), (cc == NCT - 1)
                            nc.tensor.matmul(ps_q0, wq_sb[:, cc, 0:128], xts[cc], start=st0, stop=sp0)
                            nc.tensor.matmul(ps_q1, wq_sb[:, cc, 128:256], xts[cc], start=st0, stop=sp0)
                    else:
                      for cc in range(NCT):
                        xt = xtp.tile([128, CH], F32R, tag="xt")
                        nc.sync.dma_start(
                            out=xt, in_=xT_d[cc * 128:(cc + 1) * 128, cols]
                        )
                        st0, sp0 = (cc == 0), (cc == NCT - 1)
                        nc.tensor.matmul(ps_q0, wq_sb[:, cc, 0:128], xt, start=st0, stop=sp0)
                        nc.tensor.matmul(ps_q1, wq_sb[:, cc, 128:256], xt, start=st0, stop=sp0)
                        nc.tensor.matmul(ps_k, wk_sb[:, cc, :], xt, start=st0, stop=sp0)
                        nc.tensor.matmul(ps_v, wv_sb[:, cc, :], xt, start=st0, stop=sp0)

                    if m == 0:
                        cos_t = csp.tile([128, CH], F32, tag="cos", bufs=2)
                        nc.scalar.dma_start(out=cos_t, in_=cos_d[:])
                        sin_t = csp.tile([128, CH], F32, tag="sin", bufs=2)
                        nc.scalar.dma_start(out=sin_t, in_=sin_d[:])
                    else:
                        # rotate the previous chunk's tables by CH positions
                        # (per-partition angle), off the DMA wire entirely
                        cos_p, sin_p = cos_t, sin_t
                        ta = rp.tile([128, CH], F32, tag="t2")
                        nc.vector.tensor_scalar(
                            out=ta, in0=sin_p, scalar1=rot_s, scalar2=None,
                            op0=mybir.AluOpType.mult,
                        )
                        cos_t = csp.tile([128, CH], F32, tag="cos", bufs=2)
                        nc.vector.scalar_tensor_tensor(
                            out=cos_t, in0=cos_p, scalar=rot_c, in1=ta,
                            op0=mybir.AluOpType.mult,
                            op1=mybir.AluOpType.subtract,
                        )
                        tb = rp.tile([128, CH], F32, tag="t2")
                        nc.vector.tensor_scalar(
                            out=tb, in0=cos_p, scalar1=rot_s, scalar2=None,
                            op0=mybir.AluOpType.mult,
                        )
                        sin_t = csp.tile([128, CH], F32, tag="sin", bufs=2)
                        nc.vector.scalar_tensor_tensor(
                            out=sin_t, in0=sin_p, scalar=rot_c, in1=tb,
                            op0=mybir.AluOpType.mult,
                            op1=mybir.AluOpType.add,
                        )

                    # V: evacuate then PE-transpose to natural [s, d] layout
                    # (fp32r copy: vN is fp32r anyway, and fp32r transpose runs
                    # 1.5 cyc/row vs 2.0 for fp32)
                    vtmp = rp.tile([128, CH], F32R, tag="qraw2")
                    nc.vector.tensor_copy(vtmp, ps_v)
                    ps_vt = trps.tile([128, CH], F32, tag="tr")
                    for u in range(4):
                        nc.tensor.transpose(
                            ps_vt.bitcast(F32R)[:, u * 128:(u + 1) * 128],
                            vtmp[:, u * 128:(u + 1) * 128],
                            ident,
                        )
                    nc.vector.tensor_copy(
                        vN[:, 4 * m:4 * m + 4, :],
                        ps_vt.rearrange("p (u d) -> p u d", u=4),
                    )

                    for psrc, dstT in ((ps_q0, qT[0]), (ps_q1, qT[1]), (ps_k, kT)):
                        raw = rp.tile([128, CH], F32R, tag="qraw")
                        nc.vector.tensor_copy(raw, psrc)
                        ps_sw = trps.tile([128, CH], F32, tag="tr")
                        nc.tensor.matmul(ps_sw, pswap, raw, start=True, stop=True)
                        t2 = rp.tile([128, CH], F32, tag="t2")
                        nc.vector.tensor_tensor(
                            t2, raw.bitcast(F32), cos_t, op=mybir.AluOpType.mult
                        )
                        # sw *= sin in place (PSUM), then add -> rope output
                        nc.vector.tensor_tensor(ps_sw, ps_sw, sin_t, op=mybir.AluOpType.mult)
                        nc.vector.tensor_tensor(
                            dstT[:, cols], t2, ps_sw, op=mybir.AluOpType.add
                        )

                    # partial block sums for this chunk's 4 key blocks
                    nc.vector.tensor_reduce(
                        bm[:, 4 * m:4 * m + 4],
                        kT.bitcast(F32)[:, cols].rearrange("p (b t) -> p b t", b=4),
                        axis=mybir.AxisListType.X,
                        op=mybir.AluOpType.add,
                    )
                    # gating flags for this chunk's query blocks (needs
                    # bm 0..i); chunks 6-7 are deferred into the attention
                    # phase so the phase boundary is not serialized on them
                    if 2 <= m <= 5:
                        _gating(nc, m, qT, bm, Ft, ident, trps, "g", gp)

            # ---------------- phases 3+4 ------------------------------------
            _phase34(nc, tc, qT, kT, vN, trik, oh2, sel2, ones_r, Ft, wo_d,
                     out_d, bm, ident)

    nc.compile()
    _CACHE[key] = nc
    return nc


def _phase34(nc, tc, qT, kT, vN, trik, oh2, sel2, ones_r, Ft, wo_d, out_d,
             bm, ident):
    wop_cm = tc.tile_pool(name="wop", bufs=1)
    wop = wop_cm.__enter__()
    wo_sb = wop.tile([128, HPC, DIM], F32R, tag="wo")
    nc.gpsimd.dma_start(out=wo_sb, in_=wo_d.rearrange("(t p) d -> p t d", p=128))
    # ------- phase 3: attention with interleaved output projection -------
    # (wo(m) right after attn(m) so the 32MB output DMA spreads over the
    # whole kernel instead of piling into a DMA-bound tail phase)
    # PSUM budget (16KB/partition): pss "s" 2x[128,2,CH] = 8KB, pso "o"
    # 2x[128,CH] + "w" 1x[128,CH] = 6KB, psd "den2" 1x[2,CH] = 2KB.
    # Broadcast scratch and the final wo emission reuse the pss "s" slots.
    # ps_w gets its own tag so the deferred wo matmuls interleave freely into
    # the attention exp-latency gaps instead of queueing behind ps_o's WAR.
    with (
        tc.tile_pool(name="att", bufs=4) as ap,
        tc.tile_pool(name="attb", bufs=2) as ab,
        tc.tile_pool(name="oTs", bufs=4) as otp,
        tc.tile_pool(name="outp", bufs=6) as outp,
        tc.tile_pool(name="att_s", bufs=2, space="PSUM") as pss,
        tc.tile_pool(name="att_o", bufs=2, space="PSUM") as pso,
        tc.tile_pool(name="att_d", bufs=1, space="PSUM") as psd,
    ):
        # ---- wo(m') emission: projection for s-tiles of chunk m' ----
        # Emitted piecewise, one output tile per attention j-iteration of the
        # NEXT chunk, so the wo matmuls fill the PE's exp-latency gaps.
        def wo_pieces(mm, oTc_mm, final=False):
            for u in range(4):
                st = 4 * mm + u
                for n in range(4):
                    ncols = slice(n * 512, (n + 1) * 512)
                    if final or piece_slot[0] == "s":
                        # score slots are idle at chunk boundaries / kernel
                        # end: a 2-deep pipelined drain instead of the
                        # single-bank "w" WAR chain
                        ps_w2 = pss.tile([128, HPC, CH], F32, tag="s", name="psw2")
                        ps_w = ps_w2[:, 0, :]
                    else:
                        ps_w = pso.tile([128, 512], F32, tag="w", name="psw", bufs=1)
                    nc.tensor.matmul(
                        ps_w,
                        oTc_mm[0][:, u * 128:(u + 1) * 128],
                        wo_sb[:, 0, ncols],
                        start=True,
                        stop=False,
                    )
                    nc.tensor.matmul(
                        ps_w,
                        oTc_mm[1][:, u * 128:(u + 1) * 128],
                        wo_sb[:, 1, ncols],
                        start=False,
                        stop=True,
                    )
                    osb = outp.tile([128, 512], F32, tag="ow")
                    # copies stay off the Act engine: at chunk boundaries an
                    # Act-side copy queues in front of the next chunk's exps,
                    # which hold the score slots and stall the QK stream
                    nc.vector.tensor_copy(osb, ps_w)
                    if (st * 4 + n) % 3 == 0:
                        nc.sync.dma_start(
                            out=out_d[st * 128:(st + 1) * 128, ncols], in_=osb
                        )
                    else:
                        nc.gpsimd.dma_start(
                            out=out_d[st * 128:(st + 1) * 128, ncols], in_=osb
                        )
                    yield True

        wo_queue = []  # pending wo piece generators (FIFO across chunks)
        piece_slot = ["w"]  # PSUM scratch for the next piece ("w" or "s")

        def fire_wo(n=1):
            # at most n pieces; a second piece per attention step would stall
            # the in-order PE stream on the single-bank ps_w WAR
            while n > 0 and wo_queue:
                if next(wo_queue[0], None) is None:
                    wo_queue.pop(0)
                else:
                    n -= 1

        for m in range(NCHUNK):
            nsk = 8 if m < 2 else 4 * m + 4
            ps_o = [pso.tile([128, CH], F32, tag="o", name=f"o{h}") for h in range(HPC)]
            oTc = [
                otp.tile([128, CH], F32R, tag="oTc", name=f"oTc{h}")
                for h in range(HPC)
            ]
            # precompute diagonal-mask tiles for this chunk's band (off the
            # exp->PV critical path): one broadcast matmul + one max for both
            # heads and all 4 band blocks at once
            mks = None
            if m >= 2:
                ps_bc = pss.tile([128, HPC, CH], F32, tag="s", name="ps_bc")
                for h in range(HPC):
                    nc.tensor.matmul(
                        ps_bc[:, h, :],
                        ones_r,
                        Ft[:, h, (4 * m - 8) * 128:(4 * m - 4) * 128],
                        start=True,
                        stop=True,
                    )
                mks = ab.tile([128, HPC, CH], F32, tag="mk", bufs=2)
                trik_b = trik.rearrange("p (a b t) -> p a b t", a=1, b=1).broadcast_to(
                    [128, HPC, 4, BLOCK]
                )
                nc.vector.tensor_tensor(
                    mks.rearrange("p h (b t) -> p h b t", b=4),
                    trik_b,
                    ps_bc.rearrange("p h (b t) -> p h b t", b=4),
                    op=mybir.AluOpType.max,
                )
            ps_den = psd.tile([2, CH], F32, tag="den2", name="den2", bufs=1)

            def colspan(j):
                band = m >= 2 and j >= 4 * m
                # pad ap=128 matmuls (4 cyc/row below ap 256) to ap=256
                col0 = (j - 4 * m) * 128 if band else 0
                colp = min(col0, CH - 256) if band else 0
                return band, col0, colp

            def emit_qk_exp(j):
                band, col0, colp = colspan(j)
                ps_s = pss.tile([128, HPC, CH], F32, tag="s", bufs=2)
                for h in range(HPC):
                    nc.tensor.matmul(
                        ps_s[:, h, colp:],
                        kT[:, j * 128:(j + 1) * 128],
                        qT[h][:, m * CH + colp:(m + 1) * CH],
                        start=True,
                        stop=True,
                    )
                if colp < col0:
                    # overwrite the pad region so exp underflows to exact 0
                    nc.vector.memset(ps_s[:, :, colp:col0], PAD_NEG)
                pexp = ap.tile([128, HPC, CH], F32R, tag="pexp", bufs=4)
                nc.scalar.activation(
                    out=pexp[:, :, colp:],
                    in_=ps_s[:, :, colp:],
                    func=mybir.ActivationFunctionType.Exp,
                    scale=INV_SQRT_D,
                )
                if band:
                    nc.vector.tensor_tensor(
                        pexp[:, :, col0:col0 + 128],
                        pexp.bitcast(F32)[:, :, col0:col0 + 128],
                        mks[:, :, col0:col0 + 128],
                        op=mybir.AluOpType.mult,
                    )
                return pexp

            def emit_pv_den(j, pexp):
                _, _, colp = colspan(j)
                for h in range(HPC):
                    nc.tensor.matmul(
                        ps_o[h][:, colp:],
                        vN[:, j, :],
                        pexp[:, h, colp:],
                        start=(j == 0),
                        stop=(j == nsk - 1),
                    )
                    nc.tensor.matmul(
                        ps_den[:, colp:],
                        oh2[:, h, :],
                        pexp[:, h, colp:],
                        start=(j == 0 and h == 0),
                        stop=(j == nsk - 1 and h == HPC - 1),
                    )

            # software-pipeline by two j: QK/exp of j+1/j+2 are emitted before
            # PV/den of j, so the PE never sits in-order behind exp latency;
            # wo output tiles of the previous chunk (spread evenly over the
            # j-loop) fill the remaining slack
            pexps = [emit_qk_exp(0)]
            if nsk > 1:
                pexps.append(emit_qk_exp(1))
            for j in range(nsk):
                if j + 2 < nsk:
                    pexps.append(emit_qk_exp(j + 2))
                emit_pv_den(j, pexps[j])
                pexps[j] = None  # release reference
                # spread the ~16 pending pieces over all nsk steps (still at
                # most one per step), so late js keep their latency filler
                if len(wo_queue) > 1 or j % max(1, nsk // 16) == 0:
                    fire_wo(1)
            # drain bunched-up pieces via the (now idle) score slots so the
            # backlog overlaps the normalize chain 2-deep instead of
            # serializing on the single "w" bank
            piece_slot[0] = "s"
            fire_wo(3)
            piece_slot[0] = "w"
            # normalisation: reciprocal of the two dens, broadcast via the
            # "w" bank (keeps the "s" slots free so the next chunk's QKs can
            # start during this chain), then scale ps_o into fp32r oTc
            rec2 = ab.tile([2, CH], F32R, tag="rec")
            nc.vector.reciprocal(rec2, ps_den)
            bc_sb = ab.tile([128, HPC, CH], F32, tag="bcs")
            for h in range(HPC):
                ps_rb = pso.tile([128, 512], F32, tag="w", name="ps_rb", bufs=1)
                nc.tensor.matmul(
                    ps_rb, sel2[:, h, :], rec2, start=True, stop=True
                )
                nc.scalar.copy(bc_sb[:, h, :], ps_rb)
                nc.vector.tensor_tensor(
                    oTc[h], ps_o[h], bc_sb[:, h, :], op=mybir.AluOpType.mult
                )
            # deferred gating for the last two phase-1 chunks: the PE
            # matmuls double as filler during this chunk's boundary chain
            if m == 2:
                _gating(nc, 6, qT, bm, Ft, ident, pso, "w", ab)
            elif m == 3:
                _gating(nc, 7, qT, bm, Ft, ident, pso, "w", ab)
            wo_queue.append(
                wo_pieces(m, oTc, final=(m == NCHUNK - 1))
            )
        while wo_queue:
            fire_wo(1)
    wop_cm.__exit__(None, None, None)


def kernel(x, wq, wk, wv, wo):
    bs = np.asarray(x).shape[0]
    in_maps = make_in_maps(x, wq, wk, wv, wo)
    nc = _build_nc()
    res = run_bass_kernel_spmd(nc, in_maps, list(range(N_CORES)))
    out = res.results[0]["out"].astype(np.float64)
    for c in range(1, N_CORES):
        out += res.results[c]["out"]
    return out.astype(np.float32).reshape(bs, SEQ, DIM)


if __name__ == "__main__":
    rng = np.random.default_rng(0)
    xs = {
        "x": rng.standard_normal((1, SEQ, DIM), dtype=np.float32),
        "wq": rng.standard_normal((DIM, DIM), dtype=np.float32) * (DIM ** -0.5),
        "wk": rng.standard_normal((DIM, DIM // 2), dtype=np.float32) * (DIM ** -0.5),
        "wv": rng.standard_normal((DIM, DIM // 2), dtype=np.float32) * (DIM ** -0.5),
        "wo": rng.standard_normal((DIM, DIM), dtype=np.float32) * (DIM ** -0.5),
    }
    out = kernel(**xs)
    print("out", out.shape, out.dtype, np.abs(out).max())


# revision 54
# speedup vs baseline: 1.1073x; 1.0061x over previous
"""MixtureOfBlockAttention TRN2 kernel — 8-core head-parallel (TP) Bass/Tile implementation.

Semantics (verified equivalent to the reference, rel err ~2e-2 budget dominated
by top-k near-tie flips caused by fp32r rounding of x/w — irreducible without
dropping fp32r):
the reference mask `maximum(token_mask, causal*NEG_INF)` masks a position iff
it is BOTH future AND in a non-selected block. Consequences:
  - query blocks 0..7 attend to ALL tokens of key blocks 0..7 (dense, no mask);
  - query block i>=8 attends densely to key blocks 0..i-1, and within its own
    (diagonal) block applies strict causal masking ONLY for rows whose own
    block is not among their top-8 gating blocks.
Selection rank for query s in block i (i>=8): own block selected iff
  #{j < i : g[s,j] > g[s,i]} < 8, with g = q . (block sums of roped k)
(positive-scale invariant, so block sums replace means and the 1/sqrt(d)
factor is dropped).

Sharding: 16 query heads / 8 cores = 2 heads per core; KV head c serves both.
wq/wk/wv column-sliced, wo row-sliced; partial outputs summed on host.
Host-side layout prep: x is transposed to xT[c, s] (the PE contracts over the
partition dim, so both matmul operands need c on partitions) and float inputs
are pre-rounded to fp32r; both are pure data-layout transforms.

All big matmuls run in float32r (TF32-like input rounding, fp32 accumulate,
full PE rate at N>=256). Optimizations vs the original baseline (all
numerically neutral except the rope-table recurrence, which shifts the final
rel err by <1e-6 with no top-k flips):
  - per-j exp fused across the 2 heads (one Act instruction, PSUM [128,2,512]);
  - softmax denominators for both heads accumulate into one [2,512] PSUM bank
    via [128,2] one-hot-column stationaries;
  - reciprocal-broadcast and notflag-broadcast matmuls emitted at ap>=512
    instead of 4x/8x ap=128 pieces (fp32r runs 4 cyc/row below ap 256);
  - diagonal-band j=4m+3 matmuls padded from ap=128 to ap=256 with a -1e5
    PSUM memset in the pad so exp underflows to exact 0 there;
  - V-transpose and notflag-transpose in fp32r (1.5 cyc/row) not fp32 (2.0);
  - rope cos/sin tables generated on device by a per-chunk angle-rotation
    recurrence (saves 4MB/core of HBM reads and the phase-1 DMA deficit);
  - attention j-loop software-pipelined by one step (QK/exp of j+1 emitted
    before PV/den of j) so the in-order PE stream hides the exp latency;
  - the deferred wo projection is emitted as 16 per-chunk output tiles fired
    at most one per attention step from a cross-chunk FIFO, filling the PE's
    residual exp-latency slack without ever stalling it on the ps_w WAR;
  - gating reordered (all matmuls, then DVE compare chains, then batched
    transposes) and chunks 6-7's gating deferred into the attention phase;
  - PSUM: scores 2x[128,2,512] + 2 ps_o + 1 ps_w + 1 den2 = exactly 8 banks.
"""

import math
import sys

import numpy as np

if "/opt/trn_rl_repo" not in sys.path:
    sys.path.insert(0, "/opt/trn_rl_repo")

import concourse.bacc as bacc
import concourse.mybir as mybir
import concourse.tile as tile
from concourse.bass_utils import run_bass_kernel_spmd

F32 = mybir.dt.float32
F32R = mybir.dt.float32r

SEQ = 4096
DIM = 2048
HEAD_DIM = 128
N_HEADS = 16
N_CORES = 8
HPC = N_HEADS // N_CORES       # heads per core = 2
DPC = HPC * HEAD_DIM           # q/o dims per core = 256
BLOCK = 128
NB = SEQ // BLOCK              # 32 key blocks
TOPK = 8
NCHUNK = 8                     # s-chunks of 512
CH = SEQ // NCHUNK             # 512
NCT = DIM // 128               # 16 contraction tiles
INV_SQRT_D = 1.0 / math.sqrt(HEAD_DIM)
PAD_NEG = -100000.0

_CACHE = {}


def _round_fp32r(a):
    """Round fp32 to the fp32r grid (top-11-bit mantissa, round-to-nearest)."""
    a = np.ascontiguousarray(a, dtype=np.float32)
    try:
        from neuron_dtypes import static_cast_fp32_to_fp32r

        return static_cast_fp32_to_fp32r(a).view(np.float32).astype(np.float32)
    except Exception:
        u = a.view(np.uint32)
        return ((u + np.uint32(0x800)) & np.uint32(0xFFFFF000)).view(np.float32).copy()


def _host_constants():
    if "consts" in _CACHE:
        return _CACHE["consts"]
    p = np.arange(HEAD_DIM // 2, dtype=np.float64)
    inv_freq = 1.0 / (10000.0 ** (2.0 * p / HEAD_DIM))
    ang = np.arange(SEQ, dtype=np.float64)[None, :] * inv_freq[:, None]  # [64, S]
    cos = np.cos(ang).astype(np.float32)
    sin = np.sin(ang).astype(np.float32)
    cos_ds = np.ascontiguousarray(np.repeat(cos, 2, axis=0))   # [128, S]
    sin_ds = np.empty((HEAD_DIM, SEQ), dtype=np.float32)       # signed sin
    sin_ds[0::2] = -sin
    sin_ds[1::2] = sin
    # per-partition rotation by CH positions: next-chunk tables via
    # cos' = cos*C - sin_ds*S_row ; sin_ds' = sin_ds*C + cos*S_row
    # (S_row carries the sign convention of the interleaved sin_ds rows)
    inv_freq = 1.0 / (10000.0 ** (2.0 * p / HEAD_DIM))
    c512 = np.cos(CH * inv_freq)
    s512 = np.sin(CH * inv_freq)
    rotC = np.repeat(c512, 2).astype(np.float32)[:, None]      # [128, 1]
    rotS = np.empty((HEAD_DIM,), dtype=np.float64)
    rotS[0::2] = -s512
    rotS[1::2] = s512
    rotS = rotS.astype(np.float32)[:, None]                    # [128, 1]
    pswap = np.zeros((128, 128), dtype=np.float32)             # swap 2p <-> 2p+1
    idx = np.arange(128)
    pswap[idx, idx ^ 1] = 1.0
    identm = np.eye(128, dtype=np.float32)
    r = np.arange(BLOCK)
    trikeep = (r[:, None] <= r[None, :]).astype(np.float32)    # keep iff sk <= sq
    ones_row = np.ones((1, 128), dtype=np.float32)
    # one-hot-column stationaries for per-head den accumulation into [2, CH]:
    # oh2[:, h, :] is [128, 2] with column h all-ones
    oh2 = np.zeros((128, 2, 2), dtype=np.float32)
    oh2[:, 0, 0] = 1.0
    oh2[:, 1, 1] = 1.0
    # one-hot-row stationaries for per-head [2,CH] -> [128,CH] broadcast:
    # sel2[:, h, :] is [2, 128] with row h all-ones
    sel2 = np.zeros((2, 2, 128), dtype=np.float32)
    sel2[0, 0, :] = 1.0
    sel2[1, 1, :] = 1.0
    _CACHE["consts"] = (cos_ds, sin_ds, rotC, rotS, pswap, identm, trikeep, ones_row, oh2, sel2)
    return _CACHE["consts"]


def make_in_maps(x, wq, wk, wv, wo):
    """Shard + lay out the full inputs for the 8 cores."""
    x2 = np.asarray(x, dtype=np.float32).reshape(SEQ, DIM)
    xT = _round_fp32r(np.ascontiguousarray(x2.T))
    wq = np.asarray(wq, dtype=np.float32)
    wk = np.asarray(wk, dtype=np.float32)
    wv = np.asarray(wv, dtype=np.float32)
    wo = np.asarray(wo, dtype=np.float32)
    cos_ds, sin_ds, rotC, rotS, pswap, identm, trikeep, ones_row, oh2, sel2 = _host_constants()
    pswap_r = _round_fp32r(pswap)
    ones_row_r = _round_fp32r(ones_row)
    oh2_r = _round_fp32r(oh2)
    sel2_r = _round_fp32r(sel2)
    in_maps = []
    for c in range(N_CORES):
        in_maps.append(
            {
                "xT": xT,
                "wq": _round_fp32r(wq[:, c * DPC:(c + 1) * DPC]),
                "wk": _round_fp32r(wk[:, c * HEAD_DIM:(c + 1) * HEAD_DIM]),
                "wv": _round_fp32r(wv[:, c * HEAD_DIM:(c + 1) * HEAD_DIM]),
                "wo": _round_fp32r(wo[c * DPC:(c + 1) * DPC, :]),
                "cos0": np.ascontiguousarray(cos_ds[:, 0:CH]),
                "sin0": np.ascontiguousarray(sin_ds[:, 0:CH]),
                "rotC": rotC,
                "rotS": rotS,
                "pswap": pswap_r,
                "identm": _round_fp32r(identm),
                "trikeep": trikeep,
                "ones_r": ones_row_r,
                "oh2": oh2_r,
                "sel2": sel2_r,
            }
        )
    return in_maps


def _gating(nc, m, qT, bm, Ft, ident, ps_pool, ps_tag, sb_pool):
    """Own-block top-k flags for chunk m's 4 query blocks (both heads).

    All 8 gating matmuls first, then the DVE compare chains, then the 8
    transposes batched 4-per-PSUM-bank, so the in-order PE stream never
    waits mid-chain. PSUM scratch comes from (ps_pool, ps_tag) so this can
    run late, inside the attention phase, for the last two chunks.
    """
    import concourse.mybir as mybir

    F32 = mybir.dt.float32
    F32R = mybir.dt.float32r
    pairs = [(h, i) for h in range(HPC) for i in range(4 * m, 4 * m + 4)]
    nbk = 4 * m + 4  # even N; cols > i unused
    ps_g8 = ps_pool.tile([128, 8, NB], F32, tag=ps_tag, bufs=1, name="g8")
    for p, (h, i) in enumerate(pairs):
        nc.tensor.matmul(
            ps_g8[:, p, 0:nbk],
            qT[h][:, i * 128:(i + 1) * 128],
            bm[:, 0:nbk],
            start=True,
            stop=True,
        )
    nfs = []
    for p, (h, i) in enumerate(pairs):
        cmp = sb_pool.tile([128, NB], F32, tag="cmp", bufs=2, name="cmp")
        cnt = sb_pool.tile([128, 1], F32, tag="cnt", bufs=2, name="cnt")
        nc.vector.tensor_scalar(
            out=cmp[:, 0:i],
            in0=ps_g8[:, p, 0:i],
            scalar1=ps_g8[:, p, i:i + 1],
            scalar2=None,
            op0=mybir.AluOpType.is_gt,
        )
        nc.vector.tensor_reduce(
            cnt, cmp[:, 0:i], axis=mybir.AxisListType.X, op=mybir.AluOpType.add
        )
        # notflag: 1.0 -> own block selected (keep all)
        # (fp32r tiles: values are exactly 0.0/1.0)
        nf = sb_pool.tile([128, 1], F32R, tag="nf", bufs=8, name=f"nf{p}")
        nc.vector.tensor_scalar(
            out=nf,
            in0=cnt,
            scalar1=float(TOPK) - 0.5,
            scalar2=None,
            op0=mybir.AluOpType.is_lt,
        )
        nfs.append(nf)
    for h in range(HPC):
        ps_ft4 = ps_pool.tile([1, 4, 128], F32, tag=ps_tag, bufs=1, name="ft4")
        for t in range(4):
            nc.tensor.transpose(
                ps_ft4.bitcast(F32R)[:, t, :], nfs[4 * h + t], ident
            )
        nc.vector.tensor_copy(
            Ft[:, h, (4 * m - 8) * 128:(4 * m - 4) * 128],
            ps_ft4.rearrange("o f t -> o (f t)"),
        )



def _build_nc(reps=1):
    key = f"nc{reps}"
    if key in _CACHE:
        return _CACHE[key]
    nc = bacc.Bacc(None, target_bir_lowering=False)

    xT_d = nc.dram_tensor("xT", [DIM, SEQ], F32R, kind="ExternalInput")
    wq_d = nc.dram_tensor("wq", [DIM, DPC], F32R, kind="ExternalInput")
    wk_d = nc.dram_tensor("wk", [DIM, HEAD_DIM], F32R, kind="ExternalInput")
    wv_d = nc.dram_tensor("wv", [DIM, HEAD_DIM], F32R, kind="ExternalInput")
    wo_d = nc.dram_tensor("wo", [DPC, DIM], F32R, kind="ExternalInput")
    cos_d = nc.dram_tensor("cos0", [HEAD_DIM, CH], F32, kind="ExternalInput")
    sin_d = nc.dram_tensor("sin0", [HEAD_DIM, CH], F32, kind="ExternalInput")
    rotc_d = nc.dram_tensor("rotC", [HEAD_DIM, 1], F32, kind="ExternalInput")
    rots_d = nc.dram_tensor("rotS", [HEAD_DIM, 1], F32, kind="ExternalInput")
    psw_d = nc.dram_tensor("pswap", [128, 128], F32R, kind="ExternalInput")
    idm_d = nc.dram_tensor("identm", [128, 128], F32R, kind="ExternalInput")
    trk_d = nc.dram_tensor("trikeep", [BLOCK, BLOCK], F32, kind="ExternalInput")
    onr_d = nc.dram_tensor("ones_r", [1, 128], F32R, kind="ExternalInput")
    oh2_d = nc.dram_tensor("oh2", [128, 2, 2], F32R, kind="ExternalInput")
    sel2_d = nc.dram_tensor("sel2", [2, 2, 128], F32R, kind="ExternalInput")
    out_d = nc.dram_tensor("out", [SEQ, DIM], F32, kind="ExternalOutput")

    with tile.TileContext(nc) as tc, nc.allow_low_precision(
        reason="float32r rounding of matmul operands is intentional"
    ):
      for _rep in range(reps):
        with tc.tile_pool(name="persist", bufs=1) as per:
            qT = [per.tile([128, SEQ], F32R, tag=f"qT{h}", name=f"qT{h}") for h in range(HPC)]
            kT = per.tile([128, SEQ], F32R, tag="kT")
            vN = per.tile([128, NB, 128], F32R, tag="vN")   # [s-in-tile, sk-tile, d]
            ident = per.tile([128, 128], F32R, tag="ident")
            pswap = per.tile([128, 128], F32R, tag="pswap")
            trik = per.tile([BLOCK, BLOCK], F32, tag="trik")
            ones_r = per.tile([1, 128], F32R, tag="ones_r")
            oh2 = per.tile([128, 2, 2], F32R, tag="oh2")    # [k, h, den-col]
            sel2 = per.tile([2, 2, 128], F32R, tag="sel2")  # [den-row, h, p]
            bm = per.tile([128, NB], F32R, tag="bm")
            # per-head notflag rows: Ft[0, h, (i-TOPK)*128:...] is the [1,128]
            # notflag row for query block i of head h, at base partition 0
            Ft = per.tile([1, HPC, (NB - TOPK) * 128], F32R, tag="Ft")

            # dummy exp so the Exp act-table load overlaps the initial weight
            # DMAs instead of stalling the first attention chunk
            warm = per.tile([1, 1], F32, tag="warm")
            nc.vector.memset(warm, 0.0)
            nc.scalar.activation(
                out=warm, in_=warm, func=mybir.ActivationFunctionType.Exp
            )

            # ---------------- phase 1: projections + rope -------------------
            with (
                tc.tile_pool(name="wpool", bufs=1) as wp,
                tc.tile_pool(name="xtp", bufs=17) as xtp,
                tc.tile_pool(name="ropep", bufs=2) as rp,
                tc.tile_pool(name="csin", bufs=2) as csp,
                # acc_ps declared first so its PSUM range lines up with the
                # attention score pool's range: the last acc_ps readers (rope
                # copies) finish well before the gating tail that occupies
                # pj_ps, letting chunk-0 QK matmuls start during the tail
                tc.tile_pool(name="acc_ps", bufs=4, space="PSUM") as accps,
                tc.tile_pool(name="pj_ps", bufs=2, space="PSUM") as trps,
            ):
                wq_sb = wp.tile([128, NCT, DPC], F32R, tag="wq")
                wk_sb = wp.tile([128, NCT, HEAD_DIM], F32R, tag="wk")
                wv_sb = wp.tile([128, NCT, HEAD_DIM], F32R, tag="wv")
                wq_r = wq_d.rearrange("(t p) d -> p t d", p=128)
                wk_r = wk_d.rearrange("(t p) d -> p t d", p=128)
                wv_r = wv_d.rearrange("(t p) d -> p t d", p=128)
                # k/v weights first: chunk 0 runs its k/v matmuls while the
                # (2x bigger) wq still streams in
                nc.gpsimd.dma_start(out=wk_sb[:, 0:4, :], in_=wk_r[:, 0:4, :])
                nc.gpsimd.dma_start(out=wv_sb[:, 0:4, :], in_=wv_r[:, 0:4, :])
                nc.gpsimd.dma_start(out=wk_sb[:, 4:16, :], in_=wk_r[:, 4:16, :])
                nc.gpsimd.dma_start(out=wv_sb[:, 4:16, :], in_=wv_r[:, 4:16, :])
                nc.gpsimd.dma_start(out=wq_sb[:, 0:8, :], in_=wq_r[:, 0:8, :])
                nc.gpsimd.dma_start(out=wq_sb[:, 8:16, :], in_=wq_r[:, 8:16, :])

                rot_c = wp.tile([128, 1], F32, tag="rotc")
                nc.scalar.dma_start(out=rot_c, in_=rotc_d[:])
                rot_s = wp.tile([128, 1], F32, tag="rots")
                nc.scalar.dma_start(out=rot_s, in_=rots_d[:])
                gp = wp  # reuse the bufs=1 pool scope for small gating tiles
                for m in range(NCHUNK):
                    cols = slice(m * CH, (m + 1) * CH)
                    ps_q0 = accps.tile([128, CH], F32, tag="acc")
                    ps_q1 = accps.tile([128, CH], F32, tag="acc")
                    ps_k = accps.tile([128, CH], F32, tag="acc")
                    ps_v = accps.tile([128, CH], F32, tag="acc")
                    if m <= 1:
                        # two passes (k/v then q) to match the weight-arrival
                        # order; the xt tiles stay resident for the q pass
                        xts = []
                        for cc in range(NCT):
                            xt = xtp.tile([128, CH], F32R, tag="xt")
                            # first tiles ride the otherwise-idle Act HWDGE
                            # queue in parallel with the sync queue
                            q = nc.scalar if (m == 0 and cc < 4) else nc.sync
                            q.dma_start(
                                out=xt, in_=xT_d[cc * 128:(cc + 1) * 128, cols]
                            )
                            xts.append(xt)
                            st0, sp0 = (cc == 0), (cc == NCT - 1)
                            nc.tensor.matmul(ps_k, wk_sb[:, cc, :], xt, start=st0, stop=sp0)
                            nc.tensor.matmul(ps_v, wv_sb[:, cc, :], xt, start=st0, stop=sp0)
                        # consts (needed from the rope stage onwards) follow
                        # the early x tiles on the Act HWDGE queue
                        if m == 0:
                          nc.scalar.dma_start(out=pswap, in_=psw_d[:])
                          nc.scalar.dma_start(out=ident, in_=idm_d[:])
                          nc.scalar.dma_start(out=trik, in_=trk_d[:])
                          nc.scalar.dma_start(out=ones_r, in_=onr_d[:])
                          nc.scalar.dma_start(out=oh2, in_=oh2_d[:])
                          nc.scalar.dma_start(out=sel2, in_=sel2_d[:])
                        for cc in range(NCT):
                            st0, sp0 = (cc == 0), (cc == NCT - 1)
                            nc.tensor.matmul(ps_q0, wq_sb[:, cc, 0:128], xts[cc], start=st0, stop=sp0)
                            nc.tensor.matmul(ps_q1, wq_sb[:, cc, 128:256], xts[cc], start=st0, stop=sp0)
                    else:
                      for cc in range(NCT):
                        xt = xtp.tile([128, CH], F32R, tag="xt")
                        nc.sync.dma_start(
                            out=xt, in_=xT_d[cc * 128:(cc + 1) * 128, cols]
                        )
                        st0, sp0 = (cc == 0), (cc == NCT - 1)
                        nc.tensor.matmul(ps_q0, wq_sb[:, cc, 0:128], xt, start=st0, stop=sp0)
                        nc.tensor.matmul(ps_q1, wq_sb[:, cc, 128:256], xt, start=st0, stop=sp0)
                        nc.tensor.matmul(ps_k, wk_sb[:, cc, :], xt, start=st0, stop=sp0)
                        nc.tensor.matmul(ps_v, wv_sb[:, cc, :], xt, start=st0, stop=sp0)

                    if m == 0:
                        cos_t = csp.tile([128, CH], F32, tag="cos", bufs=2)
                        nc.scalar.dma_start(out=cos_t, in_=cos_d[:])
                        sin_t = csp.tile([128, CH], F32, tag="sin", bufs=2)
                        nc.scalar.dma_start(out=sin_t, in_=sin_d[:])
                    else:
                        # rotate the previous chunk's tables by CH positions
                        # (per-partition angle), off the DMA wire entirely
                        cos_p, sin_p = cos_t, sin_t
                        ta = rp.tile([128, CH], F32, tag="t2")
                        nc.vector.tensor_scalar(
                            out=ta, in0=sin_p, scalar1=rot_s, scalar2=None,
                            op0=mybir.AluOpType.mult,
                        )
                        cos_t = csp.tile([128, CH], F32, tag="cos", bufs=2)
                        nc.vector.scalar_tensor_tensor(
                            out=cos_t, in0=cos_p, scalar=rot_c, in1=ta,
                            op0=mybir.AluOpType.mult,
                            op1=mybir.AluOpType.subtract,
                        )
                        tb = rp.tile([128, CH], F32, tag="t2")
                        nc.vector.tensor_scalar(
                            out=tb, in0=cos_p, scalar1=rot_s, scalar2=None,
                            op0=mybir.AluOpType.mult,
                        )
                        sin_t = csp.tile([128, CH], F32, tag="sin", bufs=2)
                        nc.vector.scalar_tensor_tensor(
                            out=sin_t, in0=sin_p, scalar=rot_c, in1=tb,
                            op0=mybir.AluOpType.mult,
                            op1=mybir.AluOpType.add,
                        )

                    # V: evacuate then PE-transpose to natural [s, d] layout
                    # (fp32r copy: vN is fp32r anyway, and fp32r transpose runs
                    # 1.5 cyc/row vs 2.0 for fp32)
                    vtmp = rp.tile([128, CH], F32R, tag="qraw2")
                    nc.vector.tensor_copy(vtmp, ps_v)
                    ps_vt = trps.tile([128, CH], F32, tag="tr")
                    for u in range(4):
                        nc.tensor.transpose(
                            ps_vt.bitcast(F32R)[:, u * 128:(u + 1) * 128],
                            vtmp[:, u * 128:(u + 1) * 128],
                            ident,
                        )
                    nc.vector.tensor_copy(
                        vN[:, 4 * m:4 * m + 4, :],
                        ps_vt.rearrange("p (u d) -> p u d", u=4),
                    )

                    # all three raw copies first: they are the last readers
                    # of the projection PSUM banks, and freeing those banks
                    # early unblocks the next chunk's matmuls (and, for the
                    # last chunk, the attention start) via the address WAR
                    raws = []
                    for psrc in (ps_q0, ps_q1, ps_k):
                        raw = rp.tile([128, CH], F32R, tag="qraw", bufs=3)
                        nc.vector.tensor_copy(raw, psrc)
                        raws.append(raw)
                    for raw, dstT in zip(raws, (qT[0], qT[1], kT)):
                        ps_sw = trps.tile([128, CH], F32, tag="tr")
                        nc.tensor.matmul(ps_sw, pswap, raw, start=True, stop=True)
                        t2 = rp.tile([128, CH], F32, tag="t2")
                        nc.vector.tensor_tensor(
                            t2, raw.bitcast(F32), cos_t, op=mybir.AluOpType.mult
                        )
                        # sw *= sin in place (PSUM), then add -> rope output
                        nc.vector.tensor_tensor(ps_sw, ps_sw, sin_t, op=mybir.AluOpType.mult)
                        nc.vector.tensor_tensor(
                            dstT[:, cols], t2, ps_sw, op=mybir.AluOpType.add
                        )

                    # partial block sums for this chunk's 4 key blocks
                    nc.vector.tensor_reduce(
                        bm[:, 4 * m:4 * m + 4],
                        kT.bitcast(F32)[:, cols].rearrange("p (b t) -> p b t", b=4),
                        axis=mybir.AxisListType.X,
                        op=mybir.AluOpType.add,
                    )
                    # gating flags for this chunk's query blocks (needs
                    # bm 0..i); chunks 6-7 are deferred into the attention
                    # phase so the phase boundary is not serialized on them
                    if 2 <= m <= 5:
                        _gating(nc, m, qT, bm, Ft, ident, trps, "g", gp)

            # ---------------- phases 3+4 ------------------------------------
            _phase34(nc, tc, qT, kT, vN, trik, oh2, sel2, ones_r, Ft, wo_d,
                     out_d, bm, ident)

    nc.compile()
    _CACHE[key] = nc
    return nc


def _phase34(nc, tc, qT, kT, vN, trik, oh2, sel2, ones_r, Ft, wo_d, out_d,
             bm, ident):
    wop_cm = tc.tile_pool(name="wop", bufs=1)
    wop = wop_cm.__enter__()
    wo_sb = wop.tile([128, HPC, DIM], F32R, tag="wo")
    nc.gpsimd.dma_start(out=wo_sb, in_=wo_d.rearrange("(t p) d -> p t d", p=128))
    # ------- phase 3: attention with interleaved output projection -------
    # (wo(m) right after attn(m) so the 32MB output DMA spreads over the
    # whole kernel instead of piling into a DMA-bound tail phase)
    # PSUM budget (16KB/partition): pss "s" 2x[128,2,CH] = 8KB, pso "o"
    # 2x[128,CH] + "w" 1x[128,CH] = 6KB, psd "den2" 1x[2,CH] = 2KB.
    # Broadcast scratch and the final wo emission reuse the pss "s" slots.
    # ps_w gets its own tag so the deferred wo matmuls interleave freely into
    # the attention exp-latency gaps instead of queueing behind ps_o's WAR.
    with (
        tc.tile_pool(name="att", bufs=4) as ap,
        tc.tile_pool(name="attb", bufs=2) as ab,
        tc.tile_pool(name="oTs", bufs=4) as otp,
        tc.tile_pool(name="outp", bufs=6) as outp,
        tc.tile_pool(name="att_s", bufs=2, space="PSUM") as pss,
        tc.tile_pool(name="att_o", bufs=2, space="PSUM") as pso,
        tc.tile_pool(name="att_d", bufs=1, space="PSUM") as psd,
    ):
        # ---- wo(m') emission: projection for s-tiles of chunk m' ----
        # Emitted piecewise, one output tile per attention j-iteration of the
        # NEXT chunk, so the wo matmuls fill the PE's exp-latency gaps.
        def wo_pieces(mm, oTc_mm, final=False):
            for u in range(4):
                st = 4 * mm + u
                for n in range(4):
                    ncols = slice(n * 512, (n + 1) * 512)
                    if final or piece_slot[0] in ("s", "f"):
                        # score slots are idle at chunk boundaries / kernel
                        # end: a 2-deep pipelined drain instead of the
                        # single-bank "w" WAR chain
                        ps_w2 = pss.tile([128, HPC, CH], F32, tag="s", name="psw2")
                        ps_w = ps_w2[:, 0, :]
                    else:
                        ps_w = pso.tile([128, 512], F32, tag="w", name="psw", bufs=1)
                    nc.tensor.matmul(
                        ps_w,
                        oTc_mm[0][:, u * 128:(u + 1) * 128],
                        wo_sb[:, 0, ncols],
                        start=True,
                        stop=False,
                    )
                    nc.tensor.matmul(
                        ps_w,
                        oTc_mm[1][:, u * 128:(u + 1) * 128],
                        wo_sb[:, 1, ncols],
                        start=False,
                        stop=True,
                    )
                    osb = outp.tile([128, 512], F32, tag="ow")
                    # copies stay off the Act engine (at chunk boundaries an
                    # Act-side copy queues in front of the next chunk's exps,
                    # which hold the score slots and stall the QK stream) —
                    # except in the final drain, where Act is idle and
                    # alternating halves the copy-limited drain rate
                    if piece_slot[0] == "f" and (st * 4 + n) % 2 == 0:
                        nc.scalar.copy(osb, ps_w)
                    else:
                        nc.vector.tensor_copy(osb, ps_w)
                    if (st * 4 + n) % 3 == 0:
                        nc.sync.dma_start(
                            out=out_d[st * 128:(st + 1) * 128, ncols], in_=osb
                        )
                    else:
                        nc.gpsimd.dma_start(
                            out=out_d[st * 128:(st + 1) * 128, ncols], in_=osb
                        )
                    yield True

        wo_queue = []  # pending wo piece generators (FIFO across chunks)
        piece_slot = ["w"]  # PSUM scratch for the next piece ("w" or "s")

        def fire_wo(n=1):
            # at most n pieces; a second piece per attention step would stall
            # the in-order PE stream on the single-bank ps_w WAR
            while n > 0 and wo_queue:
                if next(wo_queue[0], None) is None:
                    wo_queue.pop(0)
                else:
                    n -= 1

        for m in range(NCHUNK):
            nsk = 8 if m < 2 else 4 * m + 4
            ps_o = [pso.tile([128, CH], F32, tag="o", name=f"o{h}") for h in range(HPC)]
            oTc = [
                otp.tile([128, CH], F32R, tag="oTc", name=f"oTc{h}")
                for h in range(HPC)
            ]
            # precompute diagonal-mask tiles for this chunk's band (off the
            # exp->PV critical path): one broadcast matmul + one max for both
            # heads and all 4 band blocks at once
            mks = None
            if m >= 2:
                ps_bc = pss.tile([128, HPC, CH], F32, tag="s", name="ps_bc")
                for h in range(HPC):
                    nc.tensor.matmul(
                        ps_bc[:, h, :],
                        ones_r,
                        Ft[:, h, (4 * m - 8) * 128:(4 * m - 4) * 128],
                        start=True,
                        stop=True,
                    )
                mks = ab.tile([128, HPC, CH], F32, tag="mk", bufs=2)
                trik_b = trik.rearrange("p (a b t) -> p a b t", a=1, b=1).broadcast_to(
                    [128, HPC, 4, BLOCK]
                )
                nc.vector.tensor_tensor(
                    mks.rearrange("p h (b t) -> p h b t", b=4),
                    trik_b,
                    ps_bc.rearrange("p h (b t) -> p h b t", b=4),
                    op=mybir.AluOpType.max,
                )
            ps_den = psd.tile([2, CH], F32, tag="den2", name="den2", bufs=1)

            def colspan(j):
                band = m >= 2 and j >= 4 * m
                # pad ap=128 matmuls (4 cyc/row below ap 256) to ap=256
                col0 = (j - 4 * m) * 128 if band else 0
                colp = min(col0, CH - 256) if band else 0
                return band, col0, colp

            def emit_qk_exp(j):
                band, col0, colp = colspan(j)
                ps_s = pss.tile([128, HPC, CH], F32, tag="s", bufs=2)
                for h in range(HPC):
                    nc.tensor.matmul(
                        ps_s[:, h, colp:],
                        kT[:, j * 128:(j + 1) * 128],
                        qT[h][:, m * CH + colp:(m + 1) * CH],
                        start=True,
                        stop=True,
                    )
                if colp < col0:
                    # overwrite the pad region so exp underflows to exact 0
                    nc.vector.memset(ps_s[:, :, colp:col0], PAD_NEG)
                pexp = ap.tile([128, HPC, CH], F32R, tag="pexp", bufs=4)
                nc.scalar.activation(
                    out=pexp[:, :, colp:],
                    in_=ps_s[:, :, colp:],
                    func=mybir.ActivationFunctionType.Exp,
                    scale=INV_SQRT_D,
                )
                if band:
                    nc.vector.tensor_tensor(
                        pexp[:, :, col0:col0 + 128],
                        pexp.bitcast(F32)[:, :, col0:col0 + 128],
                        mks[:, :, col0:col0 + 128],
                        op=mybir.AluOpType.mult,
                    )
                return pexp

            def emit_pv_den(j, pexp):
                _, _, colp = colspan(j)
                for h in range(HPC):
                    nc.tensor.matmul(
                        ps_o[h][:, colp:],
                        vN[:, j, :],
                        pexp[:, h, colp:],
                        start=(j == 0),
                        stop=(j == nsk - 1),
                    )
                    nc.tensor.matmul(
                        ps_den[:, colp:],
                        oh2[:, h, :],
                        pexp[:, h, colp:],
                        start=(j == 0 and h == 0),
                        stop=(j == nsk - 1 and h == HPC - 1),
                    )

            # software-pipeline by two j: QK/exp of j+1/j+2 are emitted before
            # PV/den of j, so the PE never sits in-order behind exp latency;
            # wo output tiles of the previous chunk (spread evenly over the
            # j-loop) fill the remaining slack
            pexps = [emit_qk_exp(0)]
            if nsk > 1:
                pexps.append(emit_qk_exp(1))
            for j in range(nsk):
                if j + 2 < nsk:
                    pexps.append(emit_qk_exp(j + 2))
                emit_pv_den(j, pexps[j])
                pexps[j] = None  # release reference
                # spread the ~16 pending pieces over all nsk steps (still at
                # most one per step), so late js keep their latency filler
                if len(wo_queue) > 1 or j % max(1, nsk // 16) == 0:
                    fire_wo(1)
            # drain bunched-up pieces via the (now idle) score slots so the
            # backlog overlaps the normalize chain 2-deep instead of
            # serializing on the single "w" bank
            piece_slot[0] = "s"
            fire_wo(3)
            piece_slot[0] = "w"
            # normalisation: reciprocal of the two dens, broadcast via the
            # "w" bank (keeps the "s" slots free so the next chunk's QKs can
            # start during this chain), then scale ps_o into fp32r oTc
            rec2 = ab.tile([2, CH], F32R, tag="rec")
            nc.vector.reciprocal(rec2, ps_den)
            bc_sb = ab.tile([128, HPC, CH], F32, tag="bcs")
            for h in range(HPC):
                ps_rb = pso.tile([128, 512], F32, tag="w", name="ps_rb", bufs=1)
                nc.tensor.matmul(
                    ps_rb, sel2[:, h, :], rec2, start=True, stop=True
                )
                nc.scalar.copy(bc_sb[:, h, :], ps_rb)
                nc.vector.tensor_tensor(
                    oTc[h], ps_o[h], bc_sb[:, h, :], op=mybir.AluOpType.mult
                )
            # deferred gating for the last two phase-1 chunks: the PE
            # matmuls double as filler during this chunk's boundary chain
            if m == 2:
                _gating(nc, 6, qT, bm, Ft, ident, pso, "w", ab)
            elif m == 3:
                _gating(nc, 7, qT, bm, Ft, ident, pso, "w", ab)
            wo_queue.append(
                wo_pieces(m, oTc, final=(m == NCHUNK - 1))
            )
        piece_slot[0] = "f"
        while wo_queue:
            fire_wo(1)
    wop_cm.__exit__(None, None, None)


def kernel(x, wq, wk, wv, wo):
    bs = np.asarray(x).shape[0]
    in_maps = make_in_maps(x, wq, wk, wv, wo)
    nc = _build_nc()
    res = run_bass_kernel_spmd(nc, in_maps, list(range(N_CORES)))
    out = res.results[0]["out"].astype(np.float64)
    for c in range(1, N_CORES):
        out += res.results[c]["out"]
    return out.astype(np.float32).reshape(bs, SEQ, DIM)


if __name__ == "__main__":
    rng = np.random.default_rng(0)
    xs = {
        "x": rng.standard_normal((1, SEQ, DIM), dtype=np.float32),
        "wq": rng.standard_normal((DIM, DIM), dtype=np.float32) * (DIM ** -0.5),
        "wk": rng.standard_normal((DIM, DIM // 2), dtype=np.float32) * (DIM ** -0.5),
        "wv": rng.standard_normal((DIM, DIM // 2), dtype=np.float32) * (DIM ** -0.5),
        "wo": rng.standard_normal((DIM, DIM), dtype=np.float32) * (DIM ** -0.5),
    }
    out = kernel(**xs)
    print("out", out.shape, out.dtype, np.abs(out).max())


# revision 55
# speedup vs baseline: 1.1111x; 1.0034x over previous
"""MixtureOfBlockAttention TRN2 kernel — 8-core head-parallel (TP) Bass/Tile implementation.

Semantics (verified equivalent to the reference, rel err ~2e-2 budget dominated
by top-k near-tie flips caused by fp32r rounding of x/w — irreducible without
dropping fp32r):
the reference mask `maximum(token_mask, causal*NEG_INF)` masks a position iff
it is BOTH future AND in a non-selected block. Consequences:
  - query blocks 0..7 attend to ALL tokens of key blocks 0..7 (dense, no mask);
  - query block i>=8 attends densely to key blocks 0..i-1, and within its own
    (diagonal) block applies strict causal masking ONLY for rows whose own
    block is not among their top-8 gating blocks.
Selection rank for query s in block i (i>=8): own block selected iff
  #{j < i : g[s,j] > g[s,i]} < 8, with g = q . (block sums of roped k)
(positive-scale invariant, so block sums replace means and the 1/sqrt(d)
factor is dropped).

Sharding: 16 query heads / 8 cores = 2 heads per core; KV head c serves both.
wq/wk/wv column-sliced, wo row-sliced; partial outputs summed on host.
Host-side layout prep: x is transposed to xT[c, s] (the PE contracts over the
partition dim, so both matmul operands need c on partitions) and float inputs
are pre-rounded to fp32r; both are pure data-layout transforms.

All big matmuls run in float32r (TF32-like input rounding, fp32 accumulate,
full PE rate at N>=256). Optimizations vs the original baseline (all
numerically neutral except the rope-table recurrence, which shifts the final
rel err by <1e-6 with no top-k flips):
  - per-j exp fused across the 2 heads (one Act instruction, PSUM [128,2,512]);
  - softmax denominators for both heads accumulate into one [2,512] PSUM bank
    via [128,2] one-hot-column stationaries;
  - reciprocal-broadcast and notflag-broadcast matmuls emitted at ap>=512
    instead of 4x/8x ap=128 pieces (fp32r runs 4 cyc/row below ap 256);
  - diagonal-band j=4m+3 matmuls padded from ap=128 to ap=256 with a -1e5
    PSUM memset in the pad so exp underflows to exact 0 there;
  - V-transpose and notflag-transpose in fp32r (1.5 cyc/row) not fp32 (2.0);
  - rope cos/sin tables generated on device by a per-chunk angle-rotation
    recurrence (saves 4MB/core of HBM reads and the phase-1 DMA deficit);
  - attention j-loop software-pipelined by one step (QK/exp of j+1 emitted
    before PV/den of j) so the in-order PE stream hides the exp latency;
  - the deferred wo projection is emitted as 16 per-chunk output tiles fired
    at most one per attention step from a cross-chunk FIFO, filling the PE's
    residual exp-latency slack without ever stalling it on the ps_w WAR;
  - gating reordered (all matmuls, then DVE compare chains, then batched
    transposes) and chunks 6-7's gating deferred into the attention phase;
  - PSUM: scores 2x[128,2,512] + 2 ps_o + 1 ps_w + 1 den2 = exactly 8 banks.
"""

import math
import sys

import numpy as np

if "/opt/trn_rl_repo" not in sys.path:
    sys.path.insert(0, "/opt/trn_rl_repo")

import concourse.bacc as bacc
import concourse.mybir as mybir
import concourse.tile as tile
from concourse.bass_utils import run_bass_kernel_spmd

F32 = mybir.dt.float32
F32R = mybir.dt.float32r

SEQ = 4096
DIM = 2048
HEAD_DIM = 128
N_HEADS = 16
N_CORES = 8
HPC = N_HEADS // N_CORES       # heads per core = 2
DPC = HPC * HEAD_DIM           # q/o dims per core = 256
BLOCK = 128
NB = SEQ // BLOCK              # 32 key blocks
TOPK = 8
NCHUNK = 8                     # s-chunks of 512
CH = SEQ // NCHUNK             # 512
NCT = DIM // 128               # 16 contraction tiles
INV_SQRT_D = 1.0 / math.sqrt(HEAD_DIM)
PAD_NEG = -100000.0

_CACHE = {}


def _round_fp32r(a):
    """Round fp32 to the fp32r grid (top-11-bit mantissa, round-to-nearest)."""
    a = np.ascontiguousarray(a, dtype=np.float32)
    try:
        from neuron_dtypes import static_cast_fp32_to_fp32r

        return static_cast_fp32_to_fp32r(a).view(np.float32).astype(np.float32)
    except Exception:
        u = a.view(np.uint32)
        return ((u + np.uint32(0x800)) & np.uint32(0xFFFFF000)).view(np.float32).copy()


def _host_constants():
    if "consts" in _CACHE:
        return _CACHE["consts"]
    p = np.arange(HEAD_DIM // 2, dtype=np.float64)
    inv_freq = 1.0 / (10000.0 ** (2.0 * p / HEAD_DIM))
    ang = np.arange(SEQ, dtype=np.float64)[None, :] * inv_freq[:, None]  # [64, S]
    cos = np.cos(ang).astype(np.float32)
    sin = np.sin(ang).astype(np.float32)
    cos_ds = np.ascontiguousarray(np.repeat(cos, 2, axis=0))   # [128, S]
    sin_ds = np.empty((HEAD_DIM, SEQ), dtype=np.float32)       # signed sin
    sin_ds[0::2] = -sin
    sin_ds[1::2] = sin
    # per-partition rotation by CH positions: next-chunk tables via
    # cos' = cos*C - sin_ds*S_row ; sin_ds' = sin_ds*C + cos*S_row
    # (S_row carries the sign convention of the interleaved sin_ds rows)
    inv_freq = 1.0 / (10000.0 ** (2.0 * p / HEAD_DIM))
    c512 = np.cos(CH * inv_freq)
    s512 = np.sin(CH * inv_freq)
    rotC = np.repeat(c512, 2).astype(np.float32)[:, None]      # [128, 1]
    rotS = np.empty((HEAD_DIM,), dtype=np.float64)
    rotS[0::2] = -s512
    rotS[1::2] = s512
    rotS = rotS.astype(np.float32)[:, None]                    # [128, 1]
    pswap = np.zeros((128, 128), dtype=np.float32)             # swap 2p <-> 2p+1
    idx = np.arange(128)
    pswap[idx, idx ^ 1] = 1.0
    identm = np.eye(128, dtype=np.float32)
    r = np.arange(BLOCK)
    trikeep = (r[:, None] <= r[None, :]).astype(np.float32)    # keep iff sk <= sq
    ones_row = np.ones((1, 128), dtype=np.float32)
    # one-hot-column stationaries for per-head den accumulation into [2, CH]:
    # oh2[:, h, :] is [128, 2] with column h all-ones
    oh2 = np.zeros((128, 2, 2), dtype=np.float32)
    oh2[:, 0, 0] = 1.0
    oh2[:, 1, 1] = 1.0
    # one-hot-row stationaries for per-head [2,CH] -> [128,CH] broadcast:
    # sel2[:, h, :] is [2, 128] with row h all-ones
    sel2 = np.zeros((2, 2, 128), dtype=np.float32)
    sel2[0, 0, :] = 1.0
    sel2[1, 1, :] = 1.0
    _CACHE["consts"] = (cos_ds, sin_ds, rotC, rotS, pswap, identm, trikeep, ones_row, oh2, sel2)
    return _CACHE["consts"]


def make_in_maps(x, wq, wk, wv, wo):
    """Shard + lay out the full inputs for the 8 cores."""
    x2 = np.asarray(x, dtype=np.float32).reshape(SEQ, DIM)
    xT = _round_fp32r(np.ascontiguousarray(x2.T))
    wq = np.asarray(wq, dtype=np.float32)
    wk = np.asarray(wk, dtype=np.float32)
    wv = np.asarray(wv, dtype=np.float32)
    wo = np.asarray(wo, dtype=np.float32)
    cos_ds, sin_ds, rotC, rotS, pswap, identm, trikeep, ones_row, oh2, sel2 = _host_constants()
    pswap_r = _round_fp32r(pswap)
    ones_row_r = _round_fp32r(ones_row)
    oh2_r = _round_fp32r(oh2)
    sel2_r = _round_fp32r(sel2)
    in_maps = []
    for c in range(N_CORES):
        in_maps.append(
            {
                "xT": xT,
                "wq": _round_fp32r(wq[:, c * DPC:(c + 1) * DPC]),
                "wk": _round_fp32r(wk[:, c * HEAD_DIM:(c + 1) * HEAD_DIM]),
                "wv": _round_fp32r(wv[:, c * HEAD_DIM:(c + 1) * HEAD_DIM]),
                "wo": _round_fp32r(wo[c * DPC:(c + 1) * DPC, :]),
                "cos0": np.ascontiguousarray(cos_ds[:, 0:CH]),
                "sin0": np.ascontiguousarray(sin_ds[:, 0:CH]),
                "rotC": rotC,
                "rotS": rotS,
                "pswap": pswap_r,
                "identm": _round_fp32r(identm),
                "trikeep": trikeep,
                "ones_r": ones_row_r,
                "oh2": oh2_r,
                "sel2": sel2_r,
            }
        )
    return in_maps


def _gating(nc, m, qT, bm, Ft, ident, ps_pool, ps_tag, sb_pool):
    """Own-block top-k flags for chunk m's 4 query blocks (both heads).

    All 8 gating matmuls first, then the DVE compare chains, then the 8
    transposes batched 4-per-PSUM-bank, so the in-order PE stream never
    waits mid-chain. PSUM scratch comes from (ps_pool, ps_tag) so this can
    run late, inside the attention phase, for the last two chunks.
    """
    import concourse.mybir as mybir

    F32 = mybir.dt.float32
    F32R = mybir.dt.float32r
    pairs = [(h, i) for h in range(HPC) for i in range(4 * m, 4 * m + 4)]
    nbk = 4 * m + 4  # even N; cols > i unused
    ps_g8 = ps_pool.tile([128, 8, NB], F32, tag=ps_tag, bufs=1, name="g8")
    for p, (h, i) in enumerate(pairs):
        nc.tensor.matmul(
            ps_g8[:, p, 0:nbk],
            qT[h][:, i * 128:(i + 1) * 128],
            bm[:, 0:nbk],
            start=True,
            stop=True,
        )
    nfs = []
    for p, (h, i) in enumerate(pairs):
        cmp = sb_pool.tile([128, NB], F32, tag="cmp", bufs=2, name="cmp")
        cnt = sb_pool.tile([128, 1], F32, tag="cnt", bufs=2, name="cnt")
        nc.vector.tensor_scalar(
            out=cmp[:, 0:i],
            in0=ps_g8[:, p, 0:i],
            scalar1=ps_g8[:, p, i:i + 1],
            scalar2=None,
            op0=mybir.AluOpType.is_gt,
        )
        nc.vector.tensor_reduce(
            cnt, cmp[:, 0:i], axis=mybir.AxisListType.X, op=mybir.AluOpType.add
        )
        # notflag: 1.0 -> own block selected (keep all)
        # (fp32r tiles: values are exactly 0.0/1.0)
        nf = sb_pool.tile([128, 1], F32R, tag="nf", bufs=8, name=f"nf{p}")
        nc.vector.tensor_scalar(
            out=nf,
            in0=cnt,
            scalar1=float(TOPK) - 0.5,
            scalar2=None,
            op0=mybir.AluOpType.is_lt,
        )
        nfs.append(nf)
    for h in range(HPC):
        ps_ft4 = ps_pool.tile([1, 4, 128], F32, tag=ps_tag, bufs=1, name="ft4")
        for t in range(4):
            nc.tensor.transpose(
                ps_ft4.bitcast(F32R)[:, t, :], nfs[4 * h + t], ident
            )
        nc.vector.tensor_copy(
            Ft[:, h, (4 * m - 8) * 128:(4 * m - 4) * 128],
            ps_ft4.rearrange("o f t -> o (f t)"),
        )



def _build_nc(reps=1):
    key = f"nc{reps}"
    if key in _CACHE:
        return _CACHE[key]
    nc = bacc.Bacc(None, target_bir_lowering=False)

    xT_d = nc.dram_tensor("xT", [DIM, SEQ], F32R, kind="ExternalInput")
    wq_d = nc.dram_tensor("wq", [DIM, DPC], F32R, kind="ExternalInput")
    wk_d = nc.dram_tensor("wk", [DIM, HEAD_DIM], F32R, kind="ExternalInput")
    wv_d = nc.dram_tensor("wv", [DIM, HEAD_DIM], F32R, kind="ExternalInput")
    wo_d = nc.dram_tensor("wo", [DPC, DIM], F32R, kind="ExternalInput")
    cos_d = nc.dram_tensor("cos0", [HEAD_DIM, CH], F32, kind="ExternalInput")
    sin_d = nc.dram_tensor("sin0", [HEAD_DIM, CH], F32, kind="ExternalInput")
    rotc_d = nc.dram_tensor("rotC", [HEAD_DIM, 1], F32, kind="ExternalInput")
    rots_d = nc.dram_tensor("rotS", [HEAD_DIM, 1], F32, kind="ExternalInput")
    psw_d = nc.dram_tensor("pswap", [128, 128], F32R, kind="ExternalInput")
    idm_d = nc.dram_tensor("identm", [128, 128], F32R, kind="ExternalInput")
    trk_d = nc.dram_tensor("trikeep", [BLOCK, BLOCK], F32, kind="ExternalInput")
    onr_d = nc.dram_tensor("ones_r", [1, 128], F32R, kind="ExternalInput")
    oh2_d = nc.dram_tensor("oh2", [128, 2, 2], F32R, kind="ExternalInput")
    sel2_d = nc.dram_tensor("sel2", [2, 2, 128], F32R, kind="ExternalInput")
    out_d = nc.dram_tensor("out", [SEQ, DIM], F32, kind="ExternalOutput")

    with tile.TileContext(nc) as tc, nc.allow_low_precision(
        reason="float32r rounding of matmul operands is intentional"
    ):
      for _rep in range(reps):
        with tc.tile_pool(name="persist", bufs=1) as per:
            qT = [per.tile([128, SEQ], F32R, tag=f"qT{h}", name=f"qT{h}") for h in range(HPC)]
            kT = per.tile([128, SEQ], F32R, tag="kT")
            vN = per.tile([128, NB, 128], F32R, tag="vN")   # [s-in-tile, sk-tile, d]
            ident = per.tile([128, 128], F32R, tag="ident")
            pswap = per.tile([128, 128], F32R, tag="pswap")
            trik = per.tile([BLOCK, BLOCK], F32, tag="trik")
            ones_r = per.tile([1, 128], F32R, tag="ones_r")
            oh2 = per.tile([128, 2, 2], F32R, tag="oh2")    # [k, h, den-col]
            sel2 = per.tile([2, 2, 128], F32R, tag="sel2")  # [den-row, h, p]
            bm = per.tile([128, NB], F32R, tag="bm")
            # per-head notflag rows: Ft[0, h, (i-TOPK)*128:...] is the [1,128]
            # notflag row for query block i of head h, at base partition 0
            Ft = per.tile([1, HPC, (NB - TOPK) * 128], F32R, tag="Ft")

            # dummy exp so the Exp act-table load overlaps the initial weight
            # DMAs instead of stalling the first attention chunk
            warm = per.tile([1, 1], F32, tag="warm")
            nc.vector.memset(warm, 0.0)
            nc.scalar.activation(
                out=warm, in_=warm, func=mybir.ActivationFunctionType.Exp
            )

            # ---------------- phase 1: projections + rope -------------------
            with (
                tc.tile_pool(name="wpool", bufs=1) as wp,
                tc.tile_pool(name="xtp", bufs=17) as xtp,
                tc.tile_pool(name="ropep", bufs=2) as rp,
                tc.tile_pool(name="csin", bufs=2) as csp,
                # acc_ps declared first so its PSUM range lines up with the
                # attention score pool's range: the last acc_ps readers (rope
                # copies) finish well before the gating tail that occupies
                # pj_ps, letting chunk-0 QK matmuls start during the tail
                tc.tile_pool(name="acc_ps", bufs=4, space="PSUM") as accps,
                tc.tile_pool(name="pj_ps", bufs=2, space="PSUM") as trps,
            ):
                wq_sb = wp.tile([128, NCT, DPC], F32R, tag="wq")
                wk_sb = wp.tile([128, NCT, HEAD_DIM], F32R, tag="wk")
                wv_sb = wp.tile([128, NCT, HEAD_DIM], F32R, tag="wv")
                wq_r = wq_d.rearrange("(t p) d -> p t d", p=128)
                wk_r = wk_d.rearrange("(t p) d -> p t d", p=128)
                wv_r = wv_d.rearrange("(t p) d -> p t d", p=128)
                # k/v weight heads ride the fast SP HWDGE queue (ahead of
                # the x tiles) so chunk 0's k/v matmuls start ~2us in; the
                # later-needed tails take the slow-dispatch SWDGE queue
                nc.sync.dma_start(out=wk_sb[:, 0:4, :], in_=wk_r[:, 0:4, :])
                nc.sync.dma_start(out=wv_sb[:, 0:4, :], in_=wv_r[:, 0:4, :])
                nc.gpsimd.dma_start(out=wk_sb[:, 4:16, :], in_=wk_r[:, 4:16, :])
                nc.gpsimd.dma_start(out=wv_sb[:, 4:16, :], in_=wv_r[:, 4:16, :])
                nc.gpsimd.dma_start(out=wq_sb[:, 0:8, :], in_=wq_r[:, 0:8, :])
                nc.gpsimd.dma_start(out=wq_sb[:, 8:16, :], in_=wq_r[:, 8:16, :])

                rot_c = wp.tile([128, 1], F32, tag="rotc")
                nc.scalar.dma_start(out=rot_c, in_=rotc_d[:])
                rot_s = wp.tile([128, 1], F32, tag="rots")
                nc.scalar.dma_start(out=rot_s, in_=rots_d[:])
                gp = wp  # reuse the bufs=1 pool scope for small gating tiles
                for m in range(NCHUNK):
                    cols = slice(m * CH, (m + 1) * CH)
                    ps_q0 = accps.tile([128, CH], F32, tag="acc")
                    ps_q1 = accps.tile([128, CH], F32, tag="acc")
                    ps_k = accps.tile([128, CH], F32, tag="acc")
                    ps_v = accps.tile([128, CH], F32, tag="acc")
                    if m <= 1:
                        # two passes (k/v then q) to match the weight-arrival
                        # order; the xt tiles stay resident for the q pass
                        xts = []
                        for cc in range(NCT):
                            xt = xtp.tile([128, CH], F32R, tag="xt")
                            # first tiles ride the otherwise-idle Act HWDGE
                            # queue in parallel with the sync queue
                            q = nc.scalar if (m == 0 and cc < 4) else nc.sync
                            q.dma_start(
                                out=xt, in_=xT_d[cc * 128:(cc + 1) * 128, cols]
                            )
                            xts.append(xt)
                            st0, sp0 = (cc == 0), (cc == NCT - 1)
                            nc.tensor.matmul(ps_k, wk_sb[:, cc, :], xt, start=st0, stop=sp0)
                            nc.tensor.matmul(ps_v, wv_sb[:, cc, :], xt, start=st0, stop=sp0)
                        # consts (needed from the rope stage onwards) follow
                        # the early x tiles on the Act HWDGE queue
                        if m == 0:
                          nc.scalar.dma_start(out=pswap, in_=psw_d[:])
                          nc.scalar.dma_start(out=ident, in_=idm_d[:])
                          nc.scalar.dma_start(out=trik, in_=trk_d[:])
                          nc.scalar.dma_start(out=ones_r, in_=onr_d[:])
                          nc.scalar.dma_start(out=oh2, in_=oh2_d[:])
                          nc.scalar.dma_start(out=sel2, in_=sel2_d[:])
                        for cc in range(NCT):
                            st0, sp0 = (cc == 0), (cc == NCT - 1)
                            nc.tensor.matmul(ps_q0, wq_sb[:, cc, 0:128], xts[cc], start=st0, stop=sp0)
                            nc.tensor.matmul(ps_q1, wq_sb[:, cc, 128:256], xts[cc], start=st0, stop=sp0)
                    else:
                      for cc in range(NCT):
                        xt = xtp.tile([128, CH], F32R, tag="xt")
                        nc.sync.dma_start(
                            out=xt, in_=xT_d[cc * 128:(cc + 1) * 128, cols]
                        )
                        st0, sp0 = (cc == 0), (cc == NCT - 1)
                        nc.tensor.matmul(ps_q0, wq_sb[:, cc, 0:128], xt, start=st0, stop=sp0)
                        nc.tensor.matmul(ps_q1, wq_sb[:, cc, 128:256], xt, start=st0, stop=sp0)
                        nc.tensor.matmul(ps_k, wk_sb[:, cc, :], xt, start=st0, stop=sp0)
                        nc.tensor.matmul(ps_v, wv_sb[:, cc, :], xt, start=st0, stop=sp0)

                    if m == 0:
                        cos_t = csp.tile([128, CH], F32, tag="cos", bufs=2)
                        nc.scalar.dma_start(out=cos_t, in_=cos_d[:])
                        sin_t = csp.tile([128, CH], F32, tag="sin", bufs=2)
                        nc.scalar.dma_start(out=sin_t, in_=sin_d[:])
                    else:
                        # rotate the previous chunk's tables by CH positions
                        # (per-partition angle), off the DMA wire entirely
                        cos_p, sin_p = cos_t, sin_t
                        ta = rp.tile([128, CH], F32, tag="t2")
                        nc.vector.tensor_scalar(
                            out=ta, in0=sin_p, scalar1=rot_s, scalar2=None,
                            op0=mybir.AluOpType.mult,
                        )
                        cos_t = csp.tile([128, CH], F32, tag="cos", bufs=2)
                        nc.vector.scalar_tensor_tensor(
                            out=cos_t, in0=cos_p, scalar=rot_c, in1=ta,
                            op0=mybir.AluOpType.mult,
                            op1=mybir.AluOpType.subtract,
                        )
                        tb = rp.tile([128, CH], F32, tag="t2")
                        nc.vector.tensor_scalar(
                            out=tb, in0=cos_p, scalar1=rot_s, scalar2=None,
                            op0=mybir.AluOpType.mult,
                        )
                        sin_t = csp.tile([128, CH], F32, tag="sin", bufs=2)
                        nc.vector.scalar_tensor_tensor(
                            out=sin_t, in0=sin_p, scalar=rot_c, in1=tb,
                            op0=mybir.AluOpType.mult,
                            op1=mybir.AluOpType.add,
                        )

                    # V: evacuate then PE-transpose to natural [s, d] layout
                    # (fp32r copy: vN is fp32r anyway, and fp32r transpose runs
                    # 1.5 cyc/row vs 2.0 for fp32)
                    vtmp = rp.tile([128, CH], F32R, tag="qraw2")
                    nc.vector.tensor_copy(vtmp, ps_v)
                    ps_vt = trps.tile([128, CH], F32, tag="tr")
                    for u in range(4):
                        nc.tensor.transpose(
                            ps_vt.bitcast(F32R)[:, u * 128:(u + 1) * 128],
                            vtmp[:, u * 128:(u + 1) * 128],
                            ident,
                        )
                    nc.vector.tensor_copy(
                        vN[:, 4 * m:4 * m + 4, :],
                        ps_vt.rearrange("p (u d) -> p u d", u=4),
                    )

                    # all three raw copies first: they are the last readers
                    # of the projection PSUM banks, and freeing those banks
                    # early unblocks the next chunk's matmuls (and, for the
                    # last chunk, the attention start) via the address WAR
                    raws = []
                    for psrc in (ps_q0, ps_q1, ps_k):
                        raw = rp.tile([128, CH], F32R, tag="qraw", bufs=3)
                        nc.vector.tensor_copy(raw, psrc)
                        raws.append(raw)
                    for raw, dstT in zip(raws, (qT[0], qT[1], kT)):
                        ps_sw = trps.tile([128, CH], F32, tag="tr")
                        nc.tensor.matmul(ps_sw, pswap, raw, start=True, stop=True)
                        t2 = rp.tile([128, CH], F32, tag="t2")
                        nc.vector.tensor_tensor(
                            t2, raw.bitcast(F32), cos_t, op=mybir.AluOpType.mult
                        )
                        # sw *= sin in place (PSUM), then add -> rope output
                        nc.vector.tensor_tensor(ps_sw, ps_sw, sin_t, op=mybir.AluOpType.mult)
                        nc.vector.tensor_tensor(
                            dstT[:, cols], t2, ps_sw, op=mybir.AluOpType.add
                        )

                    # partial block sums for this chunk's 4 key blocks
                    nc.vector.tensor_reduce(
                        bm[:, 4 * m:4 * m + 4],
                        kT.bitcast(F32)[:, cols].rearrange("p (b t) -> p b t", b=4),
                        axis=mybir.AxisListType.X,
                        op=mybir.AluOpType.add,
                    )
                    # gating flags for this chunk's query blocks (needs
                    # bm 0..i); chunks 6-7 are deferred into the attention
                    # phase so the phase boundary is not serialized on them
                    if 2 <= m <= 5:
                        _gating(nc, m, qT, bm, Ft, ident, trps, "g", gp)

            # ---------------- phases 3+4 ------------------------------------
            _phase34(nc, tc, qT, kT, vN, trik, oh2, sel2, ones_r, Ft, wo_d,
                     out_d, bm, ident)

    nc.compile()
    _CACHE[key] = nc
    return nc


def _phase34(nc, tc, qT, kT, vN, trik, oh2, sel2, ones_r, Ft, wo_d, out_d,
             bm, ident):
    wop_cm = tc.tile_pool(name="wop", bufs=1)
    wop = wop_cm.__enter__()
    wo_sb = wop.tile([128, HPC, DIM], F32R, tag="wo")
    nc.gpsimd.dma_start(out=wo_sb, in_=wo_d.rearrange("(t p) d -> p t d", p=128))
    # ------- phase 3: attention with interleaved output projection -------
    # (wo(m) right after attn(m) so the 32MB output DMA spreads over the
    # whole kernel instead of piling into a DMA-bound tail phase)
    # PSUM budget (16KB/partition): pss "s" 2x[128,2,CH] = 8KB, pso "o"
    # 2x[128,CH] + "w" 1x[128,CH] = 6KB, psd "den2" 1x[2,CH] = 2KB.
    # Broadcast scratch and the final wo emission reuse the pss "s" slots.
    # ps_w gets its own tag so the deferred wo matmuls interleave freely into
    # the attention exp-latency gaps instead of queueing behind ps_o's WAR.
    with (
        tc.tile_pool(name="att", bufs=4) as ap,
        tc.tile_pool(name="attb", bufs=2) as ab,
        tc.tile_pool(name="oTs", bufs=4) as otp,
        tc.tile_pool(name="outp", bufs=6) as outp,
        tc.tile_pool(name="att_s", bufs=2, space="PSUM") as pss,
        tc.tile_pool(name="att_o", bufs=2, space="PSUM") as pso,
        tc.tile_pool(name="att_d", bufs=1, space="PSUM") as psd,
    ):
        # ---- wo(m') emission: projection for s-tiles of chunk m' ----
        # Emitted piecewise, one output tile per attention j-iteration of the
        # NEXT chunk, so the wo matmuls fill the PE's exp-latency gaps.
        def wo_pieces(mm, oTc_mm, final=False):
            for u in range(4):
                st = 4 * mm + u
                for n in range(4):
                    ncols = slice(n * 512, (n + 1) * 512)
                    if final or piece_slot[0] in ("s", "f"):
                        # score slots are idle at chunk boundaries / kernel
                        # end: a 2-deep pipelined drain instead of the
                        # single-bank "w" WAR chain
                        ps_w2 = pss.tile([128, HPC, CH], F32, tag="s", name="psw2")
                        ps_w = ps_w2[:, 0, :]
                    else:
                        ps_w = pso.tile([128, 512], F32, tag="w", name="psw", bufs=1)
                    nc.tensor.matmul(
                        ps_w,
                        oTc_mm[0][:, u * 128:(u + 1) * 128],
                        wo_sb[:, 0, ncols],
                        start=True,
                        stop=False,
                    )
                    nc.tensor.matmul(
                        ps_w,
                        oTc_mm[1][:, u * 128:(u + 1) * 128],
                        wo_sb[:, 1, ncols],
                        start=False,
                        stop=True,
                    )
                    osb = outp.tile([128, 512], F32, tag="ow")
                    # copies stay off the Act engine (at chunk boundaries an
                    # Act-side copy queues in front of the next chunk's exps,
                    # which hold the score slots and stall the QK stream) —
                    # except in the final drain, where Act is idle and
                    # alternating halves the copy-limited drain rate
                    if piece_slot[0] == "f" and (st * 4 + n) % 2 == 0:
                        nc.scalar.copy(osb, ps_w)
                    else:
                        nc.vector.tensor_copy(osb, ps_w)
                    if (st * 4 + n) % 3 == 0:
                        nc.sync.dma_start(
                            out=out_d[st * 128:(st + 1) * 128, ncols], in_=osb
                        )
                    else:
                        nc.gpsimd.dma_start(
                            out=out_d[st * 128:(st + 1) * 128, ncols], in_=osb
                        )
                    yield True

        wo_queue = []  # pending wo piece generators (FIFO across chunks)
        piece_slot = ["w"]  # PSUM scratch for the next piece ("w" or "s")

        def fire_wo(n=1):
            # at most n pieces; a second piece per attention step would stall
            # the in-order PE stream on the single-bank ps_w WAR
            while n > 0 and wo_queue:
                if next(wo_queue[0], None) is None:
                    wo_queue.pop(0)
                else:
                    n -= 1

        for m in range(NCHUNK):
            nsk = 8 if m < 2 else 4 * m + 4
            ps_o = [pso.tile([128, CH], F32, tag="o", name=f"o{h}") for h in range(HPC)]
            oTc = [
                otp.tile([128, CH], F32R, tag="oTc", name=f"oTc{h}")
                for h in range(HPC)
            ]
            # precompute diagonal-mask tiles for this chunk's band (off the
            # exp->PV critical path): one broadcast matmul + one max for both
            # heads and all 4 band blocks at once
            mks = None
            if m >= 2:
                ps_bc = pss.tile([128, HPC, CH], F32, tag="s", name="ps_bc")
                for h in range(HPC):
                    nc.tensor.matmul(
                        ps_bc[:, h, :],
                        ones_r,
                        Ft[:, h, (4 * m - 8) * 128:(4 * m - 4) * 128],
                        start=True,
                        stop=True,
                    )
                mks = ab.tile([128, HPC, CH], F32, tag="mk", bufs=2)
                trik_b = trik.rearrange("p (a b t) -> p a b t", a=1, b=1).broadcast_to(
                    [128, HPC, 4, BLOCK]
                )
                nc.vector.tensor_tensor(
                    mks.rearrange("p h (b t) -> p h b t", b=4),
                    trik_b,
                    ps_bc.rearrange("p h (b t) -> p h b t", b=4),
                    op=mybir.AluOpType.max,
                )
            ps_den = psd.tile([2, CH], F32, tag="den2", name="den2", bufs=1)

            def colspan(j):
                band = m >= 2 and j >= 4 * m
                # pad ap=128 matmuls (4 cyc/row below ap 256) to ap=256
                col0 = (j - 4 * m) * 128 if band else 0
                colp = min(col0, CH - 256) if band else 0
                return band, col0, colp

            def emit_qk_exp(j):
                band, col0, colp = colspan(j)
                ps_s = pss.tile([128, HPC, CH], F32, tag="s", bufs=2)
                for h in range(HPC):
                    nc.tensor.matmul(
                        ps_s[:, h, colp:],
                        kT[:, j * 128:(j + 1) * 128],
                        qT[h][:, m * CH + colp:(m + 1) * CH],
                        start=True,
                        stop=True,
                    )
                if colp < col0:
                    # overwrite the pad region so exp underflows to exact 0
                    nc.vector.memset(ps_s[:, :, colp:col0], PAD_NEG)
                pexp = ap.tile([128, HPC, CH], F32R, tag="pexp", bufs=4)
                nc.scalar.activation(
                    out=pexp[:, :, colp:],
                    in_=ps_s[:, :, colp:],
                    func=mybir.ActivationFunctionType.Exp,
                    scale=INV_SQRT_D,
                )
                if band:
                    nc.vector.tensor_tensor(
                        pexp[:, :, col0:col0 + 128],
                        pexp.bitcast(F32)[:, :, col0:col0 + 128],
                        mks[:, :, col0:col0 + 128],
                        op=mybir.AluOpType.mult,
                    )
                return pexp

            def emit_pv_den(j, pexp):
                _, _, colp = colspan(j)
                for h in range(HPC):
                    nc.tensor.matmul(
                        ps_o[h][:, colp:],
                        vN[:, j, :],
                        pexp[:, h, colp:],
                        start=(j == 0),
                        stop=(j == nsk - 1),
                    )
                    nc.tensor.matmul(
                        ps_den[:, colp:],
                        oh2[:, h, :],
                        pexp[:, h, colp:],
                        start=(j == 0 and h == 0),
                        stop=(j == nsk - 1 and h == HPC - 1),
                    )

            # software-pipeline by two j: QK/exp of j+1/j+2 are emitted before
            # PV/den of j, so the PE never sits in-order behind exp latency;
            # wo output tiles of the previous chunk (spread evenly over the
            # j-loop) fill the remaining slack
            pexps = [emit_qk_exp(0)]
            if nsk > 1:
                pexps.append(emit_qk_exp(1))
            for j in range(nsk):
                if j + 2 < nsk:
                    pexps.append(emit_qk_exp(j + 2))
                emit_pv_den(j, pexps[j])
                pexps[j] = None  # release reference
                # spread the ~16 pending pieces over all nsk steps (still at
                # most one per step), so late js keep their latency filler
                if len(wo_queue) > 1 or j % max(1, nsk // 16) == 0:
                    fire_wo(1)
            # drain bunched-up pieces via the (now idle) score slots so the
            # backlog overlaps the normalize chain 2-deep instead of
            # serializing on the single "w" bank
            piece_slot[0] = "s"
            fire_wo(3)
            piece_slot[0] = "w"
            # normalisation: reciprocal of the two dens, broadcast via the
            # "w" bank (keeps the "s" slots free so the next chunk's QKs can
            # start during this chain), then scale ps_o into fp32r oTc
            rec2 = ab.tile([2, CH], F32R, tag="rec")
            nc.vector.reciprocal(rec2, ps_den)
            bc_sb = ab.tile([128, HPC, CH], F32, tag="bcs")
            for h in range(HPC):
                ps_rb = pso.tile([128, 512], F32, tag="w", name="ps_rb", bufs=1)
                nc.tensor.matmul(
                    ps_rb, sel2[:, h, :], rec2, start=True, stop=True
                )
                nc.scalar.copy(bc_sb[:, h, :], ps_rb)
                nc.vector.tensor_tensor(
                    oTc[h], ps_o[h], bc_sb[:, h, :], op=mybir.AluOpType.mult
                )
            # deferred gating for the last two phase-1 chunks: the PE
            # matmuls double as filler during this chunk's boundary chain
            if m == 2:
                _gating(nc, 6, qT, bm, Ft, ident, pso, "w", ab)
            elif m == 3:
                _gating(nc, 7, qT, bm, Ft, ident, pso, "w", ab)
            wo_queue.append(
                wo_pieces(m, oTc, final=(m == NCHUNK - 1))
            )
        piece_slot[0] = "f"
        while wo_queue:
            fire_wo(1)
    wop_cm.__exit__(None, None, None)


def kernel(x, wq, wk, wv, wo):
    bs = np.asarray(x).shape[0]
    in_maps = make_in_maps(x, wq, wk, wv, wo)
    nc = _build_nc()
    res = run_bass_kernel_spmd(nc, in_maps, list(range(N_CORES)))
    out = res.results[0]["out"].astype(np.float64)
    for c in range(1, N_CORES):
        out += res.results[c]["out"]
    return out.astype(np.float32).reshape(bs, SEQ, DIM)


if __name__ == "__main__":
    rng = np.random.default_rng(0)
    xs = {
        "x": rng.standard_normal((1, SEQ, DIM), dtype=np.float32),
        "wq": rng.standard_normal((DIM, DIM), dtype=np.float32) * (DIM ** -0.5),
        "wk": rng.standard_normal((DIM, DIM // 2), dtype=np.float32) * (DIM ** -0.5),
        "wv": rng.standard_normal((DIM, DIM // 2), dtype=np.float32) * (DIM ** -0.5),
        "wo": rng.standard_normal((DIM, DIM), dtype=np.float32) * (DIM ** -0.5),
    }
    out = kernel(**xs)
    print("out", out.shape, out.dtype, np.abs(out).max())


# revision 57
# speedup vs baseline: 1.1114x; 1.0003x over previous
"""MixtureOfBlockAttention TRN2 kernel — 8-core head-parallel (TP) Bass/Tile implementation.

Semantics (verified equivalent to the reference, rel err ~2e-2 budget dominated
by top-k near-tie flips caused by fp32r rounding of x/w — irreducible without
dropping fp32r):
the reference mask `maximum(token_mask, causal*NEG_INF)` masks a position iff
it is BOTH future AND in a non-selected block. Consequences:
  - query blocks 0..7 attend to ALL tokens of key blocks 0..7 (dense, no mask);
  - query block i>=8 attends densely to key blocks 0..i-1, and within its own
    (diagonal) block applies strict causal masking ONLY for rows whose own
    block is not among their top-8 gating blocks.
Selection rank for query s in block i (i>=8): own block selected iff
  #{j < i : g[s,j] > g[s,i]} < 8, with g = q . (block sums of roped k)
(positive-scale invariant, so block sums replace means and the 1/sqrt(d)
factor is dropped).

Sharding: 16 query heads / 8 cores = 2 heads per core; KV head c serves both.
wq/wk/wv column-sliced, wo row-sliced; partial outputs summed on host.
Host-side layout prep: x is transposed to xT[c, s] (the PE contracts over the
partition dim, so both matmul operands need c on partitions) and float inputs
are pre-rounded to fp32r; both are pure data-layout transforms.

All big matmuls run in float32r (TF32-like input rounding, fp32 accumulate,
full PE rate at N>=256). Optimizations vs the original baseline (all
numerically neutral except the rope-table recurrence, which shifts the final
rel err by <1e-6 with no top-k flips):
  - per-j exp fused across the 2 heads (one Act instruction, PSUM [128,2,512]);
  - softmax denominators for both heads accumulate into one [2,512] PSUM bank
    via [128,2] one-hot-column stationaries;
  - reciprocal-broadcast and notflag-broadcast matmuls emitted at ap>=512
    instead of 4x/8x ap=128 pieces (fp32r runs 4 cyc/row below ap 256);
  - diagonal-band j=4m+3 matmuls padded from ap=128 to ap=256 with a -1e5
    PSUM memset in the pad so exp underflows to exact 0 there;
  - V-transpose and notflag-transpose in fp32r (1.5 cyc/row) not fp32 (2.0);
  - rope cos/sin tables generated on device by a per-chunk angle-rotation
    recurrence (saves 4MB/core of HBM reads and the phase-1 DMA deficit);
  - attention j-loop software-pipelined by one step (QK/exp of j+1 emitted
    before PV/den of j) so the in-order PE stream hides the exp latency;
  - the deferred wo projection is emitted as 16 per-chunk output tiles fired
    at most one per attention step from a cross-chunk FIFO, filling the PE's
    residual exp-latency slack without ever stalling it on the ps_w WAR;
  - gating reordered (all matmuls, then DVE compare chains, then batched
    transposes) and chunks 6-7's gating deferred into the attention phase;
  - PSUM: scores 2x[128,2,512] + 2 ps_o + 1 ps_w + 1 den2 = exactly 8 banks.
"""

import math
import sys

import numpy as np

if "/opt/trn_rl_repo" not in sys.path:
    sys.path.insert(0, "/opt/trn_rl_repo")

import concourse.bacc as bacc
import concourse.mybir as mybir
import concourse.tile as tile
from concourse.bass_utils import run_bass_kernel_spmd

F32 = mybir.dt.float32
F32R = mybir.dt.float32r

SEQ = 4096
DIM = 2048
HEAD_DIM = 128
N_HEADS = 16
N_CORES = 8
HPC = N_HEADS // N_CORES       # heads per core = 2
DPC = HPC * HEAD_DIM           # q/o dims per core = 256
BLOCK = 128
NB = SEQ // BLOCK              # 32 key blocks
TOPK = 8
NCHUNK = 8                     # s-chunks of 512
CH = SEQ // NCHUNK             # 512
NCT = DIM // 128               # 16 contraction tiles
INV_SQRT_D = 1.0 / math.sqrt(HEAD_DIM)
PAD_NEG = -100000.0

_CACHE = {}


def _round_fp32r(a):
    """Round fp32 to the fp32r grid (top-11-bit mantissa, round-to-nearest)."""
    a = np.ascontiguousarray(a, dtype=np.float32)
    try:
        from neuron_dtypes import static_cast_fp32_to_fp32r

        return static_cast_fp32_to_fp32r(a).view(np.float32).astype(np.float32)
    except Exception:
        u = a.view(np.uint32)
        return ((u + np.uint32(0x800)) & np.uint32(0xFFFFF000)).view(np.float32).copy()


def _host_constants():
    if "consts" in _CACHE:
        return _CACHE["consts"]
    p = np.arange(HEAD_DIM // 2, dtype=np.float64)
    inv_freq = 1.0 / (10000.0 ** (2.0 * p / HEAD_DIM))
    ang = np.arange(SEQ, dtype=np.float64)[None, :] * inv_freq[:, None]  # [64, S]
    cos = np.cos(ang).astype(np.float32)
    sin = np.sin(ang).astype(np.float32)
    cos_ds = np.ascontiguousarray(np.repeat(cos, 2, axis=0))   # [128, S]
    sin_ds = np.empty((HEAD_DIM, SEQ), dtype=np.float32)       # signed sin
    sin_ds[0::2] = -sin
    sin_ds[1::2] = sin
    # per-partition rotation by CH positions: next-chunk tables via
    # cos' = cos*C - sin_ds*S_row ; sin_ds' = sin_ds*C + cos*S_row
    # (S_row carries the sign convention of the interleaved sin_ds rows)
    inv_freq = 1.0 / (10000.0 ** (2.0 * p / HEAD_DIM))
    c512 = np.cos(CH * inv_freq)
    s512 = np.sin(CH * inv_freq)
    rotC = np.repeat(c512, 2).astype(np.float32)[:, None]      # [128, 1]
    rotS = np.empty((HEAD_DIM,), dtype=np.float64)
    rotS[0::2] = -s512
    rotS[1::2] = s512
    rotS = rotS.astype(np.float32)[:, None]                    # [128, 1]
    pswap = np.zeros((128, 128), dtype=np.float32)             # swap 2p <-> 2p+1
    idx = np.arange(128)
    pswap[idx, idx ^ 1] = 1.0
    identm = np.eye(128, dtype=np.float32)
    r = np.arange(BLOCK)
    trikeep = (r[:, None] <= r[None, :]).astype(np.float32)    # keep iff sk <= sq
    ones_row = np.ones((1, 128), dtype=np.float32)
    # one-hot-column stationaries for per-head den accumulation into [2, CH]:
    # oh2[:, h, :] is [128, 2] with column h all-ones
    oh2 = np.zeros((128, 2, 2), dtype=np.float32)
    oh2[:, 0, 0] = 1.0
    oh2[:, 1, 1] = 1.0
    # one-hot-row stationaries for per-head [2,CH] -> [128,CH] broadcast:
    # sel2[:, h, :] is [2, 128] with row h all-ones
    sel2 = np.zeros((2, 2, 128), dtype=np.float32)
    sel2[0, 0, :] = 1.0
    sel2[1, 1, :] = 1.0
    _CACHE["consts"] = (cos_ds, sin_ds, rotC, rotS, pswap, identm, trikeep, ones_row, oh2, sel2)
    return _CACHE["consts"]


def make_in_maps(x, wq, wk, wv, wo):
    """Shard + lay out the full inputs for the 8 cores."""
    x2 = np.asarray(x, dtype=np.float32).reshape(SEQ, DIM)
    xT = _round_fp32r(np.ascontiguousarray(x2.T))
    wq = np.asarray(wq, dtype=np.float32)
    wk = np.asarray(wk, dtype=np.float32)
    wv = np.asarray(wv, dtype=np.float32)
    wo = np.asarray(wo, dtype=np.float32)
    cos_ds, sin_ds, rotC, rotS, pswap, identm, trikeep, ones_row, oh2, sel2 = _host_constants()
    pswap_r = _round_fp32r(pswap)
    ones_row_r = _round_fp32r(ones_row)
    oh2_r = _round_fp32r(oh2)
    sel2_r = _round_fp32r(sel2)
    in_maps = []
    for c in range(N_CORES):
        in_maps.append(
            {
                "xT": xT,
                "wq": _round_fp32r(wq[:, c * DPC:(c + 1) * DPC]),
                "wk": _round_fp32r(wk[:, c * HEAD_DIM:(c + 1) * HEAD_DIM]),
                "wv": _round_fp32r(wv[:, c * HEAD_DIM:(c + 1) * HEAD_DIM]),
                "wo": _round_fp32r(wo[c * DPC:(c + 1) * DPC, :]),
                "cos0": np.ascontiguousarray(cos_ds[:, 0:CH]),
                "sin0": np.ascontiguousarray(sin_ds[:, 0:CH]),
                "rotC": rotC,
                "rotS": rotS,
                "pswap": pswap_r,
                "identm": _round_fp32r(identm),
                "trikeep": trikeep,
                "ones_r": ones_row_r,
                "oh2": oh2_r,
                "sel2": sel2_r,
            }
        )
    return in_maps


def _gating(nc, m, qT, bm, Ft, ident, ps_pool, ps_tag, sb_pool):
    """Own-block top-k flags for chunk m's 4 query blocks (both heads).

    All 8 gating matmuls first, then the DVE compare chains, then the 8
    transposes batched 4-per-PSUM-bank, so the in-order PE stream never
    waits mid-chain. PSUM scratch comes from (ps_pool, ps_tag) so this can
    run late, inside the attention phase, for the last two chunks.
    """
    import concourse.mybir as mybir

    F32 = mybir.dt.float32
    F32R = mybir.dt.float32r
    pairs = [(h, i) for h in range(HPC) for i in range(4 * m, 4 * m + 4)]
    nbk = 4 * m + 4  # even N; cols > i unused
    ps_g8 = ps_pool.tile([128, 8, NB], F32, tag=ps_tag, bufs=1, name="g8")
    for p, (h, i) in enumerate(pairs):
        nc.tensor.matmul(
            ps_g8[:, p, 0:nbk],
            qT[h][:, i * 128:(i + 1) * 128],
            bm[:, 0:nbk],
            start=True,
            stop=True,
        )
    nfs = []
    for p, (h, i) in enumerate(pairs):
        cmp = sb_pool.tile([128, NB], F32, tag="cmp", bufs=2, name="cmp")
        cnt = sb_pool.tile([128, 1], F32, tag="cnt", bufs=2, name="cnt")
        nc.vector.tensor_scalar(
            out=cmp[:, 0:i],
            in0=ps_g8[:, p, 0:i],
            scalar1=ps_g8[:, p, i:i + 1],
            scalar2=None,
            op0=mybir.AluOpType.is_gt,
        )
        nc.vector.tensor_reduce(
            cnt, cmp[:, 0:i], axis=mybir.AxisListType.X, op=mybir.AluOpType.add
        )
        # notflag: 1.0 -> own block selected (keep all)
        # (fp32r tiles: values are exactly 0.0/1.0)
        nf = sb_pool.tile([128, 1], F32R, tag="nf", bufs=8, name=f"nf{p}")
        nc.vector.tensor_scalar(
            out=nf,
            in0=cnt,
            scalar1=float(TOPK) - 0.5,
            scalar2=None,
            op0=mybir.AluOpType.is_lt,
        )
        nfs.append(nf)
    for h in range(HPC):
        ps_ft4 = ps_pool.tile([1, 4, 128], F32, tag=ps_tag, bufs=1, name="ft4")
        for t in range(4):
            nc.tensor.transpose(
                ps_ft4.bitcast(F32R)[:, t, :], nfs[4 * h + t], ident
            )
        nc.vector.tensor_copy(
            Ft[:, h, (4 * m - 8) * 128:(4 * m - 4) * 128],
            ps_ft4.rearrange("o f t -> o (f t)"),
        )



def _build_nc(reps=1):
    key = f"nc{reps}"
    if key in _CACHE:
        return _CACHE[key]
    nc = bacc.Bacc(None, target_bir_lowering=False)

    xT_d = nc.dram_tensor("xT", [DIM, SEQ], F32R, kind="ExternalInput")
    wq_d = nc.dram_tensor("wq", [DIM, DPC], F32R, kind="ExternalInput")
    wk_d = nc.dram_tensor("wk", [DIM, HEAD_DIM], F32R, kind="ExternalInput")
    wv_d = nc.dram_tensor("wv", [DIM, HEAD_DIM], F32R, kind="ExternalInput")
    wo_d = nc.dram_tensor("wo", [DPC, DIM], F32R, kind="ExternalInput")
    cos_d = nc.dram_tensor("cos0", [HEAD_DIM, CH], F32, kind="ExternalInput")
    sin_d = nc.dram_tensor("sin0", [HEAD_DIM, CH], F32, kind="ExternalInput")
    rotc_d = nc.dram_tensor("rotC", [HEAD_DIM, 1], F32, kind="ExternalInput")
    rots_d = nc.dram_tensor("rotS", [HEAD_DIM, 1], F32, kind="ExternalInput")
    psw_d = nc.dram_tensor("pswap", [128, 128], F32R, kind="ExternalInput")
    idm_d = nc.dram_tensor("identm", [128, 128], F32R, kind="ExternalInput")
    trk_d = nc.dram_tensor("trikeep", [BLOCK, BLOCK], F32, kind="ExternalInput")
    onr_d = nc.dram_tensor("ones_r", [1, 128], F32R, kind="ExternalInput")
    oh2_d = nc.dram_tensor("oh2", [128, 2, 2], F32R, kind="ExternalInput")
    sel2_d = nc.dram_tensor("sel2", [2, 2, 128], F32R, kind="ExternalInput")
    out_d = nc.dram_tensor("out", [SEQ, DIM], F32, kind="ExternalOutput")

    with tile.TileContext(nc) as tc, nc.allow_low_precision(
        reason="float32r rounding of matmul operands is intentional"
    ):
      for _rep in range(reps):
        with tc.tile_pool(name="persist", bufs=1) as per:
            qT = [per.tile([128, SEQ], F32R, tag=f"qT{h}", name=f"qT{h}") for h in range(HPC)]
            kT = per.tile([128, SEQ], F32R, tag="kT")
            vN = per.tile([128, NB, 128], F32R, tag="vN")   # [s-in-tile, sk-tile, d]
            ident = per.tile([128, 128], F32R, tag="ident")
            pswap = per.tile([128, 128], F32R, tag="pswap")
            trik = per.tile([BLOCK, BLOCK], F32, tag="trik")
            ones_r = per.tile([1, 128], F32R, tag="ones_r")
            oh2 = per.tile([128, 2, 2], F32R, tag="oh2")    # [k, h, den-col]
            sel2 = per.tile([2, 2, 128], F32R, tag="sel2")  # [den-row, h, p]
            bm = per.tile([128, NB], F32R, tag="bm")
            # per-head notflag rows: Ft[0, h, (i-TOPK)*128:...] is the [1,128]
            # notflag row for query block i of head h, at base partition 0
            Ft = per.tile([1, HPC, (NB - TOPK) * 128], F32R, tag="Ft")

            # dummy exp so the Exp act-table load overlaps the initial weight
            # DMAs instead of stalling the first attention chunk
            warm = per.tile([1, 1], F32, tag="warm")
            nc.vector.memset(warm, 0.0)
            nc.scalar.activation(
                out=warm, in_=warm, func=mybir.ActivationFunctionType.Exp
            )

            # ---------------- phase 1: projections + rope -------------------
            with (
                tc.tile_pool(name="wpool", bufs=1) as wp,
                tc.tile_pool(name="xtp", bufs=17) as xtp,
                tc.tile_pool(name="ropep", bufs=2) as rp,
                tc.tile_pool(name="csin", bufs=2) as csp,
                # acc_ps declared first so its PSUM range lines up with the
                # attention score pool's range: the last acc_ps readers (rope
                # copies) finish well before the gating tail that occupies
                # pj_ps, letting chunk-0 QK matmuls start during the tail
                tc.tile_pool(name="acc_ps", bufs=4, space="PSUM") as accps,
                tc.tile_pool(name="pj_ps", bufs=2, space="PSUM") as trps,
            ):
                wq_sb = wp.tile([128, NCT, DPC], F32R, tag="wq")
                wk_sb = wp.tile([128, NCT, HEAD_DIM], F32R, tag="wk")
                wv_sb = wp.tile([128, NCT, HEAD_DIM], F32R, tag="wv")
                wq_r = wq_d.rearrange("(t p) d -> p t d", p=128)
                wk_r = wk_d.rearrange("(t p) d -> p t d", p=128)
                wv_r = wv_d.rearrange("(t p) d -> p t d", p=128)
                # k/v weight heads ride the fast SP HWDGE queue (ahead of
                # the x tiles) so chunk 0's k/v matmuls start ~2us in; the
                # later-needed tails take the slow-dispatch SWDGE queue
                nc.sync.dma_start(out=wk_sb[:, 0:4, :], in_=wk_r[:, 0:4, :])
                nc.sync.dma_start(out=wv_sb[:, 0:4, :], in_=wv_r[:, 0:4, :])
                nc.gpsimd.dma_start(out=wk_sb[:, 4:16, :], in_=wk_r[:, 4:16, :])
                nc.gpsimd.dma_start(out=wv_sb[:, 4:16, :], in_=wv_r[:, 4:16, :])
                nc.gpsimd.dma_start(out=wq_sb[:, 0:8, :], in_=wq_r[:, 0:8, :])
                nc.gpsimd.dma_start(out=wq_sb[:, 8:16, :], in_=wq_r[:, 8:16, :])

                rot_c = wp.tile([128, 1], F32, tag="rotc")
                nc.scalar.dma_start(out=rot_c, in_=rotc_d[:])
                rot_s = wp.tile([128, 1], F32, tag="rots")
                nc.scalar.dma_start(out=rot_s, in_=rots_d[:])
                gp = wp  # reuse the bufs=1 pool scope for small gating tiles
                for m in range(NCHUNK):
                    cols = slice(m * CH, (m + 1) * CH)
                    ps_q0 = accps.tile([128, CH], F32, tag="acc")
                    ps_q1 = accps.tile([128, CH], F32, tag="acc")
                    ps_k = accps.tile([128, CH], F32, tag="acc")
                    ps_v = accps.tile([128, CH], F32, tag="acc")
                    if m <= 1:
                        # two passes (k/v then q) to match the weight-arrival
                        # order; the xt tiles stay resident for the q pass
                        xts = []
                        for cc in range(NCT):
                            xt = xtp.tile([128, CH], F32R, tag="xt")
                            # first tiles ride the otherwise-idle Act HWDGE
                            # queue in parallel with the sync queue
                            q = nc.scalar if (m == 0 and cc < 4) else nc.sync
                            q.dma_start(
                                out=xt, in_=xT_d[cc * 128:(cc + 1) * 128, cols]
                            )
                            xts.append(xt)
                            st0, sp0 = (cc == 0), (cc == NCT - 1)
                            nc.tensor.matmul(ps_k, wk_sb[:, cc, :], xt, start=st0, stop=sp0)
                            nc.tensor.matmul(ps_v, wv_sb[:, cc, :], xt, start=st0, stop=sp0)
                        # consts (needed from the rope stage onwards) follow
                        # the early x tiles on the Act HWDGE queue
                        if m == 0:
                          nc.scalar.dma_start(out=pswap, in_=psw_d[:])
                          nc.scalar.dma_start(out=ident, in_=idm_d[:])
                          nc.scalar.dma_start(out=trik, in_=trk_d[:])
                          nc.scalar.dma_start(out=ones_r, in_=onr_d[:])
                          nc.scalar.dma_start(out=oh2, in_=oh2_d[:])
                          nc.scalar.dma_start(out=sel2, in_=sel2_d[:])
                        for cc in range(NCT):
                            st0, sp0 = (cc == 0), (cc == NCT - 1)
                            nc.tensor.matmul(ps_q0, wq_sb[:, cc, 0:128], xts[cc], start=st0, stop=sp0)
                            nc.tensor.matmul(ps_q1, wq_sb[:, cc, 128:256], xts[cc], start=st0, stop=sp0)
                    else:
                      for cc in range(NCT):
                        xt = xtp.tile([128, CH], F32R, tag="xt")
                        nc.sync.dma_start(
                            out=xt, in_=xT_d[cc * 128:(cc + 1) * 128, cols]
                        )
                        st0, sp0 = (cc == 0), (cc == NCT - 1)
                        nc.tensor.matmul(ps_q0, wq_sb[:, cc, 0:128], xt, start=st0, stop=sp0)
                        nc.tensor.matmul(ps_q1, wq_sb[:, cc, 128:256], xt, start=st0, stop=sp0)
                        nc.tensor.matmul(ps_k, wk_sb[:, cc, :], xt, start=st0, stop=sp0)
                        nc.tensor.matmul(ps_v, wv_sb[:, cc, :], xt, start=st0, stop=sp0)

                    if m == 0:
                        cos_t = csp.tile([128, CH], F32, tag="cos", bufs=2)
                        nc.scalar.dma_start(out=cos_t, in_=cos_d[:])
                        sin_t = csp.tile([128, CH], F32, tag="sin", bufs=2)
                        nc.scalar.dma_start(out=sin_t, in_=sin_d[:])
                    else:
                        # rotate the previous chunk's tables by CH positions
                        # (per-partition angle), off the DMA wire entirely
                        cos_p, sin_p = cos_t, sin_t
                        ta = rp.tile([128, CH], F32, tag="t2")
                        nc.vector.tensor_scalar(
                            out=ta, in0=sin_p, scalar1=rot_s, scalar2=None,
                            op0=mybir.AluOpType.mult,
                        )
                        cos_t = csp.tile([128, CH], F32, tag="cos", bufs=2)
                        nc.vector.scalar_tensor_tensor(
                            out=cos_t, in0=cos_p, scalar=rot_c, in1=ta,
                            op0=mybir.AluOpType.mult,
                            op1=mybir.AluOpType.subtract,
                        )
                        tb = rp.tile([128, CH], F32, tag="t2")
                        nc.vector.tensor_scalar(
                            out=tb, in0=cos_p, scalar1=rot_s, scalar2=None,
                            op0=mybir.AluOpType.mult,
                        )
                        sin_t = csp.tile([128, CH], F32, tag="sin", bufs=2)
                        nc.vector.scalar_tensor_tensor(
                            out=sin_t, in0=sin_p, scalar=rot_c, in1=tb,
                            op0=mybir.AluOpType.mult,
                            op1=mybir.AluOpType.add,
                        )

                    # V: evacuate then PE-transpose to natural [s, d] layout
                    # (fp32r copy: vN is fp32r anyway, and fp32r transpose runs
                    # 1.5 cyc/row vs 2.0 for fp32)
                    vtmp = rp.tile([128, CH], F32R, tag="qraw2")
                    nc.vector.tensor_copy(vtmp, ps_v)
                    ps_vt = trps.tile([128, CH], F32, tag="tr")
                    for u in range(4):
                        nc.tensor.transpose(
                            ps_vt.bitcast(F32R)[:, u * 128:(u + 1) * 128],
                            vtmp[:, u * 128:(u + 1) * 128],
                            ident,
                        )
                    nc.vector.tensor_copy(
                        vN[:, 4 * m:4 * m + 4, :],
                        ps_vt.rearrange("p (u d) -> p u d", u=4),
                    )

                    # all three raw copies first: they are the last readers
                    # of the projection PSUM banks, and freeing those banks
                    # early unblocks the next chunk's matmuls (and, for the
                    # last chunk, the attention start) via the address WAR
                    raws = []
                    for psrc in (ps_q0, ps_q1, ps_k):
                        raw = rp.tile([128, CH], F32R, tag="qraw", bufs=3)
                        nc.vector.tensor_copy(raw, psrc)
                        raws.append(raw)
                    for raw, dstT in zip(raws, (qT[0], qT[1], kT)):
                        ps_sw = trps.tile([128, CH], F32, tag="tr")
                        nc.tensor.matmul(ps_sw, pswap, raw, start=True, stop=True)
                        t2 = rp.tile([128, CH], F32, tag="t2")
                        nc.vector.tensor_tensor(
                            t2, raw.bitcast(F32), cos_t, op=mybir.AluOpType.mult
                        )
                        # sw *= sin in place (PSUM), then add -> rope output
                        nc.vector.tensor_tensor(ps_sw, ps_sw, sin_t, op=mybir.AluOpType.mult)
                        nc.vector.tensor_tensor(
                            dstT[:, cols], t2, ps_sw, op=mybir.AluOpType.add
                        )

                    # partial block sums for this chunk's 4 key blocks
                    nc.vector.tensor_reduce(
                        bm[:, 4 * m:4 * m + 4],
                        kT.bitcast(F32)[:, cols].rearrange("p (b t) -> p b t", b=4),
                        axis=mybir.AxisListType.X,
                        op=mybir.AluOpType.add,
                    )
                    # gating flags for this chunk's query blocks (needs
                    # bm 0..i); chunks 6-7 are deferred into the attention
                    # phase so the phase boundary is not serialized on them
                    if 2 <= m <= 5:
                        _gating(nc, m, qT, bm, Ft, ident, trps, "g", gp)

            # ---------------- phases 3+4 ------------------------------------
            _phase34(nc, tc, qT, kT, vN, trik, oh2, sel2, ones_r, Ft, wo_d,
                     out_d, bm, ident)

    nc.compile()
    _CACHE[key] = nc
    return nc


def _phase34(nc, tc, qT, kT, vN, trik, oh2, sel2, ones_r, Ft, wo_d, out_d,
             bm, ident):
    wop_cm = tc.tile_pool(name="wop", bufs=1)
    wop = wop_cm.__enter__()
    wo_sb = wop.tile([128, HPC, DIM], F32R, tag="wo")
    nc.gpsimd.dma_start(out=wo_sb, in_=wo_d.rearrange("(t p) d -> p t d", p=128))
    # ------- phase 3: attention with interleaved output projection -------
    # (wo(m) right after attn(m) so the 32MB output DMA spreads over the
    # whole kernel instead of piling into a DMA-bound tail phase)
    # PSUM budget (16KB/partition): pss "s" 2x[128,2,CH] = 8KB, pso "o"
    # 2x[128,CH] + "w" 1x[128,CH] = 6KB, psd "den2" 1x[2,CH] = 2KB.
    # Broadcast scratch and the final wo emission reuse the pss "s" slots.
    # ps_w gets its own tag so the deferred wo matmuls interleave freely into
    # the attention exp-latency gaps instead of queueing behind ps_o's WAR.
    with (
        tc.tile_pool(name="att", bufs=4) as ap,
        tc.tile_pool(name="attb", bufs=2) as ab,
        tc.tile_pool(name="oTs", bufs=4) as otp,
        tc.tile_pool(name="outp", bufs=6) as outp,
        tc.tile_pool(name="att_s", bufs=2, space="PSUM") as pss,
        tc.tile_pool(name="att_o", bufs=2, space="PSUM") as pso,
        tc.tile_pool(name="att_d", bufs=1, space="PSUM") as psd,
    ):
        # ---- wo(m') emission: projection for s-tiles of chunk m' ----
        # Emitted piecewise, one output tile per attention j-iteration of the
        # NEXT chunk, so the wo matmuls fill the PE's exp-latency gaps.
        def wo_pieces(mm, oTc_mm, final=False):
            for u in range(4):
                st = 4 * mm + u
                for n in range(4):
                    ncols = slice(n * 512, (n + 1) * 512)
                    if final or piece_slot[0] in ("s", "f"):
                        # score slots are idle at chunk boundaries / kernel
                        # end: a 2-deep pipelined drain instead of the
                        # single-bank "w" WAR chain
                        ps_w2 = pss.tile([128, HPC, CH], F32, tag="s", name="psw2")
                        ps_w = ps_w2[:, 0, :]
                    else:
                        ps_w = pso.tile([128, 512], F32, tag="w", name="psw", bufs=1)
                    nc.tensor.matmul(
                        ps_w,
                        oTc_mm[0][:, u * 128:(u + 1) * 128],
                        wo_sb[:, 0, ncols],
                        start=True,
                        stop=False,
                    )
                    nc.tensor.matmul(
                        ps_w,
                        oTc_mm[1][:, u * 128:(u + 1) * 128],
                        wo_sb[:, 1, ncols],
                        start=False,
                        stop=True,
                    )
                    osb = outp.tile([128, 512], F32, tag="ow")
                    # copies stay off the Act engine (at chunk boundaries an
                    # Act-side copy queues in front of the next chunk's exps,
                    # which hold the score slots and stall the QK stream) —
                    # except in the final drain, where Act is idle and
                    # alternating halves the copy-limited drain rate
                    if piece_slot[0] == "f" and (st * 4 + n) % 2 == 0:
                        nc.scalar.copy(osb, ps_w)
                    else:
                        nc.vector.tensor_copy(osb, ps_w)
                    if (st * 4 + n) % 3 == 0:
                        nc.sync.dma_start(
                            out=out_d[st * 128:(st + 1) * 128, ncols], in_=osb
                        )
                    else:
                        nc.gpsimd.dma_start(
                            out=out_d[st * 128:(st + 1) * 128, ncols], in_=osb
                        )
                    yield True

        wo_queue = []  # pending wo piece generators (FIFO across chunks)
        piece_slot = ["w"]  # PSUM scratch for the next piece ("w" or "s")

        def fire_wo(n=1):
            # at most n pieces; a second piece per attention step would stall
            # the in-order PE stream on the single-bank ps_w WAR
            while n > 0 and wo_queue:
                if next(wo_queue[0], None) is None:
                    wo_queue.pop(0)
                else:
                    n -= 1

        for m in range(NCHUNK):
            nsk = 8 if m < 2 else 4 * m + 4
            ps_o = [pso.tile([128, CH], F32, tag="o", name=f"o{h}") for h in range(HPC)]
            oTc = [
                otp.tile([128, CH], F32R, tag="oTc", name=f"oTc{h}")
                for h in range(HPC)
            ]
            # precompute diagonal-mask tiles for this chunk's band (off the
            # exp->PV critical path): one broadcast matmul + one max for both
            # heads and all 4 band blocks at once
            mks = None
            if m >= 2:
                ps_bc = pss.tile([128, HPC, CH], F32, tag="s", name="ps_bc")
                for h in range(HPC):
                    nc.tensor.matmul(
                        ps_bc[:, h, :],
                        ones_r,
                        Ft[:, h, (4 * m - 8) * 128:(4 * m - 4) * 128],
                        start=True,
                        stop=True,
                    )
                mks = ab.tile([128, HPC, CH], F32, tag="mk", bufs=2)
                trik_b = trik.rearrange("p (a b t) -> p a b t", a=1, b=1).broadcast_to(
                    [128, HPC, 4, BLOCK]
                )
                nc.vector.tensor_tensor(
                    mks.rearrange("p h (b t) -> p h b t", b=4),
                    trik_b,
                    ps_bc.rearrange("p h (b t) -> p h b t", b=4),
                    op=mybir.AluOpType.max,
                )
            ps_den = psd.tile([2, CH], F32, tag="den2", name="den2", bufs=1)

            def colspan(j):
                band = m >= 2 and j >= 4 * m
                # pad ap=128 matmuls (4 cyc/row below ap 256) to ap=256
                col0 = (j - 4 * m) * 128 if band else 0
                colp = min(col0, CH - 256) if band else 0
                return band, col0, colp

            def emit_qk_exp(j):
                band, col0, colp = colspan(j)
                ps_s = pss.tile([128, HPC, CH], F32, tag="s", bufs=2)
                for h in range(HPC):
                    nc.tensor.matmul(
                        ps_s[:, h, colp:],
                        kT[:, j * 128:(j + 1) * 128],
                        qT[h][:, m * CH + colp:(m + 1) * CH],
                        start=True,
                        stop=True,
                    )
                if colp < col0:
                    # overwrite the pad region so exp underflows to exact 0
                    nc.vector.memset(ps_s[:, :, colp:col0], PAD_NEG)
                pexp = ap.tile([128, HPC, CH], F32R, tag="pexp", bufs=5)
                nc.scalar.activation(
                    out=pexp[:, :, colp:],
                    in_=ps_s[:, :, colp:],
                    func=mybir.ActivationFunctionType.Exp,
                    scale=INV_SQRT_D,
                )
                if band:
                    nc.vector.tensor_tensor(
                        pexp[:, :, col0:col0 + 128],
                        pexp.bitcast(F32)[:, :, col0:col0 + 128],
                        mks[:, :, col0:col0 + 128],
                        op=mybir.AluOpType.mult,
                    )
                return pexp

            def emit_pv_den(j, pexp):
                _, _, colp = colspan(j)
                for h in range(HPC):
                    nc.tensor.matmul(
                        ps_o[h][:, colp:],
                        vN[:, j, :],
                        pexp[:, h, colp:],
                        start=(j == 0),
                        stop=(j == nsk - 1),
                    )
                    nc.tensor.matmul(
                        ps_den[:, colp:],
                        oh2[:, h, :],
                        pexp[:, h, colp:],
                        start=(j == 0 and h == 0),
                        stop=(j == nsk - 1 and h == HPC - 1),
                    )

            # software-pipeline by two j: QK/exp of j+1/j+2 are emitted before
            # PV/den of j, so the PE never sits in-order behind exp latency;
            # wo output tiles of the previous chunk (spread evenly over the
            # j-loop) fill the remaining slack
            pexps = [emit_qk_exp(0)]
            if nsk > 1:
                pexps.append(emit_qk_exp(1))
            for j in range(nsk):
                if j + 2 < nsk:
                    pexps.append(emit_qk_exp(j + 2))
                emit_pv_den(j, pexps[j])
                pexps[j] = None  # release reference
                # spread the ~16 pending pieces over all nsk steps (still at
                # most one per step), so late js keep their latency filler
                if len(wo_queue) > 1 or j % max(1, nsk // 16) == 0:
                    fire_wo(1)
            # drain bunched-up pieces via the (now idle) score slots so the
            # backlog overlaps the normalize chain 2-deep instead of
            # serializing on the single "w" bank
            piece_slot[0] = "s"
            fire_wo(3)
            piece_slot[0] = "w"
            # normalisation: reciprocal of the two dens, broadcast via the
            # "w" bank (keeps the "s" slots free so the next chunk's QKs can
            # start during this chain), then scale ps_o into fp32r oTc
            rec2 = ab.tile([2, CH], F32R, tag="rec")
            nc.vector.reciprocal(rec2, ps_den)
            bc_sb = ab.tile([128, HPC, CH], F32, tag="bcs")
            for h in range(HPC):
                ps_rb = pso.tile([128, 512], F32, tag="w", name="ps_rb", bufs=1)
                nc.tensor.matmul(
                    ps_rb, sel2[:, h, :], rec2, start=True, stop=True
                )
                nc.scalar.copy(bc_sb[:, h, :], ps_rb)
                nc.vector.tensor_tensor(
                    oTc[h], ps_o[h], bc_sb[:, h, :], op=mybir.AluOpType.mult
                )
            # deferred gating for the last two phase-1 chunks: the PE
            # matmuls double as filler during this chunk's boundary chain
            if m == 2:
                _gating(nc, 6, qT, bm, Ft, ident, pso, "w", ab)
            elif m == 3:
                _gating(nc, 7, qT, bm, Ft, ident, pso, "w", ab)
            wo_queue.append(
                wo_pieces(m, oTc, final=(m == NCHUNK - 1))
            )
        piece_slot[0] = "f"
        while wo_queue:
            fire_wo(1)
    wop_cm.__exit__(None, None, None)


def kernel(x, wq, wk, wv, wo):
    bs = np.asarray(x).shape[0]
    in_maps = make_in_maps(x, wq, wk, wv, wo)
    nc = _build_nc()
    res = run_bass_kernel_spmd(nc, in_maps, list(range(N_CORES)))
    out = res.results[0]["out"].astype(np.float64)
    for c in range(1, N_CORES):
        out += res.results[c]["out"]
    return out.astype(np.float32).reshape(bs, SEQ, DIM)


if __name__ == "__main__":
    rng = np.random.default_rng(0)
    xs = {
        "x": rng.standard_normal((1, SEQ, DIM), dtype=np.float32),
        "wq": rng.standard_normal((DIM, DIM), dtype=np.float32) * (DIM ** -0.5),
        "wk": rng.standard_normal((DIM, DIM // 2), dtype=np.float32) * (DIM ** -0.5),
        "wv": rng.standard_normal((DIM, DIM // 2), dtype=np.float32) * (DIM ** -0.5),
        "wo": rng.standard_normal((DIM, DIM), dtype=np.float32) * (DIM ** -0.5),
    }
    out = kernel(**xs)
    print("out", out.shape, out.dtype, np.abs(out).max())
